# revision 1
# baseline (speedup 1.0000x reference)
"""Sparse attention (ProbSparse-style) Trainium2 Bass kernel.

Problem (per batch element b, data-parallel over 8 NeuronCores):
  Q = x @ Wq.T ; K = x @ Wk.T ; V = x @ Wv.T            [L=2048, D=512]
  QK_sample[l,s] = Q[l] . K[index_sample[l,s]]           [L, 40]
  M[l] = max_s QK_sample - sum_s QK_sample / L
  sel = top40(M)  (as a set; the reference scatter makes order irrelevant)
  scores = Q[sel] @ K.T / sqrt(D); attn = softmax(scores)
  ctx = broadcast(mean(V)); ctx[sel] = attn @ V

Numerics strategy (top-40 boundary gaps are as small as 0.02 in M):
  - K and V are computed with a 3-term bf16x2 split matmul
    (xh*wh + xl*wh + xh*wl, host-split halves) -> ~1e-5 absolute error,
    fp32-class, at full bf16 PE rate.
  - Approx M for ALL rows uses bf16 Q and bf16 K (error sigma ~0.2),
    extracted from per-chunk S = Q K^T PSUM blocks with fused
    tensor_tensor_reduce against a shipped u8 sample mask
    (multiply-mask max is safe: sampled max > 0 w.p. 1-2^-40;
    dup-count correction is deferred to the exact stage).
  - Candidates = { M_approx >= approx-top40 - DELTA }, DELTA=1.5 covers
    ~8 sigma; measured rank-40 to rank-64 M gap is 2.5-4.8 so the
    candidate count stays well under the 128-slot budget.
  - Exact stage on <= 128 candidate rows: gather x rows from DRAM
    (indirect DMA), exact fp32 Q_cand, exact S_cand vs the fp32-class K,
    TTR with gathered u8 mask+count rows -> exact M_cand -> exact top-40
    threshold -> softmax over S_cand -> upd = attn @ V -> indirect
    scatter of the 40 selected rows into ctx (bounds_check skips the
    rest).

kernel(**inputs) accepts the FULL inputs and returns the FULL
[8, 2048, 512] f32 output; batch is sharded over 8 cores.
"""

import math

import numpy as np
import ml_dtypes

import concourse.bacc as bacc
import concourse.bass as bass
import concourse.mybir as mybir
import concourse.tile as tile
from concourse.bass_utils import run_bass_kernel_spmd
from concourse.masks import make_identity

P = 128
L = 2048
D = 512
B = 8
NL = L // P        # 16 query chunks
ND = D // P        # 4 feature chunks
NJ = L // 512      # 4 key blocks of 512
NT = 40
SCALE = 1.0 / math.sqrt(D)
DELTA = 1.5        # candidate band below approx T40
NEG = -3.0e38
SKIP_IDX = 99999.0  # scatter index sentinel (> bounds_check -> row skipped)

f32 = mybir.dt.float32
bf16 = mybir.dt.bfloat16
u8 = mybir.dt.uint8
i32 = mybir.dt.int32
u32 = mybir.dt.uint32
AX = mybir.AxisListType
OP = mybir.AluOpType
ACTF = mybir.ActivationFunctionType


def build():
    nc = bacc.Bacc("TRN2", target_bir_lowering=False)

    x_d = nc.dram_tensor("x_nat", [L, D], f32, kind="ExternalInput")
    xth_d = nc.dram_tensor("xTh", [D, L], bf16, kind="ExternalInput")
    xtl_d = nc.dram_tensor("xTl", [D, L], bf16, kind="ExternalInput")
    xmh_d = nc.dram_tensor("xmeanTh", [D, 1], bf16, kind="ExternalInput")
    xml_d = nc.dram_tensor("xmeanTl", [D, 1], bf16, kind="ExternalInput")
    wqh_d = nc.dram_tensor("wqTh", [D, D], bf16, kind="ExternalInput")
    wkh_d = nc.dram_tensor("wkTh", [D, D], bf16, kind="ExternalInput")
    wkl_d = nc.dram_tensor("wkTl", [D, D], bf16, kind="ExternalInput")
    wvh_d = nc.dram_tensor("wvTh", [D, D], bf16, kind="ExternalInput")
    wvl_d = nc.dram_tensor("wvTl", [D, D], bf16, kind="ExternalInput")
    wq_d = nc.dram_tensor("wqT", [D, D], f32, kind="ExternalInput")
    perm_d = nc.dram_tensor("perm16", [16, 8 * P], f32, kind="ExternalInput")
    mask_d = nc.dram_tensor("mask01", [L, L], u8, kind="ExternalInput")
    cnt_d = nc.dram_tensor("countf", [L, L], u8, kind="ExternalInput")
    ctx_d = nc.dram_tensor("ctx", [L, D], f32, kind="ExternalOutput")

    with tile.TileContext(nc) as tc:
        with (
            tc.tile_pool(name="const", bufs=1) as cst,
            tc.tile_pool(name="proj", bufs=1) as proj,       # KT/KTb/QTb/V resident
            tc.tile_pool(name="mstuff", bufs=1) as mst,      # M / topk / sel smalls
            tc.tile_pool(name="mstream", bufs=3) as mstr,    # mask chunks
            tc.tile_pool(name="scr", bufs=3) as scr,         # TTR scratch
            tc.tile_pool(name="acc", bufs=2) as accp,        # per-chunk accums
            tc.tile_pool(name="cand", bufs=1) as cnd,        # exact-stage tiles
            tc.tile_pool(name="ps", bufs=3, space="PSUM") as ps,
            tc.tile_pool(name="ps_s", bufs=4, space="PSUM") as ps_s,  # S_cand (held)
            tc.tile_pool(name="dram", bufs=1, space="DRAM") as drp,
        ):
            # ---------------- constants ----------------
            ident = cst.tile([P, P], f32, tag="ident")
            make_identity(nc, ident[:])
            ones_r1 = cst.tile([1, P], f32, tag="ones_r1")
            nc.vector.memset(ones_r1[:], 1.0)
            negone = cst.tile([P, 1], f32, tag="negone")
            nc.vector.memset(negone[:], -1.0)
            negbig = cst.tile([P, 1], f32, tag="negbig")
            nc.vector.memset(negbig[:], NEG)
            big9 = cst.tile([P, 1], f32, tag="big9")
            nc.vector.memset(big9[:], SKIP_IDX)
            perm16 = cst.tile([16, 8 * P], f32, tag="perm16")
            nc.sync.dma_start(perm16[:], perm_d[:])
            qidx_i = cst.tile([P, 16], i32, tag="qidx_i")     # value p + 128*c
            nc.gpsimd.iota(qidx_i[:], pattern=[[P, 16]], base=0, channel_multiplier=1)
            qidx_f = cst.tile([P, 16], f32, tag="qidx_f")
            nc.vector.tensor_copy(qidx_f[:], qidx_i[:])
            # preload the (large) attn GPSIMD library now so kth_largest does
            # not pay the ucode reload inside the serial tail
            from concourse import library_config
            nc.gpsimd.load_library(library_config.attn)

            # resident projection outputs
            KT = [proj.tile([P, L], f32, tag=f"KT{ic}", name=f"KT{ic}") for ic in range(ND)]
            KTb = [proj.tile([P, L], bf16, tag=f"KTb{ic}", name=f"KTb{ic}") for ic in range(ND)]
            QTb = [proj.tile([P, L], bf16, tag=f"QTb{ic}", name=f"QTb{ic}") for ic in range(ND)]
            Vb = [proj.tile([P, D], bf16, tag=f"Vb{jc}", name=f"Vb{jc}") for jc in range(NL)]

            with tc.tile_pool(name="xw", bufs=1) as xw:
                # ---------------- phase 0: loads ----------------
                xTh = [xw.tile([P, L], bf16, tag=f"xTh{dc}", name=f"xTh{dc}") for dc in range(ND)]
                xTl = [xw.tile([P, L], bf16, tag=f"xTl{dc}", name=f"xTl{dc}") for dc in range(ND)]
                wqh = [xw.tile([P, D], bf16, tag=f"wqh{dc}", name=f"wqh{dc}") for dc in range(ND)]
                wkh = [xw.tile([P, D], bf16, tag=f"wkh{dc}", name=f"wkh{dc}") for dc in range(ND)]
                wkl = [xw.tile([P, D], bf16, tag=f"wkl{dc}", name=f"wkl{dc}") for dc in range(ND)]
                wvh = [xw.tile([P, D], bf16, tag=f"wvh{dc}", name=f"wvh{dc}") for dc in range(ND)]
                wvl = [xw.tile([P, D], bf16, tag=f"wvl{dc}", name=f"wvl{dc}") for dc in range(ND)]
                wqT = [xw.tile([P, D], f32, tag=f"wqT{dc}", name=f"wqT{dc}") for dc in range(ND)]
                xmh = [xw.tile([P, 1], bf16, tag=f"xmh{dc}", name=f"xmh{dc}") for dc in range(ND)]
                xml = [xw.tile([P, 1], bf16, tag=f"xml{dc}", name=f"xml{dc}") for dc in range(ND)]
                for dc in range(ND):
                    sl = slice(dc * P, (dc + 1) * P)
                    nc.sync.dma_start(xTh[dc][:], xth_d[sl, :])
                    nc.sync.dma_start(xTl[dc][:], xtl_d[sl, :])
                    nc.sync.dma_start(wqh[dc][:], wqh_d[sl, :])
                    nc.sync.dma_start(wkh[dc][:], wkh_d[sl, :])
                    nc.sync.dma_start(wkl[dc][:], wkl_d[sl, :])
                    nc.sync.dma_start(wvh[dc][:], wvh_d[sl, :])
                    nc.sync.dma_start(wvl[dc][:], wvl_d[sl, :])
                    nc.sync.dma_start(wqT[dc][:], wq_d[sl, :])
                    nc.sync.dma_start(xmh[dc][:], xmh_d[sl, :])
                    nc.sync.dma_start(xml[dc][:], xml_d[sl, :])

                # ---------------- phase 1: projections ----------------
                # K: 3-term bf16x2 (fp32-class), into KT f32 + KTb bf16
                for ic in range(ND):
                    isl = slice(ic * P, (ic + 1) * P)
                    for jb in range(NJ):
                        jsl = slice(jb * 512, (jb + 1) * 512)
                        pk = ps.tile([P, 512], f32, tag="blk")
                        n = 0
                        for dc in range(ND):
                            for lh, rh in (
                                (wkh[dc][:, isl], xTh[dc][:, jsl]),
                                (wkh[dc][:, isl], xTl[dc][:, jsl]),
                                (wkl[dc][:, isl], xTh[dc][:, jsl]),
                            ):
                                nc.tensor.matmul(
                                    pk[:], lh, rh,
                                    start=(n == 0), stop=(n == 3 * ND - 1),
                                )
                                n += 1
                        nc.scalar.copy(KT[ic][:, jsl], pk[:])
                        nc.vector.tensor_copy(KTb[ic][:, jsl], pk[:])

                # Q approx: single bf16 term
                for ic in range(ND):
                    isl = slice(ic * P, (ic + 1) * P)
                    for jb in range(NJ):
                        jsl = slice(jb * 512, (jb + 1) * 512)
                        pq = ps.tile([P, 512], f32, tag="blk")
                        for dc in range(ND):
                            nc.tensor.matmul(
                                pq[:], wqh[dc][:, isl], xTh[dc][:, jsl],
                                start=(dc == 0), stop=(dc == ND - 1),
                            )
                        nc.scalar.copy(QTb[ic][:, jsl], pq[:])

                # ---------------- phase 2: approx M (bf16 S) ----------------
                # per (lc, jb) block: one STT (masked product -> bf16 scratch,
                # fused sum accum) + one reduce_max. Combines batched at end.
                M_all = mst.tile([P, 16], f32, tag="M_all")
                amax_all = mst.tile([P, NL * NJ], f32, tag="amax_all")
                asum_all = mst.tile([P, NL * NJ], f32, tag="asum_all")
                for lc in range(NL):
                    lsl = slice(lc * P, (lc + 1) * P)
                    mk = mstr.tile([P, L], u8, tag="mk")
                    nc.sync.dma_start(mk[:], mask_d[lsl, :])
                    # V chunk lc interleaved here (single bf16 term: the upd
                    # matmul consumes bf16 anyway) to keep PE warm while the
                    # vector engine drains the S-extraction
                    pv = ps.tile([P, 512], f32, tag="blk")
                    for dc in range(ND):
                        nc.tensor.matmul(
                            pv[:], xTh[dc][:, lsl], wvh[dc][:],
                            start=(dc == 0), stop=(dc == ND - 1),
                        )
                    nc.scalar.copy(Vb[lc][:], pv[:])
                    for jb in range(NJ):
                        jsl = slice(jb * 512, (jb + 1) * 512)
                        k = lc * NJ + jb
                        pss = ps_s.tile([P, 512], f32, tag="psSc", name="pssa")
                        for ic in range(ND):
                            nc.tensor.matmul(
                                pss[:], QTb[ic][:, lsl], KTb[ic][:, jsl],
                                start=(ic == 0), stop=(ic == ND - 1),
                            )
                        s1 = scr.tile([P, 512], bf16, tag="scrt")
                        nc.vector.scalar_tensor_tensor(
                            out=s1[:], in0=pss[:], scalar=1.0, in1=mk[:, jsl],
                            op0=OP.mult, op1=OP.mult,
                            accum_out=asum_all[:, k : k + 1],
                        )
                        nc.vector.reduce_max(
                            amax_all[:, k : k + 1], s1[:], axis=AX.X
                        )
                t1 = accp.tile([P, 16], f32, tag="t1")
                t2 = accp.tile([P, 16], f32, tag="t2")
                nc.vector.reduce_max(
                    t1[:], amax_all[:].rearrange("p (c j) -> p c j", j=NJ),
                    axis=AX.X,
                )
                nc.vector.reduce_sum(
                    t2[:], asum_all[:].rearrange("p (c j) -> p c j", j=NJ),
                    axis=AX.X,
                )
                nc.vector.tensor_scalar_mul(t2[:], t2[:], -1.0 / L)
                nc.vector.tensor_tensor(
                    out=M_all[:], in0=t1[:], in1=t2[:], op=OP.add
                )

                # Vmean = xmean.T @ Wv.T via bf16x2 3-term, broadcast, ctx init
                pvm = ps.tile([1, 512], f32, tag="blk")
                n = 0
                for dc in range(ND):
                    for lh, rh in (
                        (xmh[dc][:], wvh[dc][:]),
                        (xml[dc][:], wvh[dc][:]),
                        (xmh[dc][:], wvl[dc][:]),
                    ):
                        nc.tensor.matmul(
                            pvm[:1, :], lh, rh,
                            start=(n == 0), stop=(n == 3 * ND - 1),
                        )
                        n += 1
                vmean = mst.tile([1, 512], f32, tag="vmean")
                nc.scalar.copy(vmean[:], pvm[:1, :])
                pvb = ps.tile([P, 512], f32, tag="blk")
                nc.tensor.matmul(pvb[:], ones_r1[:], vmean[:], start=True, stop=True)
                vmean_bc = mst.tile([P, 512], f32, tag="vmean_bc")
                nc.vector.tensor_copy(vmean_bc[:], pvb[:])
                for jc in range(NL):
                    nc.sync.dma_start(ctx_d[jc * P : (jc + 1) * P, :], vmean_bc[:])

                # ---------------- phase 3: approx top-40 -> candidates ------
                # approx threshold ~= 40th largest of M (rank +-1 is fine:
                # the DELTA band absorbs it) via one GPSIMD kth_largest
                kout = mst.tile([1, 2], f32, tag="kout")
                nc.gpsimd.kth_largest(
                    out_ap=kout[:], in_ap=M_all[:], n_per_lane=16, k=64,
                    quantile=1.0 - 38.5 / 2047.0,
                )
                ptb = ps.tile([P, 1], f32, tag="blk")
                nc.tensor.matmul(
                    ptb[:P, :1], ones_r1[:], kout[:, 1:2], start=True, stop=True
                )
                tbc = mst.tile([P, 1], f32, tag="tbc")
                nc.vector.tensor_copy(tbc[:], ptb[:P, :1])

                # selmask = (M - T40) >= -DELTA, one fused op
                selmask = mst.tile([P, 16], u8, tag="selmask")
                nc.vector.tensor_scalar(
                    selmask[:], M_all[:], tbc[:], -DELTA,
                    op0=OP.subtract, op1=OP.is_ge,
                )
                midx = mst.tile([P, 16], f32, tag="midx")
                nc.vector.tensor_copy(midx[:], negone[:].to_broadcast([P, 16]))
                nc.vector.copy_predicated(midx[:], selmask[:], qidx_f[:])

                pwr = ps.tile([16, P], f32, tag="blk", name="pwr")
                nc.tensor.transpose(pwr[:16, :P], midx[:], ident[:])
                wrap_in = mst.tile([16, P], f32, tag="wrap_in")
                nc.vector.tensor_copy(wrap_in[:], pwr[:16, :P])
                spg = mst.tile([16, 8], f32, tag="spg")
                nfound = mst.tile([1, 1], u32, tag="nfound")
                nc.gpsimd.sparse_gather(out=spg[:], in_=wrap_in[:], num_found=nfound[:])
                spg_cl = mst.tile([16, 8], f32, tag="spg_cl")
                nc.vector.tensor_scalar_max(spg_cl[:], spg[:], 0.0)
                nc.vector.tensor_scalar_min(spg_cl[:], spg_cl[:], float(L - 1))

                # unwrap [16,8] -> [128,1] with 8 tiny one-hot matmuls
                # (perm16[p, f*128+u] = 1 iff u == p + 16*f, shipped constant)
                pcq = ps.tile([P, 1], f32, tag="blk", name="pcq")
                for f in range(8):
                    nc.tensor.matmul(
                        pcq[:P, :1], perm16[:, f * P : (f + 1) * P],
                        spg_cl[:, f : f + 1],
                        start=(f == 0), stop=(f == 7),
                    )
                candq_f = mst.tile([P, 1], f32, tag="candq_f")
                nc.vector.tensor_copy(candq_f[:], pcq[:P, :1])
                candq_i = mst.tile([P, 1], i32, tag="candq_i")
                nc.vector.tensor_copy(candq_i[:], pcq[:P, :1])

                nf_f = mst.tile([1, 1], f32, tag="nf_f")
                nc.vector.tensor_copy(nf_f[:], nfound[:])
                pnb = ps.tile([P, 1], f32, tag="blk")
                nc.tensor.matmul(pnb[:P, :1], ones_r1[:], nf_f[:], start=True, stop=True)
                nbc = mst.tile([P, 1], f32, tag="nbc")
                nc.vector.tensor_copy(nbc[:], pnb[:P, :1])
                invalid = mst.tile([P, 1], u8, tag="invalid")
                nc.vector.tensor_tensor(
                    out=invalid[:], in0=qidx_f[:, 0:1], in1=nbc[:], op=OP.is_ge
                )

                # ---------------- phase 4a: exact candidates ----------------
                x_cand = cnd.tile([P, D], f32, tag="x_cand")
                nc.gpsimd.indirect_dma_start(
                    out=x_cand[:], out_offset=None, in_=x_d[:],
                    in_offset=bass.IndirectOffsetOnAxis(ap=candq_i[:, :1], axis=0),
                )
                xcT = [cnd.tile([P, P], f32, tag=f"xcT{dc}", name=f"xcT{dc}") for dc in range(ND)]
                for dc in range(ND):
                    pxc = ps.tile([P, P], f32, tag="blk")
                    nc.tensor.transpose(
                        pxc[:P, :P], x_cand[:, dc * P : (dc + 1) * P], ident[:]
                    )
                    nc.vector.tensor_copy(xcT[dc][:], pxc[:P, :P])

                QcT = [cnd.tile([P, P], f32, tag=f"QcT{ic}", name=f"QcT{ic}") for ic in range(ND)]
                for ic in range(ND):
                    isl = slice(ic * P, (ic + 1) * P)
                    pqc = ps.tile([P, P], f32, tag="blk")
                    for dc in range(ND):
                        nc.tensor.matmul(
                            pqc[:P, :P], wqT[dc][:, isl], xcT[dc][:],
                            start=(dc == 0), stop=(dc == ND - 1),
                        )
                    nc.vector.tensor_copy(QcT[ic][:], pqc[:P, :P])

                gm = cnd.tile([P, L], u8, tag="gm")
                nc.gpsimd.indirect_dma_start(
                    out=gm[:], out_offset=None, in_=mask_d[:],
                    in_offset=bass.IndirectOffsetOnAxis(ap=candq_i[:, :1], axis=0),
                )
                gc = cnd.tile([P, L], u8, tag="gc")
                nc.gpsimd.indirect_dma_start(
                    out=gc[:], out_offset=None, in_=cnt_d[:],
                    in_offset=bass.IndirectOffsetOnAxis(ap=candq_i[:, :1], axis=0),
                )

                psS = []
                cmax = cnd.tile([P, NJ], f32, tag="cmax")
                csum = cnd.tile([P, NJ], f32, tag="csum")
                for jb in range(NJ):
                    jsl = slice(jb * 512, (jb + 1) * 512)
                    pss2 = ps_s.tile([P, 512], f32, tag="psSc")
                    psS.append(pss2)
                    for ic in range(ND):
                        nc.tensor.matmul(
                            pss2[:], QcT[ic][:], KT[ic][:, jsl],
                            start=(ic == 0), stop=(ic == ND - 1),
                        )
                    s3 = scr.tile([P, 512], f32, tag="scrt")
                    nc.vector.tensor_tensor(
                        out=s3[:], in0=pss2[:], in1=gm[:, jsl], op=OP.mult
                    )
                    nc.vector.reduce_max(cmax[:, jb : jb + 1], s3[:], axis=AX.X)
                    s4 = scr.tile([P, 512], f32, tag="scrt")
                    nc.vector.scalar_tensor_tensor(
                        out=s4[:], in0=pss2[:], scalar=-1.0 / L, in1=gc[:, jsl],
                        op0=OP.mult, op1=OP.mult,
                        accum_out=csum[:, jb : jb + 1],
                    )
                u1 = cnd.tile([P, 1], f32, tag="u1")
                u2 = cnd.tile([P, 1], f32, tag="u2")
                M_cand = cnd.tile([P, 1], f32, tag="M_cand")
                nc.vector.reduce_max(u1[:], cmax[:], axis=AX.X)
                nc.vector.reduce_sum(u2[:], csum[:], axis=AX.X)
                nc.vector.tensor_tensor(out=M_cand[:], in0=u1[:], in1=u2[:], op=OP.add)
                nc.vector.copy_predicated(M_cand[:], invalid[:], negbig[:])

                # exact top-40 threshold among candidates
                pmc = ps.tile([1, P], f32, tag="blk")
                nc.tensor.transpose(pmc[:1, :P], M_cand[:], ident[:])
                mcT = cnd.tile([1, P], f32, tag="mcT")
                nc.vector.tensor_copy(mcT[:], pmc[:1, :P])
                etop = cnd.tile([1, NT], f32, tag="etop")
                for r in range(5):
                    nc.vector.max(out=etop[:, 8 * r : 8 * r + 8], in_=mcT[:])
                    if r < 4:
                        nc.vector.match_replace(
                            out=mcT[:], in_to_replace=etop[:, 8 * r : 8 * r + 8],
                            in_values=mcT[:], imm_value=NEG,
                        )
                pte = ps.tile([P, 1], f32, tag="blk")
                nc.tensor.matmul(
                    pte[:P, :1], ones_r1[:], etop[:, NT - 1 : NT], start=True, stop=True
                )
                tebc = cnd.tile([P, 1], f32, tag="tebc")
                nc.vector.tensor_copy(tebc[:], pte[:P, :1])
                sel2 = cnd.tile([P, 1], u8, tag="sel2")
                nc.vector.tensor_tensor(
                    out=sel2[:], in0=M_cand[:], in1=tebc[:], op=OP.is_ge
                )
                scat_f = cnd.tile([P, 1], f32, tag="scat_f")
                nc.vector.tensor_copy(scat_f[:], big9[:])
                nc.vector.copy_predicated(scat_f[:], sel2[:], candq_f[:])
                scat_i = cnd.tile([P, 1], i32, tag="scat_i")
                nc.vector.tensor_copy(scat_i[:], scat_f[:])

            # xTh/xTl/weights freed here
            with tc.tile_pool(name="expp", bufs=1) as expp:
                # ---------------- phase 4b: softmax + update ----------------
                rmax4 = expp.tile([P, NJ], f32, tag="rmax4")
                for jb in range(NJ):
                    nc.vector.reduce_max(rmax4[:, jb : jb + 1], psS[jb][:], axis=AX.X)
                rmax = expp.tile([P, 1], f32, tag="rmax")
                nc.vector.reduce_max(rmax[:], rmax4[:], axis=AX.X)
                negb = expp.tile([P, 1], f32, tag="negb")
                nc.vector.tensor_scalar_mul(negb[:], rmax[:], -SCALE)
                exp_sb = expp.tile([P, L], f32, tag="exp_sb")
                sume4 = expp.tile([P, NJ], f32, tag="sume4")
                for jb in range(NJ):
                    jsl = slice(jb * 512, (jb + 1) * 512)
                    nc.scalar.activation(
                        out=exp_sb[:, jsl], in_=psS[jb][:], func=ACTF.Exp,
                        bias=negb[:], scale=SCALE,
                        accum_out=sume4[:, jb : jb + 1],
                    )
                sume = expp.tile([P, 1], f32, tag="sume")
                nc.vector.reduce_sum(sume[:], sume4[:], axis=AX.X)
                recip = expp.tile([P, 1], f32, tag="recip")
                nc.vector.reciprocal(recip[:], sume[:])

                expT = [expp.tile([P, P], bf16, tag=f"expT{jc}", name=f"expT{jc}") for jc in range(NL)]
                for jc in range(NL):
                    pet = ps.tile([P, P], f32, tag="blk")
                    nc.tensor.transpose(
                        pet[:P, :P], exp_sb[:, jc * P : (jc + 1) * P], ident[:]
                    )
                    nc.vector.tensor_copy(expT[jc][:], pet[:P, :P])

                pu = ps.tile([P, 512], f32, tag="blk")
                for jc in range(NL):
                    nc.tensor.matmul(
                        pu[:], expT[jc][:], Vb[jc][:],
                        start=(jc == 0), stop=(jc == NL - 1),
                    )
                upd = expp.tile([P, D], f32, tag="upd")
                nc.scalar.activation(
                    out=upd[:], in_=pu[:], func=ACTF.Copy, bias=0.0, scale=recip[:]
                )
                nc.gpsimd.indirect_dma_start(
                    out=ctx_d[:],
                    out_offset=bass.IndirectOffsetOnAxis(ap=scat_i[:, :1], axis=0),
                    in_=upd[:], in_offset=None,
                    bounds_check=L - 1, oob_is_err=False,
                )

    nc.compile()
    return nc


_NC = None


def _get_nc():
    global _NC
    if _NC is None:
        _NC = build()
    return _NC


def _split_bf16(a):
    hi = a.astype(ml_dtypes.bfloat16)
    lo = (a - hi.astype(np.float32)).astype(ml_dtypes.bfloat16)
    return hi, lo


def _host_prep(x, Wq, Wk, Wv, index_sample):
    x = np.asarray(x, dtype=np.float32)
    Wq = np.asarray(Wq, dtype=np.float32)
    Wk = np.asarray(Wk, dtype=np.float32)
    Wv = np.asarray(Wv, dtype=np.float32)
    idx = np.asarray(index_sample)

    wqT = np.ascontiguousarray(Wq.T)
    wqh, _ = _split_bf16(wqT)
    wkh, wkl = _split_bf16(np.ascontiguousarray(Wk.T))
    wvh, wvl = _split_bf16(np.ascontiguousarray(Wv.T))

    rows = np.arange(L)[:, None]
    mask01 = np.zeros((L, L), dtype=np.uint8)
    mask01[rows, idx] = 1
    countf = np.zeros((L, L), dtype=np.uint8)
    np.add.at(countf, (rows, idx), 1)

    perm16 = np.zeros((16, 8 * P), dtype=np.float32)
    for f in range(8):
        for p in range(16):
            perm16[p, f * P + p + 16 * f] = 1.0
    shared = {
        "wqTh": wqh, "wkTh": wkh, "wkTl": wkl, "wvTh": wvh, "wvTl": wvl,
        "wqT": wqT, "mask01": mask01, "countf": countf, "perm16": perm16,
    }
    in_maps = []
    for b in range(B):
        xb = np.ascontiguousarray(x[b])
        xT = np.ascontiguousarray(xb.T)
        xth, xtl = _split_bf16(xT)
        xmean = (xb.astype(np.float64).mean(axis=0) / 1.0).astype(np.float32)
        xmeh, xmel = _split_bf16(xmean.reshape(D, 1))
        in_maps.append(
            {
                "x_nat": xb,
                "xTh": xth,
                "xTl": xtl,
                "xmeanTh": xmeh,
                "xmeanTl": xmel,
                **shared,
            }
        )
    return in_maps


def kernel(x, Wq, Wk, Wv, index_sample, _trace=False, _result_box=None):
    in_maps = _host_prep(x, Wq, Wk, Wv, index_sample)
    nc = _get_nc()
    res = run_bass_kernel_spmd(nc, in_maps, core_ids=list(range(B)), trace=_trace)
    if _result_box is not None:
        _result_box.append(res)
    out = np.stack([np.asarray(res.results[b]["ctx"]) for b in range(B)], axis=0)
    return out



# revision 14
# speedup vs baseline: 1.6434x; 1.6434x over previous
"""Sparse attention (ProbSparse-style) Trainium2 Bass kernel, v2.

Problem (per batch element b, data-parallel over 8 NeuronCores):
  Q = x @ Wq.T ; K = x @ Wk.T ; V = x @ Wv.T            [L=2048, D=512]
  QK_sample[l,s] = Q[l] . K[index_sample[l,s]]           [L, 40]
  M[l] = max_s QK_sample - sum_s QK_sample / L
  sel = top40(M)  (as a set; the reference scatter makes order irrelevant)
  scores = Q[sel] @ K.T / sqrt(D); attn = softmax(scores)
  ctx = broadcast(mean(V)); ctx[sel] = attn @ V

Key ideas vs v1 baseline:
  - A = Wq^T @ Wk precomputed on host: S = (x A) x^T. Kills the K and Q
    projections entirely; both approx and exact scores contract against
    the resident x^T tiles.
  - Approx M = masked max of bf16 S only (the sum/L term is <= ~0.5 and
    is absorbed by the candidate margin; validated: true top-40 rows sit
    within rank <= 40 of the approx ordering).
  - Threshold via a 64-step mu + c*sigma ladder with on-device counts
    (one 3d-broadcast compare + reduce + PE column-sum), picking the
    largest T with count >= 88 (fallback: smallest T with count <= 127).
    Replaces the 62us GPSIMD kth_largest.
  - Exact stage on <= 128 candidates: G = x_cand A (3-term bf16),
    S_cand = G x^T (3-term bf16)  ->  ~1e-4-class absolute error,
    validated 26x under the seed-0 top-40 boundary gap.
  - Softmax without max subtraction (|S*scale| <= ~9, exp is safe),
    upd = (attn @ x) @ Wv^T (kills the V projection; V never built).

kernel(**inputs) accepts FULL inputs, returns FULL [8, 2048, 512] f32;
batch is sharded over 8 cores.
"""

import math

import numpy as np
import ml_dtypes

import concourse.bacc as bacc
import concourse.bass as bass
import concourse.mybir as mybir
import concourse.tile as tile
from concourse.bass_utils import run_bass_kernel_spmd
from concourse.masks import make_identity
from concourse import library_config

P = 128
L = 2048
D = 512
B = 8
NL = L // P        # 16 query chunks
ND = D // P        # 4 feature chunks
NJ = L // 512      # 4 key blocks of 512
NT = 40
NLAD = 64          # threshold ladder steps
SCALE = 1.0 / math.sqrt(D)
NEG = -3.0e38
BIG = 3.0e38
SKIP_IDX = 99999.0  # scatter index sentinel (> bounds_check -> row skipped)

f32 = mybir.dt.float32
bf16 = mybir.dt.bfloat16
u8 = mybir.dt.uint8
i32 = mybir.dt.int32
u32 = mybir.dt.uint32
AX = mybir.AxisListType
OP = mybir.AluOpType
ACTF = mybir.ActivationFunctionType


def build():
    nc = bacc.Bacc("TRN2", target_bir_lowering=False)

    x_d = nc.dram_tensor("x_nat", [L, D], f32, kind="ExternalInput")
    xth_d = nc.dram_tensor("xTh", [D, L], bf16, kind="ExternalInput")
    xtl_d = nc.dram_tensor("xTl", [D, L], bf16, kind="ExternalInput")
    xnh_d = nc.dram_tensor("xNh", [L, D], bf16, kind="ExternalInput")
    ah_d = nc.dram_tensor("Ah", [D, D], bf16, kind="ExternalInput")
    al_d = nc.dram_tensor("Al", [D, D], bf16, kind="ExternalInput")
    wvh_d = nc.dram_tensor("wvTh", [D, D], bf16, kind="ExternalInput")
    wvl_d = nc.dram_tensor("wvTl", [D, D], bf16, kind="ExternalInput")
    xmh_d = nc.dram_tensor("xmeanTh", [D, 1], bf16, kind="ExternalInput")
    xml_d = nc.dram_tensor("xmeanTl", [D, 1], bf16, kind="ExternalInput")
    maskb_d = nc.dram_tensor("maskb", [L, L], bf16, kind="ExternalInput")
    cnt_d = nc.dram_tensor("countf", [L, L], u8, kind="ExternalInput")
    perm_d = nc.dram_tensor("perm16", [16, 8 * P], f32, kind="ExternalInput")
    qidx_d = nc.dram_tensor("qidxf", [P, NL], f32, kind="ExternalInput")
    crow_d = nc.dram_tensor("crow", [1, NLAD], f32, kind="ExternalInput")
    ctx_d = nc.dram_tensor("ctx", [L, D], f32, kind="ExternalOutput")

    with tile.TileContext(nc) as tc:
        with (
            tc.tile_pool(name="const", bufs=1) as cst,
            tc.tile_pool(name="xres", bufs=1) as xres,      # resident x / A / Wv
            tc.tile_pool(name="proj", bufs=1) as proj,      # QATb
            tc.tile_pool(name="mstuff", bufs=1) as mst,     # M / threshold smalls
            tc.tile_pool(name="mstream", bufs=3) as mstr,   # mask chunks
            tc.tile_pool(name="scr", bufs=3) as scr,        # TTR scratch
            tc.tile_pool(name="cand", bufs=1) as cnd,       # exact-stage tiles
            tc.tile_pool(name="expp", bufs=1) as expp,      # softmax/upd tiles
            tc.tile_pool(name="ps", bufs=2, space="PSUM") as ps,
            tc.tile_pool(name="psb", bufs=2, space="PSUM") as psb,    # bf16 transposes
            tc.tile_pool(name="ps_s", bufs=4, space="PSUM") as ps_s,  # S blocks
        ):
            # ---------------- constants ----------------
            ident = cst.tile([P, P], f32, tag="ident")
            make_identity(nc, ident[:])
            ident_b = cst.tile([P, P], bf16, tag="ident_b")
            nc.vector.tensor_copy(ident_b[:], ident[:])
            # preload the sparse_gather ucode so the serial tail does not
            # pay the library switch
            nc.gpsimd.load_library(library_config.sparse_gather)
            ones_r1 = cst.tile([1, P], f32, tag="ones_r1")
            nc.vector.memset(ones_r1[:], 1.0)
            ones_cf = cst.tile([P, 1], f32, tag="ones_cf")
            nc.vector.memset(ones_cf[:], 1.0)
            negbig = cst.tile([P, 1], f32, tag="negbig")
            nc.vector.memset(negbig[:], NEG)
            big9 = cst.tile([P, 1], f32, tag="big9")
            nc.vector.memset(big9[:], SKIP_IDX)
            perm16 = cst.tile([16, 8 * P], f32, tag="perm16")
            nc.sync.dma_start(perm16[:], perm_d[:])
            qidx_f = cst.tile([P, NL], f32, tag="qidx_f")
            nc.sync.dma_start(qidx_f[:], qidx_d[:])
            crow = cst.tile([1, NLAD], f32, tag="crow")
            nc.sync.dma_start(crow[:], crow_d[:])

            # ---------------- resident loads ----------------
            Ah = [xres.tile([P, D], bf16, tag=f"Ah{dc}", name=f"Ah{dc}") for dc in range(ND)]
            Al = [xres.tile([P, D], bf16, tag=f"Al{dc}", name=f"Al{dc}") for dc in range(ND)]
            xTh = [xres.tile([P, L], bf16, tag=f"xTh{dc}", name=f"xTh{dc}") for dc in range(ND)]
            xTl = [xres.tile([P, L], bf16, tag=f"xTl{dc}", name=f"xTl{dc}") for dc in range(ND)]
            wvh = [xres.tile([P, D], bf16, tag=f"wvh{dc}", name=f"wvh{dc}") for dc in range(ND)]
            wvl = [xres.tile([P, D], bf16, tag=f"wvl{dc}", name=f"wvl{dc}") for dc in range(ND)]
            xmh = [xres.tile([P, 1], bf16, tag=f"xmh{dc}", name=f"xmh{dc}") for dc in range(ND)]
            xml = [xres.tile([P, 1], bf16, tag=f"xml{dc}", name=f"xml{dc}") for dc in range(ND)]
            xNh = [xres.tile([P, D], bf16, tag=f"xNh{jc}", name=f"xNh{jc}") for jc in range(NL)]

            for dc in range(ND):
                nc.sync.dma_start(Ah[dc][:], ah_d[dc * P : (dc + 1) * P, :])
            # x^T hi in jb-major sub-blocks so the first QA matmuls can
            # start after ~1MB of input instead of ~4.5MB
            for jb in range(NJ):
                jsl = slice(jb * 512, (jb + 1) * 512)
                for dc in range(ND):
                    nc.sync.dma_start(xTh[dc][:, jsl], xth_d[dc * P : (dc + 1) * P, jsl])
            for dc in range(ND):
                sl = slice(dc * P, (dc + 1) * P)
                nc.sync.dma_start(wvh[dc][:], wvh_d[sl, :])
                nc.sync.dma_start(wvl[dc][:], wvl_d[sl, :])
                nc.sync.dma_start(xmh[dc][:], xmh_d[sl, :])
                nc.sync.dma_start(xml[dc][:], xml_d[sl, :])
            for dc in range(ND):
                sl = slice(dc * P, (dc + 1) * P)
                nc.sync.dma_start(Al[dc][:], al_d[sl, :])
                nc.sync.dma_start(xTl[dc][:], xtl_d[sl, :])

            # ---------------- Vmean -> ctx init (early, overlaps all) ----
            pvm = ps.tile([1, D], f32, tag="blk")
            n = 0
            for dc in range(ND):
                for lh, rh in (
                    (xmh[dc][:], wvh[dc][:]),
                    (xml[dc][:], wvh[dc][:]),
                    (xmh[dc][:], wvl[dc][:]),
                ):
                    nc.tensor.matmul(
                        pvm[:1, :], lh, rh,
                        start=(n == 0), stop=(n == 3 * ND - 1),
                    )
                    n += 1
            vmean = mst.tile([1, D], f32, tag="vmean")
            nc.scalar.copy(vmean[:], pvm[:1, :])
            pvb = ps.tile([P, D], f32, tag="blk")
            nc.tensor.matmul(pvb[:], ones_r1[:], vmean[:], start=True, stop=True)
            vmean_bc = mst.tile([P, D], f32, tag="vmean_bc")
            nc.vector.tensor_copy(vmean_bc[:], pvb[:])
            for jc in range(NL):
                nc.sync.dma_start(ctx_d[jc * P : (jc + 1) * P, :], vmean_bc[:])

            # ---------------- phase 1: QA^T = A^T x^T (bf16) ------------
            QATb = [proj.tile([P, L], bf16, tag=f"QATb{ic}", name=f"QATb{ic}") for ic in range(ND)]
            for jb in range(NJ):
                jsl = slice(jb * 512, (jb + 1) * 512)
                for ic in range(ND):
                    isl = slice(ic * P, (ic + 1) * P)
                    pq = ps.tile([P, 512], f32, tag="blk")
                    for dc in range(ND):
                        nc.tensor.matmul(
                            pq[:], Ah[dc][:, isl], xTh[dc][:, jsl],
                            start=(dc == 0), stop=(dc == ND - 1),
                        )
                    nc.scalar.copy(QATb[ic][:, jsl], pq[:])

            # ---------------- phase 2: approx S + masked max ------------
            # Per 128-query chunk: PE computes 4 S blocks; ScalarE evicts
            # them to a bf16 row [P, 2048]; DVE does one 2x bf16 mask-mult
            # + one wide reduce_max.  (tensor_tensor_reduce crashes TRN2
            # hardware, so the fused form is not available.)
            M_all = mst.tile([P, NL], f32, tag="M_all")
            for lc in range(NL):
                lsl = slice(lc * P, (lc + 1) * P)
                mkb = mstr.tile([P, L], bf16, tag="mkb")
                nc.sync.dma_start(mkb[:], maskb_d[lsl, :])
                # late-needed loads trickled in so they don't starve masks
                if lc < NL:
                    nc.sync.dma_start(xNh[lc][:], xnh_d[lc * P : (lc + 1) * P, :])
                sb1 = scr.tile([P, L], bf16, tag="sb1")
                for jb in range(NJ):
                    jsl = slice(jb * 512, (jb + 1) * 512)
                    pss = ps_s.tile([P, 512], f32, tag="psSc", name="pssa")
                    for ic in range(ND):
                        nc.tensor.matmul(
                            pss[:], QATb[ic][:, lsl], xTh[ic][:, jsl],
                            start=(ic == 0), stop=(ic == ND - 1),
                        )
                    nc.scalar.copy(sb1[:, jsl], pss[:])
                s1 = scr.tile([P, L], bf16, tag="s1m")
                nc.vector.tensor_tensor(
                    out=s1[:], in0=sb1[:], in1=mkb[:], op=OP.mult
                )
                nc.vector.reduce_max(M_all[:, lc : lc + 1], s1[:], axis=AX.X)

            # ---------------- phase 3: ladder threshold -----------------
            # per-partition sum and sum-of-squares
            stats2 = mst.tile([P, 2], f32, tag="stats2")
            msq = mst.tile([P, NL], f32, tag="msq")
            nc.vector.scalar_tensor_tensor(
                out=msq[:], in0=M_all[:], scalar=1.0, in1=M_all[:],
                op0=OP.mult, op1=OP.mult,
                accum_out=stats2[:, 1:2],
            )
            nc.vector.tensor_reduce(
                stats2[:, 0:1], M_all[:], axis=AX.X, op=OP.add
            )
            pst = ps.tile([1, 2], f32, tag="blk")
            nc.tensor.matmul(pst[:1, :2], ones_cf[:], stats2[:], start=True, stop=True)
            srow = mst.tile([1, 2], f32, tag="srow")
            nc.vector.tensor_copy(srow[:], pst[:1, :2])
            # mu, sigma on [1,1]
            mu = mst.tile([1, 1], f32, tag="mu")
            nc.vector.tensor_scalar_mul(mu[:], srow[:, 0:1], 1.0 / L)
            ex2 = mst.tile([1, 1], f32, tag="ex2")
            nc.vector.tensor_scalar_mul(ex2[:], srow[:, 1:2], 1.0 / L)
            mu2 = mst.tile([1, 1], f32, tag="mu2")
            nc.vector.tensor_tensor(out=mu2[:], in0=mu[:], in1=mu[:], op=OP.mult)
            var = mst.tile([1, 1], f32, tag="var")
            nc.vector.tensor_tensor(out=var[:], in0=ex2[:], in1=mu2[:], op=OP.subtract)
            sigma = mst.tile([1, 1], f32, tag="sigma")
            nc.scalar.sqrt(sigma[:], var[:])
            # Trow = mu + crow * sigma
            Trow = mst.tile([1, NLAD], f32, tag="Trow")
            nc.vector.tensor_tensor(
                out=Trow[:], in0=crow[:], in1=sigma[:].to_broadcast([1, NLAD]),
                op=OP.mult,
            )
            nc.vector.tensor_tensor(
                out=Trow[:], in0=Trow[:], in1=mu[:].to_broadcast([1, NLAD]),
                op=OP.add,
            )
            # broadcast thresholds to all partitions
            ptb = ps.tile([P, NLAD], f32, tag="blk")
            nc.tensor.matmul(ptb[:P, :NLAD], ones_r1[:], Trow[:], start=True, stop=True)
            Tb = mst.tile([P, NLAD], f32, tag="Tb")
            nc.vector.tensor_copy(Tb[:], ptb[:P, :NLAD])
            # cmp[p, j, f] = M[p, f] >= T[j]  (bf16 so PE can column-sum)
            cmpb = mst.tile([P, NLAD * NL], bf16, tag="cmpb")
            nc.vector.tensor_tensor(
                out=cmpb[:].rearrange("p (j f) -> p j f", f=NL),
                in0=M_all[:].rearrange("p (o f) -> p o f", o=1).to_broadcast([P, NLAD, NL]),
                in1=Tb[:].rearrange("p (j o) -> p j o", o=1).to_broadcast([P, NLAD, NL]),
                op=OP.is_ge,
            )
            cnt01 = mst.tile([P, NLAD], f32, tag="cnt01")
            nc.vector.tensor_reduce(
                cnt01[:], cmpb[:].rearrange("p (j f) -> p j f", f=NL),
                axis=AX.X, op=OP.add,
            )
            pcc = ps.tile([1, NLAD], f32, tag="blk")
            nc.tensor.matmul(pcc[:1, :NLAD], ones_cf[:], cnt01[:], start=True, stop=True)
            cntrow = mst.tile([1, NLAD], f32, tag="cntrow")
            nc.vector.tensor_copy(cntrow[:], pcc[:1, :NLAD])
            # largest T with count >= 88; fallback smallest T with count <= 127
            okm = mst.tile([1, NLAD], u8, tag="okm")
            nc.vector.tensor_scalar(
                okm[:], cntrow[:], 87.5, None, op0=OP.is_ge
            )
            negrow = mst.tile([1, NLAD], f32, tag="negrow")
            nc.vector.memset(negrow[:], NEG)
            bigrow = mst.tile([1, NLAD], f32, tag="bigrow")
            nc.vector.memset(bigrow[:], BIG)
            tsel = mst.tile([1, NLAD], f32, tag="tsel")
            nc.vector.select(tsel[:], okm[:], Trow[:], negrow[:])
            tstar = mst.tile([1, 1], f32, tag="tstar")
            nc.vector.reduce_max(tstar[:], tsel[:], axis=AX.X)
            ok2 = mst.tile([1, NLAD], u8, tag="ok2")
            nc.vector.tensor_scalar(
                ok2[:], cntrow[:], 127.5, None, op0=OP.is_le
            )
            tsel2 = mst.tile([1, NLAD], f32, tag="tsel2")
            nc.vector.select(tsel2[:], ok2[:], Trow[:], bigrow[:])
            tfb = mst.tile([1, 1], f32, tag="tfb")
            nc.vector.tensor_reduce(tfb[:], tsel2[:], axis=AX.X, op=OP.min)
            have = mst.tile([1, 1], u8, tag="have")
            nc.vector.tensor_scalar(
                have[:], tstar[:], -1.0e30, None, op0=OP.is_ge
            )
            tfin = mst.tile([1, 1], f32, tag="tfin")
            nc.vector.select(tfin[:], have[:], tstar[:], tfb[:])
            ptf = ps.tile([P, 1], f32, tag="blk")
            nc.tensor.matmul(ptf[:P, :1], ones_r1[:], tfin[:], start=True, stop=True)
            tbc = mst.tile([P, 1], f32, tag="tbc")
            nc.vector.tensor_copy(tbc[:], ptf[:P, :1])

            # selmask / candidate index compaction
            selmask = mst.tile([P, NL], u8, tag="selmask")
            nc.vector.tensor_scalar(
                selmask[:], M_all[:], tbc[:], 0.0,
                op0=OP.subtract, op1=OP.is_ge,
            )
            midx = mst.tile([P, NL], f32, tag="midx")
            nc.vector.memset(midx[:], -1.0)
            nc.vector.copy_predicated(midx[:], selmask[:], qidx_f[:])
            pwr = ps.tile([16, P], f32, tag="blk", name="pwr")
            nc.tensor.transpose(pwr[:16, :P], midx[:], ident[:])
            wrap_in = mst.tile([16, P], f32, tag="wrap_in")
            nc.vector.tensor_copy(wrap_in[:], pwr[:16, :P])
            spg = mst.tile([16, 8], f32, tag="spg")
            nfound = mst.tile([1, 1], u32, tag="nfound")
            nc.gpsimd.sparse_gather(out=spg[:], in_=wrap_in[:], num_found=nfound[:])
            spg_cl = mst.tile([16, 8], f32, tag="spg_cl")
            nc.vector.tensor_scalar_max(spg_cl[:], spg[:], 0.0)
            nc.vector.tensor_scalar_min(spg_cl[:], spg_cl[:], float(L - 1))
            pcq = ps.tile([P, 1], f32, tag="blk", name="pcq")
            for f in range(8):
                nc.tensor.matmul(
                    pcq[:P, :1], perm16[:, f * P : (f + 1) * P],
                    spg_cl[:, f : f + 1],
                    start=(f == 0), stop=(f == 7),
                )
            candq_f = mst.tile([P, 1], f32, tag="candq_f")
            nc.vector.tensor_copy(candq_f[:], pcq[:P, :1])
            candq_i = mst.tile([P, 1], i32, tag="candq_i")
            nc.vector.tensor_copy(candq_i[:], pcq[:P, :1])
            nf_f = mst.tile([1, 1], f32, tag="nf_f")
            nc.vector.tensor_copy(nf_f[:], nfound[:])
            pnb = ps.tile([P, 1], f32, tag="blk")
            nc.tensor.matmul(pnb[:P, :1], ones_r1[:], nf_f[:], start=True, stop=True)
            nbc = mst.tile([P, 1], f32, tag="nbc")
            nc.vector.tensor_copy(nbc[:], pnb[:P, :1])
            invalid = mst.tile([P, 1], u8, tag="invalid")
            nc.vector.tensor_tensor(
                out=invalid[:], in0=qidx_f[:, 0:1], in1=nbc[:], op=OP.is_ge
            )

            # ---------------- phase 4: exact stage ----------------------
            x_cand = cnd.tile([P, D], f32, tag="x_cand")
            nc.gpsimd.indirect_dma_start(
                out=x_cand[:], out_offset=None, in_=x_d[:],
                in_offset=bass.IndirectOffsetOnAxis(ap=candq_i[:, :1], axis=0),
            )
            gm = cnd.tile([P, L], bf16, tag="gm")
            nc.gpsimd.indirect_dma_start(
                out=gm[:], out_offset=None, in_=maskb_d[:],
                in_offset=bass.IndirectOffsetOnAxis(ap=candq_i[:, :1], axis=0),
            )
            gc = cnd.tile([P, L], u8, tag="gc")
            nc.gpsimd.indirect_dma_start(
                out=gc[:], out_offset=None, in_=cnt_d[:],
                in_offset=bass.IndirectOffsetOnAxis(ap=candq_i[:, :1], axis=0),
            )

            # x_cand^T hi/lo
            xcTh = [cnd.tile([P, P], bf16, tag=f"xcTh{dc}", name=f"xcTh{dc}") for dc in range(ND)]
            xcTl = [cnd.tile([P, P], bf16, tag=f"xcTl{dc}", name=f"xcTl{dc}") for dc in range(ND)]
            for dc in range(ND):
                pxc = ps.tile([P, P], f32, tag="blk")
                nc.tensor.transpose(
                    pxc[:P, :P], x_cand[:, dc * P : (dc + 1) * P], ident[:]
                )
                nc.vector.tensor_copy(xcTh[dc][:], pxc[:P, :P])
                nc.vector.tensor_tensor(
                    out=xcTl[dc][:], in0=pxc[:P, :P], in1=xcTh[dc][:],
                    op=OP.subtract,
                )

            # G = x_cand @ A (3-term bf16, fp32-class)
            pg = ps.tile([P, D], f32, tag="blk")
            n = 0
            for dc in range(ND):
                for lh, rh in (
                    (xcTh[dc][:], Ah[dc][:]),
                    (xcTl[dc][:], Ah[dc][:]),
                    (xcTh[dc][:], Al[dc][:]),
                ):
                    nc.tensor.matmul(
                        pg[:], lh, rh, start=(n == 0), stop=(n == 3 * ND - 1)
                    )
                    n += 1
            gsb = cnd.tile([P, D], f32, tag="gsb")
            nc.scalar.copy(gsb[:], pg[:])
            GTh = [cnd.tile([P, P], bf16, tag=f"GTh{dc}", name=f"GTh{dc}") for dc in range(ND)]
            GTl = [cnd.tile([P, P], bf16, tag=f"GTl{dc}", name=f"GTl{dc}") for dc in range(ND)]
            for dc in range(ND):
                pgt = ps.tile([P, P], f32, tag="blk")
                nc.tensor.transpose(
                    pgt[:P, :P], gsb[:, dc * P : (dc + 1) * P], ident[:]
                )
                nc.vector.tensor_copy(GTh[dc][:], pgt[:P, :P])
                nc.vector.tensor_tensor(
                    out=GTl[dc][:], in0=pgt[:P, :P], in1=GTh[dc][:],
                    op=OP.subtract,
                )

            # S_cand = G @ x^T (3-term bf16), 4 held PSUM blocks
            psS = []
            cmax = cnd.tile([P, NJ], f32, tag="cmax")
            csum = cnd.tile([P, NJ], f32, tag="csum")
            for jb in range(NJ):
                jsl = slice(jb * 512, (jb + 1) * 512)
                pss2 = ps_s.tile([P, 512], f32, tag="psSc")
                psS.append(pss2)
                n = 0
                for dc in range(ND):
                    for lh, rh in (
                        (GTh[dc][:], xTh[dc][:, jsl]),
                        (GTl[dc][:], xTh[dc][:, jsl]),
                        (GTh[dc][:], xTl[dc][:, jsl]),
                    ):
                        nc.tensor.matmul(
                            pss2[:], lh, rh,
                            start=(n == 0), stop=(n == 3 * ND - 1),
                        )
                        n += 1
                s3 = scr.tile([P, 512], f32, tag="scrt2")
                nc.vector.tensor_tensor(
                    out=s3[:], in0=pss2[:], in1=gm[:, jsl], op=OP.mult
                )
                nc.vector.reduce_max(cmax[:, jb : jb + 1], s3[:], axis=AX.X)
                s4 = scr.tile([P, 512], f32, tag="scrt2")
                nc.vector.scalar_tensor_tensor(
                    out=s4[:], in0=pss2[:], scalar=-1.0 / L, in1=gc[:, jsl],
                    op0=OP.mult, op1=OP.mult,
                    accum_out=csum[:, jb : jb + 1],
                )
            u1 = cnd.tile([P, 1], f32, tag="u1")
            u2 = cnd.tile([P, 1], f32, tag="u2")
            M_cand = cnd.tile([P, 1], f32, tag="M_cand")
            nc.vector.reduce_max(u1[:], cmax[:], axis=AX.X)
            nc.vector.reduce_sum(u2[:], csum[:], axis=AX.X)
            nc.vector.tensor_tensor(out=M_cand[:], in0=u1[:], in1=u2[:], op=OP.add)
            nc.vector.copy_predicated(M_cand[:], invalid[:], negbig[:])

            # exact top-40 threshold among candidates
            pmc = ps.tile([1, P], f32, tag="blk")
            nc.tensor.transpose(pmc[:1, :P], M_cand[:], ident[:])
            mcT = cnd.tile([1, P], f32, tag="mcT")
            nc.vector.tensor_copy(mcT[:], pmc[:1, :P])
            etop = cnd.tile([1, NT], f32, tag="etop")
            for r in range(5):
                nc.vector.max(out=etop[:, 8 * r : 8 * r + 8], in_=mcT[:])
                if r < 4:
                    nc.vector.match_replace(
                        out=mcT[:], in_to_replace=etop[:, 8 * r : 8 * r + 8],
                        in_values=mcT[:], imm_value=NEG,
                    )
            pte = ps.tile([P, 1], f32, tag="blk")
            nc.tensor.matmul(
                pte[:P, :1], ones_r1[:], etop[:, NT - 1 : NT], start=True, stop=True
            )
            tebc = cnd.tile([P, 1], f32, tag="tebc")
            nc.vector.tensor_copy(tebc[:], pte[:P, :1])
            sel2 = cnd.tile([P, 1], u8, tag="sel2")
            nc.vector.tensor_tensor(
                out=sel2[:], in0=M_cand[:], in1=tebc[:], op=OP.is_ge
            )
            scat_f = cnd.tile([P, 1], f32, tag="scat_f")
            nc.vector.tensor_copy(scat_f[:], big9[:])
            nc.vector.copy_predicated(scat_f[:], sel2[:], candq_f[:])
            scat_i = cnd.tile([P, 1], i32, tag="scat_i")
            nc.vector.tensor_copy(scat_i[:], scat_f[:])

            # ---------------- phase 5: softmax + update -----------------
            exp_sb = expp.tile([P, L], bf16, tag="exp_sb")
            sume4 = expp.tile([P, NJ], f32, tag="sume4")
            for jb in range(NJ):
                jsl = slice(jb * 512, (jb + 1) * 512)
                nc.scalar.activation(
                    out=exp_sb[:, jsl], in_=psS[jb][:], func=ACTF.Exp,
                    bias=0.0, scale=SCALE,
                    accum_out=sume4[:, jb : jb + 1],
                )
            sume = expp.tile([P, 1], f32, tag="sume")
            nc.vector.reduce_sum(sume[:], sume4[:], axis=AX.X)
            recip = expp.tile([P, 1], f32, tag="recip")
            nc.vector.reciprocal(recip[:], sume[:])

            expT = [expp.tile([P, P], bf16, tag=f"expT{jc}", name=f"expT{jc}") for jc in range(NL)]
            for jc in range(NL):
                pet = psb.tile([P, P], bf16, tag="blkb")
                nc.tensor.transpose(
                    pet[:P, :P], exp_sb[:, jc * P : (jc + 1) * P], ident_b[:]
                )
                if jc % 2 == 0:
                    nc.vector.tensor_copy(expT[jc][:], pet[:P, :P])
                else:
                    nc.scalar.copy(expT[jc][:], pet[:P, :P])

            # G2 = attn_unnorm @ x  (bf16)
            pu = ps.tile([P, D], f32, tag="blk")
            for jc in range(NL):
                nc.tensor.matmul(
                    pu[:], expT[jc][:], xNh[jc][:],
                    start=(jc == 0), stop=(jc == NL - 1),
                )
            g2b = expp.tile([P, D], bf16, tag="g2b")
            nc.scalar.copy(g2b[:], pu[:])
            G2T = [expp.tile([P, P], bf16, tag=f"G2T{dc}", name=f"G2T{dc}") for dc in range(ND)]
            for dc in range(ND):
                pg2 = psb.tile([P, P], bf16, tag="blkb")
                nc.tensor.transpose(
                    pg2[:P, :P], g2b[:, dc * P : (dc + 1) * P], ident_b[:]
                )
                nc.vector.tensor_copy(G2T[dc][:], pg2[:P, :P])
            # upd = G2 @ Wv^T / sums
            pup = ps.tile([P, D], f32, tag="blk")
            for dc in range(ND):
                nc.tensor.matmul(
                    pup[:], G2T[dc][:], wvh[dc][:],
                    start=(dc == 0), stop=(dc == ND - 1),
                )
            upd = expp.tile([P, D], f32, tag="upd")
            nc.scalar.activation(
                out=upd[:], in_=pup[:], func=ACTF.Copy, bias=0.0, scale=recip[:]
            )
            nc.gpsimd.indirect_dma_start(
                out=ctx_d[:],
                out_offset=bass.IndirectOffsetOnAxis(ap=scat_i[:, :1], axis=0),
                in_=upd[:], in_offset=None,
                bounds_check=L - 1, oob_is_err=False,
            )

    nc.compile()
    return nc


_NC = None


def _get_nc():
    global _NC
    if _NC is None:
        _NC = build()
    return _NC


def _split_bf16(a):
    hi = a.astype(ml_dtypes.bfloat16)
    lo = (a - hi.astype(np.float32)).astype(ml_dtypes.bfloat16)
    return hi, lo


def _host_prep(x, Wq, Wk, Wv, index_sample):
    x = np.asarray(x, dtype=np.float32)
    Wq = np.asarray(Wq, dtype=np.float32)
    Wk = np.asarray(Wk, dtype=np.float32)
    Wv = np.asarray(Wv, dtype=np.float32)
    idx = np.asarray(index_sample)

    A = (Wq.T.astype(np.float64) @ Wk.astype(np.float64)).astype(np.float32)
    Ah, Al = _split_bf16(A)
    wvh, wvl = _split_bf16(np.ascontiguousarray(Wv.T))

    rows = np.arange(L)[:, None]
    maskb = np.zeros((L, L), dtype=ml_dtypes.bfloat16)
    maskb[rows, idx] = 1
    countf = np.zeros((L, L), dtype=np.uint8)
    np.add.at(countf, (rows, idx), 1)

    perm16 = np.zeros((16, 8 * P), dtype=np.float32)
    for f in range(8):
        for p in range(16):
            perm16[p, f * P + p + 16 * f] = 1.0
    qidxf = (np.arange(P)[:, None] + 128 * np.arange(NL)[None, :]).astype(np.float32)
    crow = (1.2 + np.arange(NLAD, dtype=np.float32) * 0.066).reshape(1, NLAD)

    shared = {
        "Ah": Ah, "Al": Al, "wvTh": wvh, "wvTl": wvl,
        "maskb": maskb, "countf": countf, "perm16": perm16,
        "qidxf": qidxf, "crow": crow,
    }
    in_maps = []
    for b in range(B):
        xb = np.ascontiguousarray(x[b])
        xT = np.ascontiguousarray(xb.T)
        xth, xtl = _split_bf16(xT)
        xnh = xb.astype(ml_dtypes.bfloat16)
        xmean = xb.astype(np.float64).mean(axis=0).astype(np.float32)
        xmeh, xmel = _split_bf16(xmean.reshape(D, 1))
        in_maps.append(
            {
                "x_nat": xb,
                "xTh": xth,
                "xTl": xtl,
                "xNh": xnh,
                "xmeanTh": xmeh,
                "xmeanTl": xmel,
                **shared,
            }
        )
    return in_maps


def kernel(x, Wq, Wk, Wv, index_sample, _trace=False, _result_box=None):
    in_maps = _host_prep(x, Wq, Wk, Wv, index_sample)
    nc = _get_nc()
    res = run_bass_kernel_spmd(nc, in_maps, core_ids=list(range(B)), trace=_trace)
    if _result_box is not None:
        _result_box.append(res)
    out = np.stack([np.asarray(res.results[b]["ctx"]) for b in range(B)], axis=0)
    return out


# revision 31
# speedup vs baseline: 1.7394x; 1.0584x over previous
"""Sparse attention (ProbSparse-style) Trainium2 Bass kernel, v2.

Problem (per batch element b, data-parallel over 8 NeuronCores):
  Q = x @ Wq.T ; K = x @ Wk.T ; V = x @ Wv.T            [L=2048, D=512]
  QK_sample[l,s] = Q[l] . K[index_sample[l,s]]           [L, 40]
  M[l] = max_s QK_sample - sum_s QK_sample / L
  sel = top40(M)  (as a set; the reference scatter makes order irrelevant)
  scores = Q[sel] @ K.T / sqrt(D); attn = softmax(scores)
  ctx = broadcast(mean(V)); ctx[sel] = attn @ V

Key ideas vs v1 baseline:
  - A = Wq^T @ Wk precomputed on host: S = (x A) x^T. Kills the K and Q
    projections entirely; both approx and exact scores contract against
    the resident x^T tiles.
  - Approx M = masked max of bf16 S only (the sum/L term is <= ~0.5 and
    is absorbed by the candidate margin; validated: true top-40 rows sit
    within rank <= 40 of the approx ordering).
  - Threshold via a 64-step mu + c*sigma ladder with on-device counts
    (one 3d-broadcast compare + reduce + PE column-sum), picking the
    largest T with count >= 88 (fallback: smallest T with count <= 127).
    Replaces the 62us GPSIMD kth_largest.
  - Exact stage on <= 128 candidates: G = x_cand A (3-term bf16),
    S_cand = G x^T (3-term bf16)  ->  ~1e-4-class absolute error,
    validated 26x under the seed-0 top-40 boundary gap.
  - Softmax without max subtraction (|S*scale| <= ~9, exp is safe),
    upd = (attn @ x) @ Wv^T (kills the V projection; V never built).

kernel(**inputs) accepts FULL inputs, returns FULL [8, 2048, 512] f32;
batch is sharded over 8 cores.
"""

import math

import numpy as np
import ml_dtypes

import concourse.bacc as bacc
import concourse.bass as bass
import concourse.mybir as mybir
import concourse.tile as tile
from concourse.bass_utils import run_bass_kernel_spmd
from concourse.masks import make_identity
from concourse import library_config

P = 128
L = 2048
D = 512
B = 8
NL = L // P        # 16 query chunks
ND = D // P        # 4 feature chunks
NJ = L // 512      # 4 key blocks of 512
NT = 40
NLAD = 64          # threshold ladder steps
SCALE = 1.0 / math.sqrt(D)
NEG = -3.0e38
BIG = 3.0e38
SKIP_IDX = 99999.0  # scatter index sentinel (> bounds_check -> row skipped)

f32 = mybir.dt.float32
f16 = mybir.dt.float16
bf16 = mybir.dt.bfloat16
u8 = mybir.dt.uint8
i32 = mybir.dt.int32
u32 = mybir.dt.uint32
AX = mybir.AxisListType
OP = mybir.AluOpType
ACTF = mybir.ActivationFunctionType


def build():
    nc = bacc.Bacc("TRN2", target_bir_lowering=False)

    # All big operands are host-packed into [128, wide] layouts so each
    # DMA partition line is a 4-16KB contiguous DRAM run (1KB lines were
    # descriptor-bound: ~26us of startup).
    x_d = nc.dram_tensor("x_nat", [L, D], f32, kind="ExternalInput")
    xth_d = nc.dram_tensor("xThp", [P, ND * L], bf16, kind="ExternalInput")
    xtl_d = nc.dram_tensor("xTlp", [P, ND * L], bf16, kind="ExternalInput")
    xnh_d = nc.dram_tensor("xNhp", [P, NL * D], bf16, kind="ExternalInput")
    ah_d = nc.dram_tensor("Ahp", [P, ND * D], bf16, kind="ExternalInput")
    al_d = nc.dram_tensor("Alp", [P, ND * D], bf16, kind="ExternalInput")
    wvh_d = nc.dram_tensor("wvThp", [P, ND * D], bf16, kind="ExternalInput")
    wvl_d = nc.dram_tensor("wvTlp", [P, ND * D], bf16, kind="ExternalInput")
    xmh_d = nc.dram_tensor("xmeanTh", [D, 1], bf16, kind="ExternalInput")
    xml_d = nc.dram_tensor("xmeanTl", [D, 1], bf16, kind="ExternalInput")
    maskb_d = nc.dram_tensor("maskb", [L, L], bf16, kind="ExternalInput")
    cnt_d = nc.dram_tensor("countf", [L, L], u8, kind="ExternalInput")
    perm_d = nc.dram_tensor("perm16", [16, 8 * P], f16, kind="ExternalInput")
    qidx_d = nc.dram_tensor("qidxf", [P, NL], f32, kind="ExternalInput")
    crow_d = nc.dram_tensor("crow", [1, NLAD], f32, kind="ExternalInput")
    ctx_d = nc.dram_tensor("ctx", [L, D], f32, kind="ExternalOutput")

    with tile.TileContext(nc) as tc:
        with (
            tc.tile_pool(name="const", bufs=1) as cst,
            tc.tile_pool(name="xres", bufs=1) as xres,      # resident x / A / Wv
            tc.tile_pool(name="proj", bufs=1) as proj,      # QATb
            tc.tile_pool(name="mstuff", bufs=1) as mst,     # M / threshold smalls
            tc.tile_pool(name="mstream", bufs=3) as mstr,   # mask chunks
            tc.tile_pool(name="scr", bufs=3) as scr,        # TTR scratch
            tc.tile_pool(name="cand", bufs=1) as cnd,       # exact-stage tiles
            tc.tile_pool(name="expp", bufs=1) as expp,      # softmax/upd tiles
            tc.tile_pool(name="ps", bufs=2, space="PSUM") as ps,
            tc.tile_pool(name="psb", bufs=2, space="PSUM") as psb,    # bf16 transposes
            tc.tile_pool(name="ps_s", bufs=4, space="PSUM") as ps_s,  # S blocks
        ):
            # ---------------- constants ----------------
            ident = cst.tile([P, P], f32, tag="ident")
            make_identity(nc, ident[:])
            ident_b = cst.tile([P, P], bf16, tag="ident_b")
            nc.vector.tensor_copy(ident_b[:], ident[:])
            # preload the sparse_gather ucode so the serial tail does not
            # pay the library switch
            nc.gpsimd.load_library(library_config.sparse_gather)
            ones_r1 = cst.tile([1, P], f32, tag="ones_r1")
            nc.vector.memset(ones_r1[:], 1.0)
            ones_cf = cst.tile([P, 1], f32, tag="ones_cf")
            nc.vector.memset(ones_cf[:], 1.0)
            negbig = cst.tile([P, 1], f32, tag="negbig")
            nc.vector.memset(negbig[:], NEG)
            big9 = cst.tile([P, 1], f32, tag="big9")
            nc.vector.memset(big9[:], SKIP_IDX)
            qidx_f = cst.tile([P, NL], f32, tag="qidx_f")
            nc.sync.dma_start(qidx_f[:], qidx_d[:])
            crow = cst.tile([1, NLAD], f32, tag="crow")
            nc.sync.dma_start(crow[:], crow_d[:])
            perm16 = cst.tile([16, 8 * P], f16, tag="perm16")
            nc.sync.dma_start(perm16[:], perm_d[:])

            # ---------------- resident loads (packed, big lines) --------
            Ahp = xres.tile([P, ND * D], bf16, tag="Ahp")
            nc.sync.dma_start(Ahp[:], ah_d[:])
            xThp = xres.tile([P, ND * L], bf16, tag="xThp")
            nc.sync.dma_start(xThp[:], xth_d[:])
            Alp = xres.tile([P, ND * D], bf16, tag="Alp")
            nc.sync.dma_start(Alp[:], al_d[:])
            wvhp = xres.tile([P, ND * D], bf16, tag="wvhp")
            nc.sync.dma_start(wvhp[:], wvh_d[:])
            wvlp = xres.tile([P, ND * D], bf16, tag="wvlp")
            nc.sync.dma_start(wvlp[:], wvl_d[:])
            xmh = [xres.tile([P, 1], bf16, tag=f"xmh{dc}", name=f"xmh{dc}") for dc in range(ND)]
            xml = [xres.tile([P, 1], bf16, tag=f"xml{dc}", name=f"xml{dc}") for dc in range(ND)]
            for dc in range(ND):
                sl = slice(dc * P, (dc + 1) * P)
                nc.sync.dma_start(xmh[dc][:], xmh_d[sl, :])
                nc.sync.dma_start(xml[dc][:], xml_d[sl, :])
            xTlp = xres.tile([P, ND * L], bf16, tag="xTlp")
            nc.sync.dma_start(xTlp[:], xtl_d[:])
            xNhp = xres.tile([P, NL * D], bf16, tag="xNhp")
            nc.sync.dma_start(xNhp[:], xnh_d[:])

            # slice helpers over the packed tiles
            Ah = lambda dc, js: Ahp[:, dc * D + js.start : dc * D + js.stop]
            Al = lambda dc, js: Alp[:, dc * D + js.start : dc * D + js.stop]
            wvh = lambda dc, js: wvhp[:, dc * D + js.start : dc * D + js.stop]
            wvl = lambda dc, js: wvlp[:, dc * D + js.start : dc * D + js.stop]
            xTh = lambda dc, js: xThp[:, dc * L + js.start : dc * L + js.stop]
            xTl = lambda dc, js: xTlp[:, dc * L + js.start : dc * L + js.stop]
            xNh = lambda jc: xNhp[:, jc * D : (jc + 1) * D]
            SD = slice(0, D)
            SL = slice(0, L)

            # ---------------- Vmean -> ctx init (early, overlaps all) ----
            pvm = ps.tile([1, D], f32, tag="blk")
            n = 0
            for dc in range(ND):
                for lh, rh in (
                    (xmh[dc][:], wvh(dc, SD)),
                    (xml[dc][:], wvh(dc, SD)),
                    (xmh[dc][:], wvl(dc, SD)),
                ):
                    nc.tensor.matmul(
                        pvm[:1, :], lh, rh,
                        start=(n == 0), stop=(n == 3 * ND - 1),
                    )
                    n += 1
            vmean = mst.tile([1, D], f32, tag="vmean")
            nc.scalar.copy(vmean[:], pvm[:1, :])
            pvb = ps.tile([P, D], f32, tag="blk")
            nc.tensor.matmul(pvb[:], ones_r1[:], vmean[:], start=True, stop=True)
            vmean_bc = mst.tile([P, D], f32, tag="vmean_bc")
            nc.vector.tensor_copy(vmean_bc[:], pvb[:])
            for jc in range(NL):
                nc.sync.dma_start(ctx_d[jc * P : (jc + 1) * P, :], vmean_bc[:])

            # ---------------- phase 1: QA^T = A^T x^T (bf16) ------------
            QATb = [proj.tile([P, L], bf16, tag=f"QATb{ic}", name=f"QATb{ic}") for ic in range(ND)]
            for jb in range(NJ):
                jsl = slice(jb * 512, (jb + 1) * 512)
                for ic in range(ND):
                    isl = slice(ic * P, (ic + 1) * P)
                    pq = ps.tile([P, 512], f32, tag="blk")
                    for dc in range(ND):
                        nc.tensor.matmul(
                            pq[:], Ah(dc, isl), xTh(dc, jsl),
                            start=(dc == 0), stop=(dc == ND - 1),
                        )
                    nc.scalar.copy(QATb[ic][:, jsl], pq[:])

            # ---------------- phase 2: approx S + masked max ------------
            # Per 128-query chunk: PE computes 4 S blocks; ScalarE evicts
            # them to a bf16 row [P, 2048]; DVE does one 2x bf16 mask-mult
            # + one wide reduce_max.  (tensor_tensor_reduce crashes TRN2
            # hardware, so the fused form is not available.)
            M_all = mst.tile([P, NL], f32, tag="M_all")
            for lc in range(NL):
                lsl = slice(lc * P, (lc + 1) * P)
                mkb = mstr.tile([P, L], bf16, tag="mkb")
                nc.sync.dma_start(mkb[:], maskb_d[lsl, :])
                sb1 = scr.tile([P, L], bf16, tag="sb1")
                for jb in range(NJ):
                    jsl = slice(jb * 512, (jb + 1) * 512)
                    pss = ps_s.tile([P, 512], f32, tag="psSc", name="pssa")
                    for ic in range(ND):
                        nc.tensor.matmul(
                            pss[:], QATb[ic][:, lsl], xTh(ic, jsl),
                            start=(ic == 0), stop=(ic == ND - 1),
                        )
                    nc.scalar.copy(sb1[:, jsl], pss[:])
                s1 = scr.tile([P, L], bf16, tag="s1m")
                nc.vector.tensor_tensor(
                    out=s1[:], in0=sb1[:], in1=mkb[:], op=OP.mult
                )
                nc.vector.reduce_max(M_all[:, lc : lc + 1], s1[:], axis=AX.X)

            # ---------------- phase 3: ladder threshold -----------------
            # per-partition sum and sum-of-squares
            stats2 = mst.tile([P, 2], f32, tag="stats2")
            msq = mst.tile([P, NL], f32, tag="msq")
            nc.vector.scalar_tensor_tensor(
                out=msq[:], in0=M_all[:], scalar=1.0, in1=M_all[:],
                op0=OP.mult, op1=OP.mult,
                accum_out=stats2[:, 1:2],
            )
            nc.vector.tensor_reduce(
                stats2[:, 0:1], M_all[:], axis=AX.X, op=OP.add
            )
            pst = ps.tile([1, 2], f32, tag="blk")
            nc.tensor.matmul(pst[:1, :2], ones_cf[:], stats2[:], start=True, stop=True)
            srow = mst.tile([1, 2], f32, tag="srow")
            nc.vector.tensor_copy(srow[:], pst[:1, :2])
            # mu, sigma on [1,1]
            musig = mst.tile([1, 2], f32, tag="musig")
            nc.vector.tensor_scalar_mul(musig[:], srow[:], 1.0 / L)
            mu = musig[:, 0:1]
            mu2 = mst.tile([1, 1], f32, tag="mu2")
            nc.vector.tensor_tensor(out=mu2[:], in0=mu, in1=mu, op=OP.mult)
            var = mst.tile([1, 1], f32, tag="var")
            nc.vector.tensor_tensor(
                out=var[:], in0=musig[:, 1:2], in1=mu2[:], op=OP.subtract
            )
            sigma = mst.tile([1, 1], f32, tag="sigma")
            nc.scalar.sqrt(sigma[:], var[:])
            # Trow = mu + crow * sigma
            Trow = mst.tile([1, NLAD], f32, tag="Trow")
            nc.vector.tensor_tensor(
                out=Trow[:], in0=crow[:], in1=sigma[:].to_broadcast([1, NLAD]),
                op=OP.mult,
            )
            nc.vector.tensor_tensor(
                out=Trow[:], in0=Trow[:], in1=mu.to_broadcast([1, NLAD]),
                op=OP.add,
            )
            # broadcast thresholds to all partitions
            ptb = ps.tile([P, NLAD], f32, tag="blk")
            nc.tensor.matmul(ptb[:P, :NLAD], ones_r1[:], Trow[:], start=True, stop=True)
            Tb = mst.tile([P, NLAD], f32, tag="Tb")
            nc.vector.tensor_copy(Tb[:], ptb[:P, :NLAD])
            # cmp[p, j, f] = M[p, f] >= T[j]  (bf16 so PE can column-sum)
            cmpb = mst.tile([P, NLAD * NL], bf16, tag="cmpb")
            nc.vector.tensor_tensor(
                out=cmpb[:].rearrange("p (j f) -> p j f", f=NL),
                in0=M_all[:].rearrange("p (o f) -> p o f", o=1).to_broadcast([P, NLAD, NL]),
                in1=Tb[:].rearrange("p (j o) -> p j o", o=1).to_broadcast([P, NLAD, NL]),
                op=OP.is_ge,
            )
            cnt01 = mst.tile([P, NLAD], f32, tag="cnt01")
            nc.vector.tensor_reduce(
                cnt01[:], cmpb[:].rearrange("p (j f) -> p j f", f=NL),
                axis=AX.X, op=OP.add,
            )
            pcc = ps.tile([1, NLAD], f32, tag="blk")
            nc.tensor.matmul(pcc[:1, :NLAD], ones_cf[:], cnt01[:], start=True, stop=True)
            cntrow = mst.tile([1, NLAD], f32, tag="cntrow")
            nc.vector.tensor_copy(cntrow[:], pcc[:1, :NLAD])
            # largest T with count >= 88; fallback smallest T with count <= 127
            okm = mst.tile([1, NLAD], u8, tag="okm")
            nc.vector.tensor_scalar(
                okm[:], cntrow[:], 87.5, None, op0=OP.is_ge
            )
            negrow = mst.tile([1, NLAD], f32, tag="negrow")
            nc.vector.memset(negrow[:], NEG)
            bigrow = mst.tile([1, NLAD], f32, tag="bigrow")
            nc.vector.memset(bigrow[:], BIG)
            tsel = mst.tile([1, NLAD], f32, tag="tsel")
            nc.vector.select(tsel[:], okm[:], Trow[:], negrow[:])
            tstar = mst.tile([1, 1], f32, tag="tstar")
            nc.vector.reduce_max(tstar[:], tsel[:], axis=AX.X)
            ok2 = mst.tile([1, NLAD], u8, tag="ok2")
            nc.vector.tensor_scalar(
                ok2[:], cntrow[:], 127.5, None, op0=OP.is_le
            )
            tsel2 = mst.tile([1, NLAD], f32, tag="tsel2")
            nc.vector.select(tsel2[:], ok2[:], Trow[:], bigrow[:])
            tfb = mst.tile([1, 1], f32, tag="tfb")
            nc.vector.tensor_reduce(tfb[:], tsel2[:], axis=AX.X, op=OP.min)
            have = mst.tile([1, 1], u8, tag="have")
            nc.vector.tensor_scalar(
                have[:], tstar[:], -1.0e30, None, op0=OP.is_ge
            )
            tfin = mst.tile([1, 1], f32, tag="tfin")
            nc.vector.select(tfin[:], have[:], tstar[:], tfb[:])
            ptf = ps.tile([P, 1], f32, tag="blk")
            nc.tensor.matmul(ptf[:P, :1], ones_r1[:], tfin[:], start=True, stop=True)
            tbc = mst.tile([P, 1], f32, tag="tbc")
            nc.vector.tensor_copy(tbc[:], ptf[:P, :1])

            # selmask / candidate index compaction
            selmask = mst.tile([P, NL], u8, tag="selmask")
            nc.vector.tensor_scalar(
                selmask[:], M_all[:], tbc[:], 0.0,
                op0=OP.subtract, op1=OP.is_ge,
            )
            midx = mst.tile([P, NL], f32, tag="midx")
            nc.vector.memset(midx[:], -1.0)
            nc.vector.copy_predicated(midx[:], selmask[:], qidx_f[:])
            pwr = ps.tile([16, P], f32, tag="blk", name="pwr")
            nc.tensor.transpose(pwr[:16, :P], midx[:], ident[:])
            wrap_in = mst.tile([16, P], f32, tag="wrap_in")
            nc.vector.tensor_copy(wrap_in[:], pwr[:16, :P])
            spg = mst.tile([16, 8], f32, tag="spg")
            nfound = mst.tile([1, 1], u32, tag="nfound")
            nc.gpsimd.sparse_gather(out=spg[:], in_=wrap_in[:], num_found=nfound[:])
            spg_cl = mst.tile([16, 8], f32, tag="spg_cl")
            nc.vector.tensor_scalar_max(spg_cl[:], spg[:], 0.0)
            nc.vector.tensor_scalar_min(spg_cl[:], spg_cl[:], float(L - 1))
            # fp16 keeps indices <= 2047 exact and avoids the fp32 double
            # LDWEIGHTS cost of the one-hot unwrap
            spg_h = mst.tile([16, 8], f16, tag="spg_h")
            nc.vector.tensor_copy(spg_h[:], spg_cl[:])
            pcq = ps.tile([P, 1], f32, tag="blk", name="pcq")
            for f in range(8):
                nc.tensor.matmul(
                    pcq[:P, :1], perm16[:, f * P : (f + 1) * P],
                    spg_h[:, f : f + 1],
                    start=(f == 0), stop=(f == 7),
                )
            candq_f = mst.tile([P, 1], f32, tag="candq_f")
            nc.vector.tensor_copy(candq_f[:], pcq[:P, :1])
            candq_i = mst.tile([P, 1], i32, tag="candq_i")
            nc.vector.tensor_copy(candq_i[:], pcq[:P, :1])
            nf_f = mst.tile([1, 1], f32, tag="nf_f")
            nc.vector.tensor_copy(nf_f[:], nfound[:])
            pnb = ps.tile([P, 1], f32, tag="blk")
            nc.tensor.matmul(pnb[:P, :1], ones_r1[:], nf_f[:], start=True, stop=True)
            nbc = mst.tile([P, 1], f32, tag="nbc")
            nc.vector.tensor_copy(nbc[:], pnb[:P, :1])
            invalid = mst.tile([P, 1], u8, tag="invalid")
            nc.vector.tensor_tensor(
                out=invalid[:], in0=qidx_f[:, 0:1], in1=nbc[:], op=OP.is_ge
            )

            # Keep-warm: ~3.5us of throwaway matmuls gated on candq_h so
            # they run exactly during the gather window; a >3.4us PE idle
            # here would drop the HAM clock to 1.2GHz for the whole exact
            # stage.
            candq_h = mst.tile([P, 1], bf16, tag="candq_h")
            nc.vector.tensor_copy(candq_h[:], pcq[:P, :1])
            pwarm = ps.tile([1, 512], f32, tag="blk", name="pwarm")
            for w in range(14):
                nc.tensor.matmul(
                    pwarm[:1, :512], candq_h[:, :1], xThp[:, 0:512],
                    start=True, stop=True,
                )

            # ---------------- phase 4: exact stage ----------------------
            x_cand = cnd.tile([P, D], f32, tag="x_cand")
            nc.gpsimd.indirect_dma_start(
                out=x_cand[:], out_offset=None, in_=x_d[:],
                in_offset=bass.IndirectOffsetOnAxis(ap=candq_i[:, :1], axis=0),
            )
            gm = cnd.tile([P, L], bf16, tag="gm")
            nc.gpsimd.indirect_dma_start(
                out=gm[:], out_offset=None, in_=maskb_d[:],
                in_offset=bass.IndirectOffsetOnAxis(ap=candq_i[:, :1], axis=0),
            )
            gc = cnd.tile([P, L], u8, tag="gc")
            nc.gpsimd.indirect_dma_start(
                out=gc[:], out_offset=None, in_=cnt_d[:],
                in_offset=bass.IndirectOffsetOnAxis(ap=candq_i[:, :1], axis=0),
            )

            # x_cand^T hi/lo
            xcTh = [cnd.tile([P, P], bf16, tag=f"xcTh{dc}", name=f"xcTh{dc}") for dc in range(ND)]
            xcTl = [cnd.tile([P, P], bf16, tag=f"xcTl{dc}", name=f"xcTl{dc}") for dc in range(ND)]
            for dc in range(ND):
                pxc = ps.tile([P, P], f32, tag="blk")
                nc.tensor.transpose(
                    pxc[:P, :P], x_cand[:, dc * P : (dc + 1) * P], ident[:]
                )
                nc.vector.tensor_copy(xcTh[dc][:], pxc[:P, :P])
                nc.vector.tensor_tensor(
                    out=xcTl[dc][:], in0=pxc[:P, :P], in1=xcTh[dc][:],
                    op=OP.subtract,
                )

            # G = x_cand @ A (3-term bf16, fp32-class)
            pg = ps.tile([P, D], f32, tag="blk")
            n = 0
            for dc in range(ND):
                for lh, rh in (
                    (xcTh[dc][:], Ah(dc, SD)),
                    (xcTl[dc][:], Ah(dc, SD)),
                    (xcTh[dc][:], Al(dc, SD)),
                ):
                    nc.tensor.matmul(
                        pg[:], lh, rh, start=(n == 0), stop=(n == 3 * ND - 1)
                    )
                    n += 1
            gsb = cnd.tile([P, D], f32, tag="gsb")
            nc.scalar.copy(gsb[:], pg[:])
            GTh = [cnd.tile([P, P], bf16, tag=f"GTh{dc}", name=f"GTh{dc}") for dc in range(ND)]
            GTl = [cnd.tile([P, P], bf16, tag=f"GTl{dc}", name=f"GTl{dc}") for dc in range(ND)]
            for dc in range(ND):
                pgt = ps.tile([P, P], f32, tag="blk")
                nc.tensor.transpose(
                    pgt[:P, :P], gsb[:, dc * P : (dc + 1) * P], ident[:]
                )
                nc.vector.tensor_copy(GTh[dc][:], pgt[:P, :P])
                nc.vector.tensor_tensor(
                    out=GTl[dc][:], in0=pgt[:P, :P], in1=GTh[dc][:],
                    op=OP.subtract,
                )

            # S_cand = G @ x^T (3-term bf16), 4 held PSUM blocks
            psS = []
            cmax = cnd.tile([P, NJ], f32, tag="cmax")
            csum = cnd.tile([P, NJ], f32, tag="csum")
            for jb in range(NJ):
                jsl = slice(jb * 512, (jb + 1) * 512)
                pss2 = ps_s.tile([P, 512], f32, tag="psSc")
                psS.append(pss2)
                n = 0
                for dc in range(ND):
                    for lh, rh in (
                        (GTh[dc][:], xTh(dc, jsl)),
                        (GTl[dc][:], xTh(dc, jsl)),
                        (GTh[dc][:], xTl(dc, jsl)),
                    ):
                        nc.tensor.matmul(
                            pss2[:], lh, rh,
                            start=(n == 0), stop=(n == 3 * ND - 1),
                        )
                        n += 1
                s3 = scr.tile([P, 512], f32, tag="scrt2")
                nc.vector.tensor_tensor(
                    out=s3[:], in0=pss2[:], in1=gm[:, jsl], op=OP.mult
                )
                nc.vector.reduce_max(cmax[:, jb : jb + 1], s3[:], axis=AX.X)
                s4 = scr.tile([P, 512], f32, tag="scrt2")
                nc.vector.scalar_tensor_tensor(
                    out=s4[:], in0=pss2[:], scalar=-1.0 / L, in1=gc[:, jsl],
                    op0=OP.mult, op1=OP.mult,
                    accum_out=csum[:, jb : jb + 1],
                )
            u1 = cnd.tile([P, 1], f32, tag="u1")
            u2 = cnd.tile([P, 1], f32, tag="u2")
            M_cand = cnd.tile([P, 1], f32, tag="M_cand")
            nc.vector.reduce_max(u1[:], cmax[:], axis=AX.X)
            nc.vector.reduce_sum(u2[:], csum[:], axis=AX.X)
            nc.vector.tensor_tensor(out=M_cand[:], in0=u1[:], in1=u2[:], op=OP.add)
            nc.vector.copy_predicated(M_cand[:], invalid[:], negbig[:])

            # exact top-40 threshold among candidates
            pmc = ps.tile([1, P], f32, tag="blk")
            nc.tensor.transpose(pmc[:1, :P], M_cand[:], ident[:])
            mcT = cnd.tile([1, P], f32, tag="mcT")
            nc.vector.tensor_copy(mcT[:], pmc[:1, :P])
            etop = cnd.tile([1, NT], f32, tag="etop")
            for r in range(5):
                nc.vector.max(out=etop[:, 8 * r : 8 * r + 8], in_=mcT[:])
                if r < 4:
                    nc.vector.match_replace(
                        out=mcT[:], in_to_replace=etop[:, 8 * r : 8 * r + 8],
                        in_values=mcT[:], imm_value=NEG,
                    )
            pte = ps.tile([P, 1], f32, tag="blk")
            nc.tensor.matmul(
                pte[:P, :1], ones_r1[:], etop[:, NT - 1 : NT], start=True, stop=True
            )
            tebc = cnd.tile([P, 1], f32, tag="tebc")
            nc.vector.tensor_copy(tebc[:], pte[:P, :1])
            sel2 = cnd.tile([P, 1], u8, tag="sel2")
            nc.vector.tensor_tensor(
                out=sel2[:], in0=M_cand[:], in1=tebc[:], op=OP.is_ge
            )
            scat_f = cnd.tile([P, 1], f32, tag="scat_f")
            nc.vector.tensor_copy(scat_f[:], big9[:])
            nc.vector.copy_predicated(scat_f[:], sel2[:], candq_f[:])
            scat_i = cnd.tile([P, 1], i32, tag="scat_i")
            nc.vector.tensor_copy(scat_i[:], scat_f[:])

            # ---------------- phase 5: softmax + update -----------------
            exp_sb = expp.tile([P, L], bf16, tag="exp_sb")
            sume4 = expp.tile([P, NJ], f32, tag="sume4")
            for jb in range(NJ):
                jsl = slice(jb * 512, (jb + 1) * 512)
                nc.scalar.activation(
                    out=exp_sb[:, jsl], in_=psS[jb][:], func=ACTF.Exp,
                    bias=0.0, scale=SCALE,
                    accum_out=sume4[:, jb : jb + 1],
                )
            sume = expp.tile([P, 1], f32, tag="sume")
            nc.vector.reduce_sum(sume[:], sume4[:], axis=AX.X)
            recip = expp.tile([P, 1], f32, tag="recip")
            nc.vector.reciprocal(recip[:], sume[:])

            expT = [expp.tile([P, P], bf16, tag=f"expT{jc}", name=f"expT{jc}") for jc in range(NL)]
            for jc in range(NL):
                pet = psb.tile([P, P], bf16, tag="blkb")
                nc.tensor.transpose(
                    pet[:P, :P], exp_sb[:, jc * P : (jc + 1) * P], ident_b[:]
                )
                if jc % 2 == 0:
                    nc.vector.tensor_copy(expT[jc][:], pet[:P, :P])
                else:
                    nc.scalar.copy(expT[jc][:], pet[:P, :P])

            # G2 = attn_unnorm @ x  (bf16)
            pu = ps.tile([P, D], f32, tag="blk")
            for jc in range(NL):
                nc.tensor.matmul(
                    pu[:], expT[jc][:], xNh(jc),
                    start=(jc == 0), stop=(jc == NL - 1),
                )
            g2b = expp.tile([P, D], bf16, tag="g2b")
            nc.scalar.copy(g2b[:], pu[:])
            G2T = [expp.tile([P, P], bf16, tag=f"G2T{dc}", name=f"G2T{dc}") for dc in range(ND)]
            for dc in range(ND):
                pg2 = psb.tile([P, P], bf16, tag="blkb")
                nc.tensor.transpose(
                    pg2[:P, :P], g2b[:, dc * P : (dc + 1) * P], ident_b[:]
                )
                nc.vector.tensor_copy(G2T[dc][:], pg2[:P, :P])
            # upd = G2 @ Wv^T / sums
            pup = ps.tile([P, D], f32, tag="blk")
            for dc in range(ND):
                nc.tensor.matmul(
                    pup[:], G2T[dc][:], wvh(dc, SD),
                    start=(dc == 0), stop=(dc == ND - 1),
                )
            upd = expp.tile([P, D], f32, tag="upd")
            nc.scalar.activation(
                out=upd[:], in_=pup[:], func=ACTF.Copy, bias=0.0, scale=recip[:]
            )
            nc.gpsimd.indirect_dma_start(
                out=ctx_d[:],
                out_offset=bass.IndirectOffsetOnAxis(ap=scat_i[:, :1], axis=0),
                in_=upd[:], in_offset=None,
                bounds_check=L - 1, oob_is_err=False,
            )

    nc.compile()
    return nc


_NC = None


def _get_nc():
    global _NC
    if _NC is None:
        _NC = build()
    return _NC


def _split_bf16(a):
    hi = a.astype(ml_dtypes.bfloat16)
    lo = (a - hi.astype(np.float32)).astype(ml_dtypes.bfloat16)
    return hi, lo


def _host_prep(x, Wq, Wk, Wv, index_sample):
    x = np.asarray(x, dtype=np.float32)
    Wq = np.asarray(Wq, dtype=np.float32)
    Wk = np.asarray(Wk, dtype=np.float32)
    Wv = np.asarray(Wv, dtype=np.float32)
    idx = np.asarray(index_sample)

    def pack(m):
        # [ND*P, W] -> [P, ND*W]: row dc*128+p lands at columns dc*W..+W
        nd = m.shape[0] // P
        return np.ascontiguousarray(
            m.reshape(nd, P, m.shape[1]).transpose(1, 0, 2).reshape(P, -1)
        )

    A = (Wq.T.astype(np.float64) @ Wk.astype(np.float64)).astype(np.float32)
    Ah, Al = _split_bf16(A)
    wvh, wvl = _split_bf16(np.ascontiguousarray(Wv.T))

    rows = np.arange(L)[:, None]
    maskb = np.zeros((L, L), dtype=ml_dtypes.bfloat16)
    maskb[rows, idx] = 1
    countf = np.zeros((L, L), dtype=np.uint8)
    np.add.at(countf, (rows, idx), 1)

    perm16 = np.zeros((16, 8 * P), dtype=np.float16)
    for f in range(8):
        for p in range(16):
            perm16[p, f * P + p + 16 * f] = 1.0
    qidxf = (np.arange(P)[:, None] + 128 * np.arange(NL)[None, :]).astype(np.float32)
    crow = (1.2 + np.arange(NLAD, dtype=np.float32) * 0.066).reshape(1, NLAD)

    shared = {
        "Ahp": pack(Ah), "Alp": pack(Al),
        "wvThp": pack(wvh), "wvTlp": pack(wvl),
        "maskb": maskb, "countf": countf, "perm16": perm16,
        "qidxf": qidxf, "crow": crow,
    }
    in_maps = []
    for b in range(B):
        xb = np.ascontiguousarray(x[b])
        xT = np.ascontiguousarray(xb.T)
        xth, xtl = _split_bf16(xT)
        xnh = xb.astype(ml_dtypes.bfloat16)
        xmean = xb.astype(np.float64).mean(axis=0).astype(np.float32)
        xmeh, xmel = _split_bf16(xmean.reshape(D, 1))
        in_maps.append(
            {
                "x_nat": xb,
                "xThp": pack(xth),
                "xTlp": pack(xtl),
                "xNhp": pack(xnh),
                "xmeanTh": xmeh,
                "xmeanTl": xmel,
                **shared,
            }
        )
    return in_maps


def kernel(x, Wq, Wk, Wv, index_sample, _trace=False, _result_box=None):
    in_maps = _host_prep(x, Wq, Wk, Wv, index_sample)
    nc = _get_nc()
    res = run_bass_kernel_spmd(nc, in_maps, core_ids=list(range(B)), trace=_trace)
    if _result_box is not None:
        _result_box.append(res)
    out = np.stack([np.asarray(res.results[b]["ctx"]) for b in range(B)], axis=0)
    return out


# revision 41
# speedup vs baseline: 1.9102x; 1.0982x over previous
"""Sparse attention (ProbSparse-style) Trainium2 Bass kernel, v2.

Problem (per batch element b, data-parallel over 8 NeuronCores):
  Q = x @ Wq.T ; K = x @ Wk.T ; V = x @ Wv.T            [L=2048, D=512]
  QK_sample[l,s] = Q[l] . K[index_sample[l,s]]           [L, 40]
  M[l] = max_s QK_sample - sum_s QK_sample / L
  sel = top40(M)  (as a set; the reference scatter makes order irrelevant)
  scores = Q[sel] @ K.T / sqrt(D); attn = softmax(scores)
  ctx = broadcast(mean(V)); ctx[sel] = attn @ V

Key ideas vs v1 baseline:
  - A = Wq^T @ Wk precomputed on host: S = (x A) x^T. Kills the K and Q
    projections entirely; both approx and exact scores contract against
    the resident x^T tiles.
  - Approx M = masked max of bf16 S only (the sum/L term is <= ~0.5 and
    is absorbed by the candidate margin; validated: true top-40 rows sit
    within rank <= 40 of the approx ordering).
  - Threshold via a 64-step mu + c*sigma ladder with on-device counts
    (one 3d-broadcast compare + reduce + PE column-sum), picking the
    largest T with count >= 88 (fallback: smallest T with count <= 127).
    Replaces the 62us GPSIMD kth_largest.
  - Exact stage on <= 128 candidates: G = x_cand A (3-term bf16),
    S_cand = G x^T (3-term bf16)  ->  ~1e-4-class absolute error,
    validated 26x under the seed-0 top-40 boundary gap.
  - Softmax without max subtraction (|S*scale| <= ~9, exp is safe),
    upd = (attn @ x) @ Wv^T (kills the V projection; V never built).

kernel(**inputs) accepts FULL inputs, returns FULL [8, 2048, 512] f32;
batch is sharded over 8 cores.
"""

import math

import numpy as np
import ml_dtypes

import concourse.bacc as bacc
import concourse.bass as bass
import concourse.mybir as mybir
import concourse.tile as tile
from concourse.bass_utils import run_bass_kernel_spmd
from concourse.masks import make_identity
from concourse import library_config

P = 128
L = 2048
D = 512
B = 8
NL = L // P        # 16 query chunks
ND = D // P        # 4 feature chunks
NJ = L // 512      # 4 key blocks of 512
NT = 40
NLAD = 32          # threshold ladder steps
SCALE = 1.0 / math.sqrt(D)
NEG = -3.0e38
BIG = 3.0e38
SKIP_IDX = 99999.0  # scatter index sentinel (> bounds_check -> row skipped)

f32 = mybir.dt.float32
f16 = mybir.dt.float16
bf16 = mybir.dt.bfloat16
u8 = mybir.dt.uint8
i32 = mybir.dt.int32
u32 = mybir.dt.uint32
AX = mybir.AxisListType
OP = mybir.AluOpType
ACTF = mybir.ActivationFunctionType


def build():
    nc = bacc.Bacc("TRN2", target_bir_lowering=False)

    # All big operands are host-packed into [128, wide] layouts so each
    # DMA partition line is a 4-16KB contiguous DRAM run (1KB lines were
    # descriptor-bound: ~26us of startup).
    x_d = nc.dram_tensor("x_nat", [L, D], f32, kind="ExternalInput")
    xth_d = nc.dram_tensor("xThp", [P, ND * L], bf16, kind="ExternalInput")
    xtl_d = nc.dram_tensor("xTlp", [P, ND * L], bf16, kind="ExternalInput")
    xnh_d = nc.dram_tensor("xNhp", [P, NL * D], bf16, kind="ExternalInput")
    ah_d = nc.dram_tensor("Ahp", [P, ND * D], bf16, kind="ExternalInput")
    af_d = nc.dram_tensor("Afp", [P, ND * D], f32, kind="ExternalInput")
    wvh_d = nc.dram_tensor("wvThp", [P, ND * D], bf16, kind="ExternalInput")
    wvl_d = nc.dram_tensor("wvTlp", [P, ND * D], bf16, kind="ExternalInput")
    xm_d = nc.dram_tensor("xmp", [P, 2 * ND], bf16, kind="ExternalInput")
    maskb_d = nc.dram_tensor("maskb", [L, L], bf16, kind="ExternalInput")
    cnt_d = nc.dram_tensor("countf", [L, L], u8, kind="ExternalInput")
    perm_d = nc.dram_tensor("perm16", [16, 8 * P], f16, kind="ExternalInput")
    qidx_d = nc.dram_tensor("qidxf", [P, NL], f32, kind="ExternalInput")
    crow_d = nc.dram_tensor("crow", [1, NLAD], f32, kind="ExternalInput")
    ctx_d = nc.dram_tensor("ctx", [L, D], f32, kind="ExternalOutput")

    with tile.TileContext(nc) as tc:
        with (
            tc.tile_pool(name="const", bufs=1) as cst,
            tc.tile_pool(name="xres", bufs=1) as xres,      # resident x / A / Wv
            tc.tile_pool(name="proj", bufs=1) as proj,      # QATb
            tc.tile_pool(name="mstuff", bufs=1) as mst,     # M / threshold smalls
            tc.tile_pool(name="mstream", bufs=3) as mstr,   # mask chunks
            tc.tile_pool(name="scr", bufs=3) as scr,        # TTR scratch
            tc.tile_pool(name="cand", bufs=1) as cnd,       # exact-stage tiles
            tc.tile_pool(name="expp", bufs=1) as expp,      # softmax/upd tiles
            tc.tile_pool(name="ps", bufs=2, space="PSUM") as ps,
            tc.tile_pool(name="psb", bufs=2, space="PSUM") as psb,    # bf16 transposes
            tc.tile_pool(name="ps_s", bufs=4, space="PSUM") as ps_s,  # S blocks
        ):
            # ---------------- constants ----------------
            ident = cst.tile([P, P], f32, tag="ident")
            make_identity(nc, ident[:])
            ident_b = cst.tile([P, P], bf16, tag="ident_b")
            nc.vector.tensor_copy(ident_b[:], ident[:])
            # preload the sparse_gather ucode so the serial tail does not
            # pay the library switch
            nc.gpsimd.load_library(library_config.sparse_gather)
            ones_r1 = cst.tile([1, P], f32, tag="ones_r1")
            nc.vector.memset(ones_r1[:], 1.0)
            ones_cf = cst.tile([P, 1], f32, tag="ones_cf")
            nc.vector.memset(ones_cf[:], 1.0)
            negbig = cst.tile([P, 1], f32, tag="negbig")
            nc.vector.memset(negbig[:], NEG)
            big9 = cst.tile([P, 1], f32, tag="big9")
            nc.vector.memset(big9[:], SKIP_IDX)
            qidx_f = cst.tile([P, NL], f32, tag="qidx_f")
            nc.sync.dma_start(qidx_f[:], qidx_d[:])
            crow = cst.tile([1, NLAD], f32, tag="crow")
            nc.sync.dma_start(crow[:], crow_d[:])
            perm16 = cst.tile([16, 8 * P], f16, tag="perm16")
            nc.sync.dma_start(perm16[:], perm_d[:])

            # ---------------- critical loads (packed, big lines) --------
            # Only Ahp + xThp gate the first matmuls; everything the tail
            # needs is DMA'd from inside the phase-2 loop so it doesn't
            # compete for startup bandwidth.
            Ahp = xres.tile([P, ND * D], bf16, tag="Ahp")
            nc.sync.dma_start(Ahp[:], ah_d[:])
            xThp = xres.tile([P, ND * L], bf16, tag="xThp")
            nc.sync.dma_start(xThp[:], xth_d[:])
            Afp = xres.tile([P, ND * D], f32, tag="Afp")
            wvhp = xres.tile([P, ND * D], bf16, tag="wvhp")
            wvlp = xres.tile([P, ND * D], bf16, tag="wvlp")
            xmp = xres.tile([P, 2 * ND], bf16, tag="xmp")
            xTlp = xres.tile([P, ND * L], bf16, tag="xTlp")
            xNhp = xres.tile([P, NL * D], bf16, tag="xNhp")

            # slice helpers over the packed tiles
            Ah = lambda dc, js: Ahp[:, dc * D + js.start : dc * D + js.stop]
            Af = lambda dc, js: Afp[:, dc * D + js.start : dc * D + js.stop]
            wvh = lambda dc, js: wvhp[:, dc * D + js.start : dc * D + js.stop]
            wvl = lambda dc, js: wvlp[:, dc * D + js.start : dc * D + js.stop]
            xTh = lambda dc, js: xThp[:, dc * L + js.start : dc * L + js.stop]
            xTl = lambda dc, js: xTlp[:, dc * L + js.start : dc * L + js.stop]
            xNh = lambda jc: xNhp[:, jc * D : (jc + 1) * D]
            SD = slice(0, D)
            SL = slice(0, L)

            # ---------------- phase 1: QA^T = A^T x^T (bf16) ------------
            QATb = [proj.tile([P, L], bf16, tag=f"QATb{ic}", name=f"QATb{ic}") for ic in range(ND)]
            for jb in range(NJ):
                jsl = slice(jb * 512, (jb + 1) * 512)
                for ic in range(ND):
                    isl = slice(ic * P, (ic + 1) * P)
                    pq = ps.tile([P, 512], f32, tag="blk")
                    for dc in range(ND):
                        nc.tensor.matmul(
                            pq[:], Ah(dc, isl), xTh(dc, jsl),
                            start=(dc == 0), stop=(dc == ND - 1),
                        )
                    nc.scalar.copy(QATb[ic][:, jsl], pq[:])

            # ---------------- phase 2: approx S + masked max ------------
            # Per 128-query chunk: PE computes 4 S blocks; ScalarE evicts
            # them to a bf16 row [P, 2048]; DVE does one 2x bf16 mask-mult
            # + one wide reduce_max.  (tensor_tensor_reduce crashes TRN2
            # hardware, so the fused form is not available.)
            M_all = mst.tile([P, NL], f32, tag="M_all")
            M_lo = mst.tile([P, 8], f32, tag="M_lo")
            Trow = mst.tile([1, NLAD], f32, tag="Trow")
            Tb = mst.tile([P, NLAD], bf16, tag="Tb")
            for lc in range(NL):
                lsl = slice(lc * P, (lc + 1) * P)
                mkb = mstr.tile([P, L], bf16, tag="mkb")
                nc.sync.dma_start(mkb[:], maskb_d[lsl, :])
                # tail-only loads trickled in behind the mask stream
                if lc == 0:
                    nc.sync.dma_start(wvhp[:], wvh_d[:])
                    nc.sync.dma_start(wvlp[:], wvl_d[:])
                    nc.sync.dma_start(xmp[:], xm_d[:])
                elif lc == 2:
                    nc.sync.dma_start(Afp[:], af_d[:])
                elif lc == 5:
                    nc.sync.dma_start(xTlp[:], xtl_d[:])
                elif lc == 9:
                    nc.sync.dma_start(xNhp[:], xnh_d[:])
                sb1 = scr.tile([P, L], bf16, tag="sb1")
                for jb in range(NJ):
                    jsl = slice(jb * 512, (jb + 1) * 512)
                    pss = ps_s.tile([P, 512], f32, tag="psSc", name="pssa")
                    for ic in range(ND):
                        nc.tensor.matmul(
                            pss[:], QATb[ic][:, lsl], xTh(ic, jsl),
                            start=(ic == 0), stop=(ic == ND - 1),
                        )
                    nc.scalar.copy(sb1[:, jsl], pss[:])
                s1 = scr.tile([P, L], bf16, tag="s1m")
                nc.vector.tensor_tensor(
                    out=s1[:], in0=sb1[:], in1=mkb[:], op=OP.mult
                )
                if lc < 8:
                    nc.vector.reduce_max(M_lo[:, lc : lc + 1], s1[:], axis=AX.X)
                    nc.vector.tensor_copy(
                        M_all[:, lc : lc + 1], M_lo[:, lc : lc + 1]
                    )
                else:
                    nc.vector.reduce_max(M_all[:, lc : lc + 1], s1[:], axis=AX.X)
                if lc == 7:
                    # ---- early threshold stats on the first 1024 rows --
                    # (mu/sigma only steer the ladder range; the counts
                    # below verify against the full M) -- this whole chain
                    # runs under the second half of the main phase.
                    stats2 = mst.tile([P, 2], f32, tag="stats2")
                    msq = mst.tile([P, 8], f32, tag="msq")
                    nc.vector.scalar_tensor_tensor(
                        out=msq[:], in0=M_lo[:], scalar=1.0, in1=M_lo[:],
                        op0=OP.mult, op1=OP.mult,
                        accum_out=stats2[:, 1:2],
                    )
                    nc.vector.tensor_reduce(
                        stats2[:, 0:1], M_lo[:], axis=AX.X, op=OP.add
                    )
                    pst = ps.tile([1, 2], f32, tag="blk")
                    nc.tensor.matmul(
                        pst[:1, :2], ones_cf[:], stats2[:], start=True, stop=True
                    )
                    srow = mst.tile([1, 2], f32, tag="srow")
                    nc.vector.tensor_copy(srow[:], pst[:1, :2])
                    musig = mst.tile([1, 2], f32, tag="musig")
                    nc.vector.tensor_scalar_mul(musig[:], srow[:], 1.0 / 1024.0)
                    mu = musig[:, 0:1]
                    mu2 = mst.tile([1, 1], f32, tag="mu2")
                    nc.vector.tensor_tensor(out=mu2[:], in0=mu, in1=mu, op=OP.mult)
                    var = mst.tile([1, 1], f32, tag="var")
                    nc.vector.tensor_tensor(
                        out=var[:], in0=musig[:, 1:2], in1=mu2[:], op=OP.subtract
                    )
                    sigma = mst.tile([1, 1], f32, tag="sigma")
                    nc.scalar.sqrt(sigma[:], var[:])
                    nc.vector.tensor_tensor(
                        out=Trow[:], in0=crow[:],
                        in1=sigma[:].to_broadcast([1, NLAD]), op=OP.mult,
                    )
                    nc.vector.tensor_tensor(
                        out=Trow[:], in0=Trow[:], in1=mu.to_broadcast([1, NLAD]),
                        op=OP.add,
                    )
                    ptb = ps.tile([P, NLAD], f32, tag="blk")
                    nc.tensor.matmul(
                        ptb[:P, :NLAD], ones_r1[:], Trow[:], start=True, stop=True
                    )
                    nc.vector.tensor_copy(Tb[:], ptb[:P, :NLAD])

            # ---------------- phase 3: ladder counts --------------------
            M_b = mst.tile([P, NL], bf16, tag="M_b")
            nc.vector.tensor_copy(M_b[:], M_all[:])
            # cmp[p, j, f] = M[p, f] >= T[j]  (bf16 in/out -> 2x DVE, and
            # PE can column-sum the bf16 result)
            cmpb = mst.tile([P, NLAD * NL], bf16, tag="cmpb")
            nc.vector.tensor_tensor(
                out=cmpb[:].rearrange("p (j f) -> p j f", f=NL),
                in0=M_b[:].rearrange("p (o f) -> p o f", o=1).to_broadcast([P, NLAD, NL]),
                in1=Tb[:].rearrange("p (j o) -> p j o", o=1).to_broadcast([P, NLAD, NL]),
                op=OP.is_ge,
            )
            # ---------------- Vmean -> ctx init (PE idle slot) ----------
            pvm = ps.tile([1, D], f32, tag="blk")
            n = 0
            for dc in range(ND):
                for lh, rh in (
                    (xmp[:, dc : dc + 1], wvh(dc, SD)),
                    (xmp[:, ND + dc : ND + dc + 1], wvh(dc, SD)),
                    (xmp[:, dc : dc + 1], wvl(dc, SD)),
                ):
                    nc.tensor.matmul(
                        pvm[:1, :], lh, rh,
                        start=(n == 0), stop=(n == 3 * ND - 1),
                    )
                    n += 1
            vmean = mst.tile([1, D], f32, tag="vmean")
            nc.scalar.copy(vmean[:], pvm[:1, :])
            pvb = ps.tile([P, D], f32, tag="blk")
            nc.tensor.matmul(pvb[:], ones_r1[:], vmean[:], start=True, stop=True)
            vmean_bc = mst.tile([P, D], f32, tag="vmean_bc")
            nc.vector.tensor_copy(vmean_bc[:], pvb[:])
            for jc in range(NL):
                nc.sync.dma_start(ctx_d[jc * P : (jc + 1) * P, :], vmean_bc[:])

            cnt01 = mst.tile([P, NLAD], f32, tag="cnt01")
            nc.vector.tensor_reduce(
                cnt01[:], cmpb[:].rearrange("p (j f) -> p j f", f=NL),
                axis=AX.X, op=OP.add,
            )
            pcc = ps.tile([1, NLAD], f32, tag="blk")
            nc.tensor.matmul(pcc[:1, :NLAD], ones_cf[:], cnt01[:], start=True, stop=True)
            cntrow = mst.tile([1, NLAD], f32, tag="cntrow")
            nc.vector.tensor_copy(cntrow[:], pcc[:1, :NLAD])
            # largest T with count >= 88; fallback smallest T with count <= 127
            okm = mst.tile([1, NLAD], u8, tag="okm")
            nc.vector.tensor_scalar(
                okm[:], cntrow[:], 87.5, None, op0=OP.is_ge
            )
            negrow = mst.tile([1, NLAD], f32, tag="negrow")
            nc.vector.memset(negrow[:], NEG)
            bigrow = mst.tile([1, NLAD], f32, tag="bigrow")
            nc.vector.memset(bigrow[:], BIG)
            tsel = mst.tile([1, NLAD], f32, tag="tsel")
            nc.vector.select(tsel[:], okm[:], Trow[:], negrow[:])
            tstar = mst.tile([1, 1], f32, tag="tstar")
            nc.vector.reduce_max(tstar[:], tsel[:], axis=AX.X)
            ok2 = mst.tile([1, NLAD], u8, tag="ok2")
            nc.vector.tensor_scalar(
                ok2[:], cntrow[:], 127.5, None, op0=OP.is_le
            )
            tsel2 = mst.tile([1, NLAD], f32, tag="tsel2")
            nc.vector.select(tsel2[:], ok2[:], Trow[:], bigrow[:])
            tfb = mst.tile([1, 1], f32, tag="tfb")
            nc.vector.tensor_reduce(tfb[:], tsel2[:], axis=AX.X, op=OP.min)
            have = mst.tile([1, 1], u8, tag="have")
            nc.vector.tensor_scalar(
                have[:], tstar[:], -1.0e30, None, op0=OP.is_ge
            )
            tfin = mst.tile([1, 1], f32, tag="tfin")
            nc.vector.select(tfin[:], have[:], tstar[:], tfb[:])
            ptf = ps.tile([P, 1], f32, tag="blk")
            nc.tensor.matmul(ptf[:P, :1], ones_r1[:], tfin[:], start=True, stop=True)
            tbc = mst.tile([P, 1], f32, tag="tbc")
            nc.vector.tensor_copy(tbc[:], ptf[:P, :1])

            # selmask / candidate index compaction
            selmask = mst.tile([P, NL], u8, tag="selmask")
            nc.vector.tensor_scalar(
                selmask[:], M_all[:], tbc[:], 0.0,
                op0=OP.subtract, op1=OP.is_ge,
            )
            midx = mst.tile([P, NL], f32, tag="midx")
            nc.vector.memset(midx[:], -1.0)
            nc.vector.copy_predicated(midx[:], selmask[:], qidx_f[:])
            pwr = ps.tile([16, P], f32, tag="blk", name="pwr")
            nc.tensor.transpose(pwr[:16, :P], midx[:], ident[:])
            wrap_in = mst.tile([16, P], f32, tag="wrap_in")
            nc.vector.tensor_copy(wrap_in[:], pwr[:16, :P])
            spg = mst.tile([16, 8], f32, tag="spg")
            nfound = mst.tile([1, 1], u32, tag="nfound")
            nc.gpsimd.sparse_gather(out=spg[:], in_=wrap_in[:], num_found=nfound[:])
            spg_cl = mst.tile([16, 8], f32, tag="spg_cl")
            nc.vector.tensor_scalar_max(spg_cl[:], spg[:], 0.0)
            nc.vector.tensor_scalar_min(spg_cl[:], spg_cl[:], float(L - 1))
            # fp16 keeps indices <= 2047 exact and avoids the fp32 double
            # LDWEIGHTS cost of the one-hot unwrap
            spg_h = mst.tile([16, 8], f16, tag="spg_h")
            nc.vector.tensor_copy(spg_h[:], spg_cl[:])
            pcq = ps.tile([P, 1], f32, tag="blk", name="pcq")
            for f in range(8):
                nc.tensor.matmul(
                    pcq[:P, :1], perm16[:, f * P : (f + 1) * P],
                    spg_h[:, f : f + 1],
                    start=(f == 0), stop=(f == 7),
                )
            candq_f = mst.tile([P, 1], f32, tag="candq_f")
            nc.vector.tensor_copy(candq_f[:], pcq[:P, :1])
            candq_i = mst.tile([P, 1], i32, tag="candq_i")
            nc.vector.tensor_copy(candq_i[:], pcq[:P, :1])
            nf_f = mst.tile([1, 1], f32, tag="nf_f")
            nc.vector.tensor_copy(nf_f[:], nfound[:])
            pnb = ps.tile([P, 1], f32, tag="blk")
            nc.tensor.matmul(pnb[:P, :1], ones_r1[:], nf_f[:], start=True, stop=True)
            nbc = mst.tile([P, 1], f32, tag="nbc")
            nc.vector.tensor_copy(nbc[:], pnb[:P, :1])
            invalid = mst.tile([P, 1], u8, tag="invalid")
            nc.vector.tensor_tensor(
                out=invalid[:], in0=qidx_f[:, 0:1], in1=nbc[:], op=OP.is_ge
            )

            # Keep-warm: ~3.5us of throwaway matmuls gated on candq_h so
            # they run exactly during the gather window; a >3.4us PE idle
            # here would drop the HAM clock to 1.2GHz for the whole exact
            # stage.
            candq_h = mst.tile([P, 1], bf16, tag="candq_h")
            nc.vector.tensor_copy(candq_h[:], pcq[:P, :1])
            pwarm = ps.tile([1, 512], f32, tag="blk", name="pwarm")
            for w in range(14):
                nc.tensor.matmul(
                    pwarm[:1, :512], candq_h[:, :1], xThp[:, 0:512],
                    start=True, stop=True,
                )

            # ---------------- phase 4: exact stage ----------------------
            x_cand = cnd.tile([P, D], f32, tag="x_cand")
            nc.gpsimd.indirect_dma_start(
                out=x_cand[:], out_offset=None, in_=x_d[:],
                in_offset=bass.IndirectOffsetOnAxis(ap=candq_i[:, :1], axis=0),
            )
            gm = cnd.tile([P, L], bf16, tag="gm")
            nc.gpsimd.indirect_dma_start(
                out=gm[:], out_offset=None, in_=maskb_d[:],
                in_offset=bass.IndirectOffsetOnAxis(ap=candq_i[:, :1], axis=0),
            )
            gc = cnd.tile([P, L], u8, tag="gc")
            nc.gpsimd.indirect_dma_start(
                out=gc[:], out_offset=None, in_=cnt_d[:],
                in_offset=bass.IndirectOffsetOnAxis(ap=candq_i[:, :1], axis=0),
            )

            # x_cand^T (fp32 — exact G via fp32 matmul, no hi/lo casts)
            xcT = [cnd.tile([P, P], f32, tag=f"xcT{dc}", name=f"xcT{dc}") for dc in range(ND)]
            for dc in range(ND):
                pxc = ps.tile([P, P], f32, tag="blk")
                nc.tensor.transpose(
                    pxc[:P, :P], x_cand[:, dc * P : (dc + 1) * P], ident[:]
                )
                nc.vector.tensor_copy(xcT[dc][:], pxc[:P, :P])

            # G = x_cand @ A (full fp32)
            pg = ps.tile([P, D], f32, tag="blk")
            for dc in range(ND):
                nc.tensor.matmul(
                    pg[:], xcT[dc][:], Af(dc, SD),
                    start=(dc == 0), stop=(dc == ND - 1),
                )
            gsb = cnd.tile([P, D], f32, tag="gsb")
            nc.vector.tensor_copy(gsb[:], pg[:])
            GTh = [cnd.tile([P, P], bf16, tag=f"GTh{dc}", name=f"GTh{dc}") for dc in range(ND)]
            GTl = [cnd.tile([P, P], bf16, tag=f"GTl{dc}", name=f"GTl{dc}") for dc in range(ND)]
            for dc in range(ND):
                pgt = ps.tile([P, P], f32, tag="blk")
                nc.tensor.transpose(
                    pgt[:P, :P], gsb[:, dc * P : (dc + 1) * P], ident[:]
                )
                nc.vector.tensor_copy(GTh[dc][:], pgt[:P, :P])
                nc.vector.tensor_tensor(
                    out=GTl[dc][:], in0=pgt[:P, :P], in1=GTh[dc][:],
                    op=OP.subtract,
                )

            # S_cand = G @ x^T (3-term bf16), 4 held PSUM blocks
            psS = []
            cmax = cnd.tile([P, NJ], f32, tag="cmax")
            csum = cnd.tile([P, NJ], f32, tag="csum")
            for jb in range(NJ):
                jsl = slice(jb * 512, (jb + 1) * 512)
                pss2 = ps_s.tile([P, 512], f32, tag="psSc")
                psS.append(pss2)
                n = 0
                for dc in range(ND):
                    for lh, rh in (
                        (GTh[dc][:], xTh(dc, jsl)),
                        (GTl[dc][:], xTh(dc, jsl)),
                        (GTh[dc][:], xTl(dc, jsl)),
                    ):
                        nc.tensor.matmul(
                            pss2[:], lh, rh,
                            start=(n == 0), stop=(n == 3 * ND - 1),
                        )
                        n += 1
                s3 = scr.tile([P, 512], f32, tag="scrt2")
                nc.vector.tensor_tensor(
                    out=s3[:], in0=pss2[:], in1=gm[:, jsl], op=OP.mult
                )
                nc.vector.reduce_max(cmax[:, jb : jb + 1], s3[:], axis=AX.X)
                s4 = scr.tile([P, 512], f32, tag="scrt2")
                nc.vector.scalar_tensor_tensor(
                    out=s4[:], in0=pss2[:], scalar=-1.0 / L, in1=gc[:, jsl],
                    op0=OP.mult, op1=OP.mult,
                    accum_out=csum[:, jb : jb + 1],
                )
            u1 = cnd.tile([P, 1], f32, tag="u1")
            u2 = cnd.tile([P, 1], f32, tag="u2")
            M_cand = cnd.tile([P, 1], f32, tag="M_cand")
            nc.vector.reduce_max(u1[:], cmax[:], axis=AX.X)
            nc.vector.reduce_sum(u2[:], csum[:], axis=AX.X)
            nc.vector.tensor_tensor(out=M_cand[:], in0=u1[:], in1=u2[:], op=OP.add)
            nc.vector.copy_predicated(M_cand[:], invalid[:], negbig[:])

            # exact top-40 threshold among candidates
            pmc = ps.tile([1, P], f32, tag="blk")
            nc.tensor.transpose(pmc[:1, :P], M_cand[:], ident[:])
            mcT = cnd.tile([1, P], f32, tag="mcT")
            nc.vector.tensor_copy(mcT[:], pmc[:1, :P])
            etop = cnd.tile([1, NT], f32, tag="etop")
            for r in range(5):
                nc.vector.max(out=etop[:, 8 * r : 8 * r + 8], in_=mcT[:])
                if r < 4:
                    nc.vector.match_replace(
                        out=mcT[:], in_to_replace=etop[:, 8 * r : 8 * r + 8],
                        in_values=mcT[:], imm_value=NEG,
                    )
            pte = ps.tile([P, 1], f32, tag="blk")
            nc.tensor.matmul(
                pte[:P, :1], ones_r1[:], etop[:, NT - 1 : NT], start=True, stop=True
            )
            tebc = cnd.tile([P, 1], f32, tag="tebc")
            nc.vector.tensor_copy(tebc[:], pte[:P, :1])
            sel2 = cnd.tile([P, 1], u8, tag="sel2")
            nc.vector.tensor_tensor(
                out=sel2[:], in0=M_cand[:], in1=tebc[:], op=OP.is_ge
            )
            scat_f = cnd.tile([P, 1], f32, tag="scat_f")
            nc.vector.tensor_copy(scat_f[:], big9[:])
            nc.vector.copy_predicated(scat_f[:], sel2[:], candq_f[:])
            scat_i = cnd.tile([P, 1], i32, tag="scat_i")
            nc.vector.tensor_copy(scat_i[:], scat_f[:])

            # ---------------- phase 5: softmax + update -----------------
            exp_sb = expp.tile([P, L], bf16, tag="exp_sb")
            sume4 = expp.tile([P, NJ], f32, tag="sume4")
            for jb in range(NJ):
                jsl = slice(jb * 512, (jb + 1) * 512)
                nc.scalar.activation(
                    out=exp_sb[:, jsl], in_=psS[jb][:], func=ACTF.Exp,
                    bias=0.0, scale=SCALE,
                    accum_out=sume4[:, jb : jb + 1],
                )
            sume = expp.tile([P, 1], f32, tag="sume")
            nc.vector.reduce_sum(sume[:], sume4[:], axis=AX.X)
            recip = expp.tile([P, 1], f32, tag="recip")
            nc.vector.reciprocal(recip[:], sume[:])

            # expT transposes software-pipelined with the G2 accumulation
            # (depth 4) so the PE never idles long enough to re-throttle
            expT = [expp.tile([P, P], bf16, tag=f"expT{jc}", name=f"expT{jc}") for jc in range(NL)]
            pu = ps.tile([P, D], f32, tag="blk")

            def g2_mm(jc):
                nc.tensor.matmul(
                    pu[:], expT[jc][:], xNh(jc),
                    start=(jc == 0), stop=(jc == NL - 1),
                    skip_group_check=True,
                )

            for jc in range(NL):
                pet = psb.tile([P, P], bf16, tag="blkb")
                nc.tensor.transpose(
                    pet[:P, :P], exp_sb[:, jc * P : (jc + 1) * P], ident_b[:]
                )
                if jc % 2 == 0:
                    nc.vector.tensor_copy(expT[jc][:], pet[:P, :P])
                else:
                    nc.scalar.copy(expT[jc][:], pet[:P, :P])
                if jc >= 3:
                    g2_mm(jc - 3)
            for jc in range(NL - 3, NL):
                g2_mm(jc)
            g2b = expp.tile([P, D], bf16, tag="g2b")
            nc.scalar.copy(g2b[:], pu[:])
            G2T = [expp.tile([P, P], bf16, tag=f"G2T{dc}", name=f"G2T{dc}") for dc in range(ND)]
            for dc in range(ND):
                pg2 = psb.tile([P, P], bf16, tag="blkb")
                nc.tensor.transpose(
                    pg2[:P, :P], g2b[:, dc * P : (dc + 1) * P], ident_b[:]
                )
                nc.vector.tensor_copy(G2T[dc][:], pg2[:P, :P])
            # upd = G2 @ Wv^T / sums
            pup = ps.tile([P, D], f32, tag="blk")
            for dc in range(ND):
                nc.tensor.matmul(
                    pup[:], G2T[dc][:], wvh(dc, SD),
                    start=(dc == 0), stop=(dc == ND - 1),
                )
            upd = expp.tile([P, D], f32, tag="upd")
            nc.scalar.activation(
                out=upd[:], in_=pup[:], func=ACTF.Copy, bias=0.0, scale=recip[:]
            )
            nc.gpsimd.indirect_dma_start(
                out=ctx_d[:],
                out_offset=bass.IndirectOffsetOnAxis(ap=scat_i[:, :1], axis=0),
                in_=upd[:], in_offset=None,
                bounds_check=L - 1, oob_is_err=False,
            )

    nc.compile()
    return nc


_NC = None


def _get_nc():
    global _NC
    if _NC is None:
        _NC = build()
    return _NC


def _split_bf16(a):
    hi = a.astype(ml_dtypes.bfloat16)
    lo = (a - hi.astype(np.float32)).astype(ml_dtypes.bfloat16)
    return hi, lo


def _host_prep(x, Wq, Wk, Wv, index_sample):
    x = np.asarray(x, dtype=np.float32)
    Wq = np.asarray(Wq, dtype=np.float32)
    Wk = np.asarray(Wk, dtype=np.float32)
    Wv = np.asarray(Wv, dtype=np.float32)
    idx = np.asarray(index_sample)

    def pack(m):
        # [ND*P, W] -> [P, ND*W]: row dc*128+p lands at columns dc*W..+W
        nd = m.shape[0] // P
        return np.ascontiguousarray(
            m.reshape(nd, P, m.shape[1]).transpose(1, 0, 2).reshape(P, -1)
        )

    A = (Wq.T.astype(np.float64) @ Wk.astype(np.float64)).astype(np.float32)
    Ah = A.astype(ml_dtypes.bfloat16)
    wvh, wvl = _split_bf16(np.ascontiguousarray(Wv.T))

    rows = np.arange(L)[:, None]
    maskb = np.zeros((L, L), dtype=ml_dtypes.bfloat16)
    maskb[rows, idx] = 1
    countf = np.zeros((L, L), dtype=np.uint8)
    np.add.at(countf, (rows, idx), 1)

    perm16 = np.zeros((16, 8 * P), dtype=np.float16)
    for f in range(8):
        for p in range(16):
            perm16[p, f * P + p + 16 * f] = 1.0
    qidxf = (np.arange(P)[:, None] + 128 * np.arange(NL)[None, :]).astype(np.float32)
    crow = (1.2 + np.arange(NLAD, dtype=np.float32) * 0.134).reshape(1, NLAD)

    shared = {
        "Ahp": pack(Ah), "Afp": pack(A),
        "wvThp": pack(wvh), "wvTlp": pack(wvl),
        "maskb": maskb, "countf": countf, "perm16": perm16,
        "qidxf": qidxf, "crow": crow,
    }
    in_maps = []
    for b in range(B):
        xb = np.ascontiguousarray(x[b])
        xT = np.ascontiguousarray(xb.T)
        xth, xtl = _split_bf16(xT)
        xnh = xb.astype(ml_dtypes.bfloat16)
        xmean = xb.astype(np.float64).mean(axis=0).astype(np.float32)
        xmeh, xmel = _split_bf16(xmean.reshape(1, D))
        xm = np.concatenate(
            [xmeh.reshape(ND, P).T, xmel.reshape(ND, P).T], axis=1
        ).astype(ml_dtypes.bfloat16)
        in_maps.append(
            {
                "x_nat": xb,
                "xThp": pack(xth),
                "xTlp": pack(xtl),
                "xNhp": pack(xnh),
                "xmp": np.ascontiguousarray(xm),
                **shared,
            }
        )
    return in_maps


def kernel(x, Wq, Wk, Wv, index_sample, _trace=False, _result_box=None):
    in_maps = _host_prep(x, Wq, Wk, Wv, index_sample)
    nc = _get_nc()
    res = run_bass_kernel_spmd(nc, in_maps, core_ids=list(range(B)), trace=_trace)
    if _result_box is not None:
        _result_box.append(res)
    out = np.stack([np.asarray(res.results[b]["ctx"]) for b in range(B)], axis=0)
    return out


# revision 49
# speedup vs baseline: 1.9358x; 1.0134x over previous
"""Sparse attention (ProbSparse-style) Trainium2 Bass kernel, v2.

Problem (per batch element b, data-parallel over 8 NeuronCores):
  Q = x @ Wq.T ; K = x @ Wk.T ; V = x @ Wv.T            [L=2048, D=512]
  QK_sample[l,s] = Q[l] . K[index_sample[l,s]]           [L, 40]
  M[l] = max_s QK_sample - sum_s QK_sample / L
  sel = top40(M)  (as a set; the reference scatter makes order irrelevant)
  scores = Q[sel] @ K.T / sqrt(D); attn = softmax(scores)
  ctx = broadcast(mean(V)); ctx[sel] = attn @ V

Key ideas vs v1 baseline:
  - A = Wq^T @ Wk precomputed on host: S = (x A) x^T. Kills the K and Q
    projections entirely; both approx and exact scores contract against
    the resident x^T tiles.
  - Approx M = masked max of bf16 S only (the sum/L term is <= ~0.5 and
    is absorbed by the candidate margin; validated: true top-40 rows sit
    within rank <= 40 of the approx ordering).
  - Threshold via a 64-step mu + c*sigma ladder with on-device counts
    (one 3d-broadcast compare + reduce + PE column-sum), picking the
    largest T with count >= 88 (fallback: smallest T with count <= 127).
    Replaces the 62us GPSIMD kth_largest.
  - Exact stage on <= 128 candidates: G = x_cand A (3-term bf16),
    S_cand = G x^T (3-term bf16)  ->  ~1e-4-class absolute error,
    validated 26x under the seed-0 top-40 boundary gap.
  - Softmax without max subtraction (|S*scale| <= ~9, exp is safe),
    upd = (attn @ x) @ Wv^T (kills the V projection; V never built).

kernel(**inputs) accepts FULL inputs, returns FULL [8, 2048, 512] f32;
batch is sharded over 8 cores.
"""

import math

import numpy as np
import ml_dtypes

import concourse.bacc as bacc
import concourse.bass as bass
import concourse.mybir as mybir
import concourse.tile as tile
from concourse.bass_utils import run_bass_kernel_spmd
from concourse.masks import make_identity
from concourse import library_config

P = 128
L = 2048
D = 512
B = 8
NL = L // P        # 16 query chunks
ND = D // P        # 4 feature chunks
NJ = L // 512      # 4 key blocks of 512
NT = 40
NLAD = 32          # threshold ladder steps
SCALE = 1.0 / math.sqrt(D)
NEG = -3.0e38
BIG = 3.0e38
SKIP_IDX = 99999.0  # scatter index sentinel (> bounds_check -> row skipped)

f32 = mybir.dt.float32
f16 = mybir.dt.float16
bf16 = mybir.dt.bfloat16
u8 = mybir.dt.uint8
i32 = mybir.dt.int32
u32 = mybir.dt.uint32
AX = mybir.AxisListType
OP = mybir.AluOpType
ACTF = mybir.ActivationFunctionType


def build():
    nc = bacc.Bacc("TRN2", target_bir_lowering=False)

    # All big operands are host-packed into [128, wide] layouts so each
    # DMA partition line is a 4-16KB contiguous DRAM run (1KB lines were
    # descriptor-bound: ~26us of startup).
    x_d = nc.dram_tensor("x_nat", [L, D], f32, kind="ExternalInput")
    xth_d = nc.dram_tensor("xThp", [P, ND * L], bf16, kind="ExternalInput")
    xtl_d = nc.dram_tensor("xTlp", [P, ND * L], bf16, kind="ExternalInput")
    xnh_d = nc.dram_tensor("xNhp", [P, NL * D], bf16, kind="ExternalInput")
    ah_d = nc.dram_tensor("Ahp", [P, ND * D], bf16, kind="ExternalInput")
    af_d = nc.dram_tensor("Afp", [P, ND * D], f32, kind="ExternalInput")
    wvh_d = nc.dram_tensor("wvThp", [P, ND * D], bf16, kind="ExternalInput")
    wvl_d = nc.dram_tensor("wvTlp", [P, ND * D], bf16, kind="ExternalInput")
    xm_d = nc.dram_tensor("xmp", [P, 2 * ND], bf16, kind="ExternalInput")
    maskb_d = nc.dram_tensor("maskb", [L, L], bf16, kind="ExternalInput")
    cnt_d = nc.dram_tensor("countf", [L, L], u8, kind="ExternalInput")
    perm_d = nc.dram_tensor("perm16", [16, 8 * P], f16, kind="ExternalInput")
    qidx_d = nc.dram_tensor("qidxf", [P, NL], f32, kind="ExternalInput")
    crow_d = nc.dram_tensor("crow", [1, NLAD], f32, kind="ExternalInput")
    ctx_d = nc.dram_tensor("ctx", [L, D], f32, kind="ExternalOutput")

    with tile.TileContext(nc) as tc:
        with (
            tc.tile_pool(name="const", bufs=1) as cst,
            tc.tile_pool(name="xres", bufs=1) as xres,      # resident x / A / Wv
            tc.tile_pool(name="proj", bufs=1) as proj,      # QATb
            tc.tile_pool(name="mstuff", bufs=1) as mst,     # M / threshold smalls
            tc.tile_pool(name="mstream", bufs=3) as mstr,   # mask chunks
            tc.tile_pool(name="scr", bufs=3) as scr,        # TTR scratch
            tc.tile_pool(name="cand", bufs=1) as cnd,       # exact-stage tiles
            tc.tile_pool(name="expp", bufs=1) as expp,      # softmax/upd tiles
            tc.tile_pool(name="ps", bufs=2, space="PSUM") as ps,
            tc.tile_pool(name="psb", bufs=2, space="PSUM") as psb,    # bf16 transposes
            tc.tile_pool(name="ps_s", bufs=2, space="PSUM") as ps_s,  # S pairs
        ):
            # ---------------- constants ----------------
            ident = cst.tile([P, P], f32, tag="ident")
            make_identity(nc, ident[:])
            ident_b = cst.tile([P, P], bf16, tag="ident_b")
            nc.vector.tensor_copy(ident_b[:], ident[:])
            # preload the sparse_gather ucode so the serial tail does not
            # pay the library switch
            nc.gpsimd.load_library(library_config.sparse_gather)
            ones_r1 = cst.tile([1, P], f32, tag="ones_r1")
            nc.vector.memset(ones_r1[:], 1.0)
            ones_cf = cst.tile([P, 1], f32, tag="ones_cf")
            nc.vector.memset(ones_cf[:], 1.0)
            negbig = cst.tile([P, 1], f32, tag="negbig")
            nc.vector.memset(negbig[:], NEG)
            big9 = cst.tile([P, 1], f32, tag="big9")
            nc.vector.memset(big9[:], SKIP_IDX)
            qidx_f = cst.tile([P, NL], f32, tag="qidx_f")
            nc.sync.dma_start(qidx_f[:], qidx_d[:])
            crow = cst.tile([1, NLAD], f32, tag="crow")
            nc.sync.dma_start(crow[:], crow_d[:])
            perm16 = cst.tile([16, 8 * P], f16, tag="perm16")
            nc.sync.dma_start(perm16[:], perm_d[:])

            # ---------------- critical loads (packed, big lines) --------
            # Only Ahp + xThp gate the first matmuls; everything the tail
            # needs is DMA'd from inside the phase-2 loop so it doesn't
            # compete for startup bandwidth.
            Ahp = xres.tile([P, ND * D], bf16, tag="Ahp")
            nc.sync.dma_start(Ahp[:], ah_d[:])
            # x^T hi is packed jb-major: block jb holds [dc=0..3][512 cols]
            # so the first QA matmuls start after Ahp + one 0.5MB block
            xThp = xres.tile([P, ND * L], bf16, tag="xThp")
            for jb in range(NJ):
                nc.sync.dma_start(
                    xThp[:, jb * 2048 : (jb + 1) * 2048],
                    xth_d[:, jb * 2048 : (jb + 1) * 2048],
                )
            Afp = xres.tile([P, ND * D], f32, tag="Afp")
            wvhp = xres.tile([P, ND * D], bf16, tag="wvhp")
            wvlp = xres.tile([P, ND * D], bf16, tag="wvlp")
            xmp = xres.tile([P, 2 * ND], bf16, tag="xmp")
            xTlp = xres.tile([P, ND * L], bf16, tag="xTlp")
            xNhp = xres.tile([P, NL * D], bf16, tag="xNhp")

            # slice helpers over the packed tiles
            Ah = lambda dc, js: Ahp[:, dc * D + js.start : dc * D + js.stop]
            Af = lambda dc, js: Afp[:, dc * D + js.start : dc * D + js.stop]
            wvh = lambda dc, js: wvhp[:, dc * D + js.start : dc * D + js.stop]
            wvl = lambda dc, js: wvlp[:, dc * D + js.start : dc * D + js.stop]

            def _xt(tile_, dc, js):
                # jb-major packing: block jb*2048 + dc*512
                jb, r = divmod(js.start, 512)
                assert js.stop - js.start == 512 and r == 0
                off = jb * 2048 + dc * 512
                return tile_[:, off : off + 512]

            xTh = lambda dc, js: _xt(xThp, dc, js)
            xTl = lambda dc, js: _xt(xTlp, dc, js)
            xNh = lambda jc: xNhp[:, jc * D : (jc + 1) * D]
            SD = slice(0, D)
            SL = slice(0, L)

            # ---------------- phase 1: QA^T = A^T x^T (bf16) ------------
            QATb = [proj.tile([P, L], bf16, tag=f"QATb{ic}", name=f"QATb{ic}") for ic in range(ND)]
            for jb in range(NJ):
                jsl = slice(jb * 512, (jb + 1) * 512)
                for ic in range(ND):
                    isl = slice(ic * P, (ic + 1) * P)
                    pq = ps.tile([P, 512], f32, tag="blk")
                    for dc in range(ND):
                        nc.tensor.matmul(
                            pq[:], Ah(dc, isl), xTh(dc, jsl),
                            start=(dc == 0), stop=(dc == ND - 1),
                        )
                    nc.scalar.copy(QATb[ic][:, jsl], pq[:])

            # ---------------- phase 2: approx S + masked max ------------
            # Per 128-query chunk: PE computes 4 S blocks; ScalarE evicts
            # them to a bf16 row [P, 2048]; DVE does one 2x bf16 mask-mult
            # + one wide reduce_max.  (tensor_tensor_reduce crashes TRN2
            # hardware, so the fused form is not available.)
            M_all = mst.tile([P, NL], f32, tag="M_all")
            M_lo = mst.tile([P, 8], f32, tag="M_lo")
            Trow = mst.tile([1, NLAD], f32, tag="Trow")
            Tb = mst.tile([P, NLAD], bf16, tag="Tb")
            for lc in range(NL):
                lsl = slice(lc * P, (lc + 1) * P)
                mkb = mstr.tile([P, L], bf16, tag="mkb")
                nc.sync.dma_start(mkb[:], maskb_d[lsl, :])
                # tail-only loads trickled in behind the mask stream
                if lc == 0:
                    nc.sync.dma_start(wvhp[:], wvh_d[:])
                    nc.sync.dma_start(wvlp[:], wvl_d[:])
                    nc.sync.dma_start(xmp[:], xm_d[:])
                elif lc == 2:
                    nc.sync.dma_start(Afp[:], af_d[:])
                elif lc == 5:
                    nc.sync.dma_start(xTlp[:], xtl_d[:])
                elif lc == 9:
                    nc.sync.dma_start(xNhp[:], xnh_d[:])
                sb1 = scr.tile([P, L], bf16, tag="sb1")
                for jp in range(2):
                    # paired PSUM banks -> one wide eviction per 1024 cols
                    pss = ps_s.tile([P, 1024], f32, tag="psSc", name="pssa")
                    for jh in range(2):
                        jb = jp * 2 + jh
                        jsl = slice(jb * 512, (jb + 1) * 512)
                        for ic in range(ND):
                            nc.tensor.matmul(
                                pss[:, jh * 512 : (jh + 1) * 512],
                                QATb[ic][:, lsl], xTh(ic, jsl),
                                start=(ic == 0), stop=(ic == ND - 1),
                            )
                    nc.scalar.copy(
                        sb1[:, jp * 1024 : (jp + 1) * 1024], pss[:]
                    )
                # masked max: two 2x-mode masked products, one 2x max
                # combine, then a half-width 1x reduce
                t0 = scr.tile([P, 1024], bf16, tag="t0m")
                nc.vector.tensor_tensor(
                    out=t0[:], in0=sb1[:, 0:1024], in1=mkb[:, 0:1024],
                    op=OP.mult,
                )
                t1 = scr.tile([P, 1024], bf16, tag="t1m")
                nc.vector.tensor_tensor(
                    out=t1[:], in0=sb1[:, 1024:2048], in1=mkb[:, 1024:2048],
                    op=OP.mult,
                )
                t2 = scr.tile([P, 1024], bf16, tag="t2m")
                nc.vector.tensor_tensor(
                    out=t2[:], in0=t0[:], in1=t1[:], op=OP.max
                )
                if lc < 8:
                    nc.vector.reduce_max(M_lo[:, lc : lc + 1], t2[:], axis=AX.X)
                    nc.vector.tensor_copy(
                        M_all[:, lc : lc + 1], M_lo[:, lc : lc + 1]
                    )
                else:
                    nc.vector.reduce_max(M_all[:, lc : lc + 1], t2[:], axis=AX.X)
                if lc == 7:
                    # ---- early threshold stats on the first 1024 rows --
                    # (mu/sigma only steer the ladder range; the counts
                    # below verify against the full M) -- this whole chain
                    # runs under the second half of the main phase.
                    stats2 = mst.tile([P, 2], f32, tag="stats2")
                    msq = mst.tile([P, 8], f32, tag="msq")
                    nc.vector.scalar_tensor_tensor(
                        out=msq[:], in0=M_lo[:], scalar=1.0, in1=M_lo[:],
                        op0=OP.mult, op1=OP.mult,
                        accum_out=stats2[:, 1:2],
                    )
                    nc.vector.tensor_reduce(
                        stats2[:, 0:1], M_lo[:], axis=AX.X, op=OP.add
                    )
                    pst = ps.tile([1, 2], f32, tag="blk")
                    nc.tensor.matmul(
                        pst[:1, :2], ones_cf[:], stats2[:], start=True, stop=True
                    )
                    srow = mst.tile([1, 2], f32, tag="srow")
                    nc.vector.tensor_copy(srow[:], pst[:1, :2])
                    musig = mst.tile([1, 2], f32, tag="musig")
                    nc.vector.tensor_scalar_mul(musig[:], srow[:], 1.0 / 1024.0)
                    mu = musig[:, 0:1]
                    mu2 = mst.tile([1, 1], f32, tag="mu2")
                    nc.vector.tensor_tensor(out=mu2[:], in0=mu, in1=mu, op=OP.mult)
                    var = mst.tile([1, 1], f32, tag="var")
                    nc.vector.tensor_tensor(
                        out=var[:], in0=musig[:, 1:2], in1=mu2[:], op=OP.subtract
                    )
                    sigma = mst.tile([1, 1], f32, tag="sigma")
                    nc.scalar.sqrt(sigma[:], var[:])
                    nc.vector.tensor_tensor(
                        out=Trow[:], in0=crow[:],
                        in1=sigma[:].to_broadcast([1, NLAD]), op=OP.mult,
                    )
                    nc.vector.tensor_tensor(
                        out=Trow[:], in0=Trow[:], in1=mu.to_broadcast([1, NLAD]),
                        op=OP.add,
                    )
                    ptb = ps.tile([P, NLAD], f32, tag="blk")
                    nc.tensor.matmul(
                        ptb[:P, :NLAD], ones_r1[:], Trow[:], start=True, stop=True
                    )
                    nc.vector.tensor_copy(Tb[:], ptb[:P, :NLAD])

            # ---------------- phase 3: ladder counts --------------------
            M_b = mst.tile([P, NL], bf16, tag="M_b")
            nc.vector.tensor_copy(M_b[:], M_all[:])
            # cmp[p, j, f] = M[p, f] >= T[j]  (bf16 in/out -> 2x DVE, and
            # PE can column-sum the bf16 result)
            cmpb = mst.tile([P, NLAD * NL], bf16, tag="cmpb")
            nc.vector.tensor_tensor(
                out=cmpb[:].rearrange("p (j f) -> p j f", f=NL),
                in0=M_b[:].rearrange("p (o f) -> p o f", o=1).to_broadcast([P, NLAD, NL]),
                in1=Tb[:].rearrange("p (j o) -> p j o", o=1).to_broadcast([P, NLAD, NL]),
                op=OP.is_ge,
            )
            # ---------------- Vmean -> ctx init (PE idle slot) ----------
            pvm = ps.tile([1, D], f32, tag="blk")
            n = 0
            for dc in range(ND):
                for lh, rh in (
                    (xmp[:, dc : dc + 1], wvh(dc, SD)),
                    (xmp[:, ND + dc : ND + dc + 1], wvh(dc, SD)),
                    (xmp[:, dc : dc + 1], wvl(dc, SD)),
                ):
                    nc.tensor.matmul(
                        pvm[:1, :], lh, rh,
                        start=(n == 0), stop=(n == 3 * ND - 1),
                    )
                    n += 1
            vmean = mst.tile([1, D], f32, tag="vmean")
            nc.scalar.copy(vmean[:], pvm[:1, :])
            pvb = ps.tile([P, D], f32, tag="blk")
            nc.tensor.matmul(pvb[:], ones_r1[:], vmean[:], start=True, stop=True)
            vmean_bc = mst.tile([P, D], f32, tag="vmean_bc")
            nc.vector.tensor_copy(vmean_bc[:], pvb[:])
            for jc in range(NL):
                nc.sync.dma_start(ctx_d[jc * P : (jc + 1) * P, :], vmean_bc[:])

            cnt01 = mst.tile([P, NLAD], f32, tag="cnt01")
            nc.vector.tensor_reduce(
                cnt01[:], cmpb[:].rearrange("p (j f) -> p j f", f=NL),
                axis=AX.X, op=OP.add,
            )
            pcc = ps.tile([1, NLAD], f32, tag="blk")
            nc.tensor.matmul(pcc[:1, :NLAD], ones_cf[:], cnt01[:], start=True, stop=True)
            cntrow = mst.tile([1, NLAD], f32, tag="cntrow")
            nc.vector.tensor_copy(cntrow[:], pcc[:1, :NLAD])
            # largest T with count >= 88; fallback smallest T with count <= 127
            okm = mst.tile([1, NLAD], u8, tag="okm")
            nc.vector.tensor_scalar(
                okm[:], cntrow[:], 87.5, None, op0=OP.is_ge
            )
            negrow = mst.tile([1, NLAD], f32, tag="negrow")
            nc.vector.memset(negrow[:], NEG)
            bigrow = mst.tile([1, NLAD], f32, tag="bigrow")
            nc.vector.memset(bigrow[:], BIG)
            tsel = mst.tile([1, NLAD], f32, tag="tsel")
            nc.vector.select(tsel[:], okm[:], Trow[:], negrow[:])
            tstar = mst.tile([1, 1], f32, tag="tstar")
            nc.vector.reduce_max(tstar[:], tsel[:], axis=AX.X)
            ok2 = mst.tile([1, NLAD], u8, tag="ok2")
            nc.vector.tensor_scalar(
                ok2[:], cntrow[:], 127.5, None, op0=OP.is_le
            )
            tsel2 = mst.tile([1, NLAD], f32, tag="tsel2")
            nc.vector.select(tsel2[:], ok2[:], Trow[:], bigrow[:])
            tfb = mst.tile([1, 1], f32, tag="tfb")
            nc.vector.tensor_reduce(tfb[:], tsel2[:], axis=AX.X, op=OP.min)
            have = mst.tile([1, 1], u8, tag="have")
            nc.vector.tensor_scalar(
                have[:], tstar[:], -1.0e30, None, op0=OP.is_ge
            )
            tfin = mst.tile([1, 1], f32, tag="tfin")
            nc.vector.select(tfin[:], have[:], tstar[:], tfb[:])
            ptf = ps.tile([P, 1], f32, tag="blk")
            nc.tensor.matmul(ptf[:P, :1], ones_r1[:], tfin[:], start=True, stop=True)
            tbc = mst.tile([P, 1], f32, tag="tbc")
            nc.vector.tensor_copy(tbc[:], ptf[:P, :1])

            # selmask / candidate index compaction
            selmask = mst.tile([P, NL], u8, tag="selmask")
            nc.vector.tensor_scalar(
                selmask[:], M_all[:], tbc[:], 0.0,
                op0=OP.subtract, op1=OP.is_ge,
            )
            midx = mst.tile([P, NL], f32, tag="midx")
            nc.vector.memset(midx[:], -1.0)
            nc.vector.copy_predicated(midx[:], selmask[:], qidx_f[:])
            pwr = ps.tile([16, P], f32, tag="blk", name="pwr")
            nc.tensor.transpose(pwr[:16, :P], midx[:], ident[:])
            wrap_in = mst.tile([16, P], f32, tag="wrap_in")
            nc.vector.tensor_copy(wrap_in[:], pwr[:16, :P])
            spg = mst.tile([16, 8], f32, tag="spg")
            nfound = mst.tile([1, 1], u32, tag="nfound")
            nc.gpsimd.sparse_gather(out=spg[:], in_=wrap_in[:], num_found=nfound[:])
            spg_cl = mst.tile([16, 8], f32, tag="spg_cl")
            nc.vector.tensor_scalar_max(spg_cl[:], spg[:], 0.0)
            nc.vector.tensor_scalar_min(spg_cl[:], spg_cl[:], float(L - 1))
            # fp16 keeps indices <= 2047 exact and avoids the fp32 double
            # LDWEIGHTS cost of the one-hot unwrap
            spg_h = mst.tile([16, 8], f16, tag="spg_h")
            nc.vector.tensor_copy(spg_h[:], spg_cl[:])
            pcq = ps.tile([P, 1], f32, tag="blk", name="pcq")
            for f in range(8):
                nc.tensor.matmul(
                    pcq[:P, :1], perm16[:, f * P : (f + 1) * P],
                    spg_h[:, f : f + 1],
                    start=(f == 0), stop=(f == 7),
                )
            candq_f = mst.tile([P, 1], f32, tag="candq_f")
            nc.vector.tensor_copy(candq_f[:], pcq[:P, :1])
            candq_i = mst.tile([P, 1], i32, tag="candq_i")
            nc.vector.tensor_copy(candq_i[:], pcq[:P, :1])
            nf_f = mst.tile([1, 1], f32, tag="nf_f")
            nc.vector.tensor_copy(nf_f[:], nfound[:])
            pnb = ps.tile([P, 1], f32, tag="blk")
            nc.tensor.matmul(pnb[:P, :1], ones_r1[:], nf_f[:], start=True, stop=True)
            nbc = mst.tile([P, 1], f32, tag="nbc")
            nc.vector.tensor_copy(nbc[:], pnb[:P, :1])
            invalid = mst.tile([P, 1], u8, tag="invalid")
            nc.vector.tensor_tensor(
                out=invalid[:], in0=qidx_f[:, 0:1], in1=nbc[:], op=OP.is_ge
            )

            # Keep-warm: ~3.5us of throwaway matmuls gated on candq_h so
            # they run exactly during the gather window; a >3.4us PE idle
            # here would drop the HAM clock to 1.2GHz for the whole exact
            # stage.
            candq_h = mst.tile([P, 1], bf16, tag="candq_h")
            nc.vector.tensor_copy(candq_h[:], pcq[:P, :1])
            pwarm = ps.tile([1, 512], f32, tag="blk", name="pwarm")
            for w in range(14):
                nc.tensor.matmul(
                    pwarm[:1, :512], candq_h[:, :1], xThp[:, 0:512],
                    start=True, stop=True,
                )

            # ---------------- phase 4: exact stage ----------------------
            x_cand = cnd.tile([P, D], f32, tag="x_cand")
            nc.gpsimd.indirect_dma_start(
                out=x_cand[:], out_offset=None, in_=x_d[:],
                in_offset=bass.IndirectOffsetOnAxis(ap=candq_i[:, :1], axis=0),
            )
            gm = cnd.tile([P, L], bf16, tag="gm")
            nc.gpsimd.indirect_dma_start(
                out=gm[:], out_offset=None, in_=maskb_d[:],
                in_offset=bass.IndirectOffsetOnAxis(ap=candq_i[:, :1], axis=0),
            )
            gc = cnd.tile([P, L], u8, tag="gc")
            nc.gpsimd.indirect_dma_start(
                out=gc[:], out_offset=None, in_=cnt_d[:],
                in_offset=bass.IndirectOffsetOnAxis(ap=candq_i[:, :1], axis=0),
            )

            # x_cand^T (fp32 — exact G via fp32 matmul, no hi/lo casts)
            xcT = [cnd.tile([P, P], f32, tag=f"xcT{dc}", name=f"xcT{dc}") for dc in range(ND)]
            for dc in range(ND):
                pxc = ps.tile([P, P], f32, tag="blk")
                nc.tensor.transpose(
                    pxc[:P, :P], x_cand[:, dc * P : (dc + 1) * P], ident[:]
                )
                nc.vector.tensor_copy(xcT[dc][:], pxc[:P, :P])

            # G = x_cand @ A (full fp32)
            pg = ps.tile([P, D], f32, tag="blk")
            for dc in range(ND):
                nc.tensor.matmul(
                    pg[:], xcT[dc][:], Af(dc, SD),
                    start=(dc == 0), stop=(dc == ND - 1),
                )
            gsb = cnd.tile([P, D], f32, tag="gsb")
            nc.vector.tensor_copy(gsb[:], pg[:])
            GTh = [cnd.tile([P, P], bf16, tag=f"GTh{dc}", name=f"GTh{dc}") for dc in range(ND)]
            GTl = [cnd.tile([P, P], bf16, tag=f"GTl{dc}", name=f"GTl{dc}") for dc in range(ND)]
            for dc in range(ND):
                pgt = ps.tile([P, P], f32, tag="blk")
                nc.tensor.transpose(
                    pgt[:P, :P], gsb[:, dc * P : (dc + 1) * P], ident[:]
                )
                nc.vector.tensor_copy(GTh[dc][:], pgt[:P, :P])
                nc.vector.tensor_tensor(
                    out=GTl[dc][:], in0=pgt[:P, :P], in1=GTh[dc][:],
                    op=OP.subtract,
                )

            # S_cand = G @ x^T (3-term bf16), 2 held [P,1024] PSUM pairs
            psS = []
            cmax = cnd.tile([P, 2], f32, tag="cmax")
            csum = cnd.tile([P, 2], f32, tag="csum")
            for jp in range(2):
                pss2 = ps_s.tile([P, 1024], f32, tag="psSc")
                psS.append(pss2)
                for jh in range(2):
                    jb = jp * 2 + jh
                    jsl = slice(jb * 512, (jb + 1) * 512)
                    n = 0
                    for dc in range(ND):
                        for lh, rh in (
                            (GTh[dc][:], xTh(dc, jsl)),
                            (GTl[dc][:], xTh(dc, jsl)),
                            (GTh[dc][:], xTl(dc, jsl)),
                        ):
                            nc.tensor.matmul(
                                pss2[:, jh * 512 : (jh + 1) * 512], lh, rh,
                                start=(n == 0), stop=(n == 3 * ND - 1),
                            )
                            n += 1
                psl = slice(jp * 1024, (jp + 1) * 1024)
                s3 = scr.tile([P, 1024], f32, tag="scrt2")
                nc.vector.tensor_tensor(
                    out=s3[:], in0=pss2[:], in1=gm[:, psl], op=OP.mult
                )
                nc.vector.reduce_max(cmax[:, jp : jp + 1], s3[:], axis=AX.X)
                s4 = scr.tile([P, 1024], f32, tag="scrt2")
                nc.vector.scalar_tensor_tensor(
                    out=s4[:], in0=pss2[:], scalar=-1.0 / L, in1=gc[:, psl],
                    op0=OP.mult, op1=OP.mult,
                    accum_out=csum[:, jp : jp + 1],
                )
            u1 = cnd.tile([P, 1], f32, tag="u1")
            u2 = cnd.tile([P, 1], f32, tag="u2")
            M_cand = cnd.tile([P, 1], f32, tag="M_cand")
            nc.vector.reduce_max(u1[:], cmax[:], axis=AX.X)
            nc.vector.reduce_sum(u2[:], csum[:], axis=AX.X)
            nc.vector.tensor_tensor(out=M_cand[:], in0=u1[:], in1=u2[:], op=OP.add)
            nc.vector.copy_predicated(M_cand[:], invalid[:], negbig[:])

            # exact top-40 threshold among candidates
            pmc = ps.tile([1, P], f32, tag="blk")
            nc.tensor.transpose(pmc[:1, :P], M_cand[:], ident[:])
            mcT = cnd.tile([1, P], f32, tag="mcT")
            nc.vector.tensor_copy(mcT[:], pmc[:1, :P])
            etop = cnd.tile([1, NT], f32, tag="etop")
            for r in range(5):
                nc.vector.max(out=etop[:, 8 * r : 8 * r + 8], in_=mcT[:])
                if r < 4:
                    nc.vector.match_replace(
                        out=mcT[:], in_to_replace=etop[:, 8 * r : 8 * r + 8],
                        in_values=mcT[:], imm_value=NEG,
                    )
            pte = ps.tile([P, 1], f32, tag="blk")
            nc.tensor.matmul(
                pte[:P, :1], ones_r1[:], etop[:, NT - 1 : NT], start=True, stop=True
            )
            tebc = cnd.tile([P, 1], f32, tag="tebc")
            nc.vector.tensor_copy(tebc[:], pte[:P, :1])
            sel2 = cnd.tile([P, 1], u8, tag="sel2")
            nc.vector.tensor_tensor(
                out=sel2[:], in0=M_cand[:], in1=tebc[:], op=OP.is_ge
            )
            scat_f = cnd.tile([P, 1], f32, tag="scat_f")
            nc.vector.tensor_copy(scat_f[:], big9[:])
            nc.vector.copy_predicated(scat_f[:], sel2[:], candq_f[:])
            scat_i = cnd.tile([P, 1], i32, tag="scat_i")
            nc.vector.tensor_copy(scat_i[:], scat_f[:])

            # ---------------- phase 5: softmax + update -----------------
            exp_sb = expp.tile([P, L], bf16, tag="exp_sb")
            sume4 = expp.tile([P, 2], f32, tag="sume4")
            for jp in range(2):
                psl = slice(jp * 1024, (jp + 1) * 1024)
                nc.scalar.activation(
                    out=exp_sb[:, psl], in_=psS[jp][:], func=ACTF.Exp,
                    bias=0.0, scale=SCALE,
                    accum_out=sume4[:, jp : jp + 1],
                )
            sume = expp.tile([P, 1], f32, tag="sume")
            nc.vector.reduce_sum(sume[:], sume4[:], axis=AX.X)
            recip = expp.tile([P, 1], f32, tag="recip")
            nc.vector.reciprocal(recip[:], sume[:])

            # expT transposes software-pipelined with the G2 accumulation
            # (depth 4) so the PE never idles long enough to re-throttle
            expT = [expp.tile([P, P], bf16, tag=f"expT{jc}", name=f"expT{jc}") for jc in range(NL)]
            pu = ps.tile([P, D], f32, tag="blk")

            def g2_mm(jc):
                nc.tensor.matmul(
                    pu[:], expT[jc][:], xNh(jc),
                    start=(jc == 0), stop=(jc == NL - 1),
                    skip_group_check=True,
                )

            for jc in range(NL):
                pet = psb.tile([P, P], bf16, tag="blkb")
                nc.tensor.transpose(
                    pet[:P, :P], exp_sb[:, jc * P : (jc + 1) * P], ident_b[:]
                )
                if jc % 2 == 0:
                    nc.vector.tensor_copy(expT[jc][:], pet[:P, :P])
                else:
                    nc.scalar.copy(expT[jc][:], pet[:P, :P])
                if jc >= 3:
                    g2_mm(jc - 3)
            for jc in range(NL - 3, NL):
                g2_mm(jc)
            g2b = expp.tile([P, D], bf16, tag="g2b")
            nc.scalar.copy(g2b[:], pu[:])
            G2T = [expp.tile([P, P], bf16, tag=f"G2T{dc}", name=f"G2T{dc}") for dc in range(ND)]
            for dc in range(ND):
                pg2 = psb.tile([P, P], bf16, tag="blkb")
                nc.tensor.transpose(
                    pg2[:P, :P], g2b[:, dc * P : (dc + 1) * P], ident_b[:]
                )
                nc.vector.tensor_copy(G2T[dc][:], pg2[:P, :P])
            # upd = G2 @ Wv^T / sums
            pup = ps.tile([P, D], f32, tag="blk")
            for dc in range(ND):
                nc.tensor.matmul(
                    pup[:], G2T[dc][:], wvh(dc, SD),
                    start=(dc == 0), stop=(dc == ND - 1),
                )
            upd = expp.tile([P, D], f32, tag="upd")
            nc.scalar.activation(
                out=upd[:], in_=pup[:], func=ACTF.Copy, bias=0.0, scale=recip[:]
            )
            nc.gpsimd.indirect_dma_start(
                out=ctx_d[:],
                out_offset=bass.IndirectOffsetOnAxis(ap=scat_i[:, :1], axis=0),
                in_=upd[:], in_offset=None,
                bounds_check=L - 1, oob_is_err=False,
            )

    nc.compile()
    return nc


_NC = None


def _get_nc():
    global _NC
    if _NC is None:
        _NC = build()
    return _NC


def _split_bf16(a):
    hi = a.astype(ml_dtypes.bfloat16)
    lo = (a - hi.astype(np.float32)).astype(ml_dtypes.bfloat16)
    return hi, lo


def _host_prep(x, Wq, Wk, Wv, index_sample):
    x = np.asarray(x, dtype=np.float32)
    Wq = np.asarray(Wq, dtype=np.float32)
    Wk = np.asarray(Wk, dtype=np.float32)
    Wv = np.asarray(Wv, dtype=np.float32)
    idx = np.asarray(index_sample)

    def pack(m):
        # [ND*P, W] -> [P, ND*W]: row dc*128+p lands at columns dc*W..+W
        nd = m.shape[0] // P
        return np.ascontiguousarray(
            m.reshape(nd, P, m.shape[1]).transpose(1, 0, 2).reshape(P, -1)
        )

    def pack_jb(m):
        # [ND*P, NJ*512] -> [P, NJ*ND*512] (jb-major blocks)
        nd = m.shape[0] // P
        nj = m.shape[1] // 512
        return np.ascontiguousarray(
            m.reshape(nd, P, nj, 512).transpose(1, 2, 0, 3).reshape(P, -1)
        )

    A = (Wq.T.astype(np.float64) @ Wk.astype(np.float64)).astype(np.float32)
    Ah = A.astype(ml_dtypes.bfloat16)
    wvh, wvl = _split_bf16(np.ascontiguousarray(Wv.T))

    rows = np.arange(L)[:, None]
    maskb = np.zeros((L, L), dtype=ml_dtypes.bfloat16)
    maskb[rows, idx] = 1
    countf = np.zeros((L, L), dtype=np.uint8)
    np.add.at(countf, (rows, idx), 1)

    perm16 = np.zeros((16, 8 * P), dtype=np.float16)
    for f in range(8):
        for p in range(16):
            perm16[p, f * P + p + 16 * f] = 1.0
    qidxf = (np.arange(P)[:, None] + 128 * np.arange(NL)[None, :]).astype(np.float32)
    crow = (1.2 + np.arange(NLAD, dtype=np.float32) * 0.134).reshape(1, NLAD)

    shared = {
        "Ahp": pack(Ah), "Afp": pack(A),
        "wvThp": pack(wvh), "wvTlp": pack(wvl),
        "maskb": maskb, "countf": countf, "perm16": perm16,
        "qidxf": qidxf, "crow": crow,
    }
    in_maps = []
    for b in range(B):
        xb = np.ascontiguousarray(x[b])
        xT = np.ascontiguousarray(xb.T)
        xth, xtl = _split_bf16(xT)
        xnh = xb.astype(ml_dtypes.bfloat16)
        xmean = xb.astype(np.float64).mean(axis=0).astype(np.float32)
        xmeh, xmel = _split_bf16(xmean.reshape(1, D))
        xm = np.concatenate(
            [xmeh.reshape(ND, P).T, xmel.reshape(ND, P).T], axis=1
        ).astype(ml_dtypes.bfloat16)
        in_maps.append(
            {
                "x_nat": xb,
                "xThp": pack_jb(xth),
                "xTlp": pack_jb(xtl),
                "xNhp": pack(xnh),
                "xmp": np.ascontiguousarray(xm),
                **shared,
            }
        )
    return in_maps


def kernel(x, Wq, Wk, Wv, index_sample, _trace=False, _result_box=None):
    in_maps = _host_prep(x, Wq, Wk, Wv, index_sample)
    nc = _get_nc()
    res = run_bass_kernel_spmd(nc, in_maps, core_ids=list(range(B)), trace=_trace)
    if _result_box is not None:
        _result_box.append(res)
    out = np.stack([np.asarray(res.results[b]["ctx"]) for b in range(B)], axis=0)
    return out


# revision 50
# speedup vs baseline: 1.9526x; 1.0087x over previous
"""Sparse attention (ProbSparse-style) Trainium2 Bass kernel, v2.

Problem (per batch element b, data-parallel over 8 NeuronCores):
  Q = x @ Wq.T ; K = x @ Wk.T ; V = x @ Wv.T            [L=2048, D=512]
  QK_sample[l,s] = Q[l] . K[index_sample[l,s]]           [L, 40]
  M[l] = max_s QK_sample - sum_s QK_sample / L
  sel = top40(M)  (as a set; the reference scatter makes order irrelevant)
  scores = Q[sel] @ K.T / sqrt(D); attn = softmax(scores)
  ctx = broadcast(mean(V)); ctx[sel] = attn @ V

Key ideas vs v1 baseline:
  - A = Wq^T @ Wk precomputed on host: S = (x A) x^T. Kills the K and Q
    projections entirely; both approx and exact scores contract against
    the resident x^T tiles.
  - Approx M = masked max of bf16 S only (the sum/L term is <= ~0.5 and
    is absorbed by the candidate margin; validated: true top-40 rows sit
    within rank <= 40 of the approx ordering).
  - Threshold via a 64-step mu + c*sigma ladder with on-device counts
    (one 3d-broadcast compare + reduce + PE column-sum), picking the
    largest T with count >= 88 (fallback: smallest T with count <= 127).
    Replaces the 62us GPSIMD kth_largest.
  - Exact stage on <= 128 candidates: G = x_cand A (3-term bf16),
    S_cand = G x^T (3-term bf16)  ->  ~1e-4-class absolute error,
    validated 26x under the seed-0 top-40 boundary gap.
  - Softmax without max subtraction (|S*scale| <= ~9, exp is safe),
    upd = (attn @ x) @ Wv^T (kills the V projection; V never built).

kernel(**inputs) accepts FULL inputs, returns FULL [8, 2048, 512] f32;
batch is sharded over 8 cores.
"""

import math

import numpy as np
import ml_dtypes

import concourse.bacc as bacc
import concourse.bass as bass
import concourse.mybir as mybir
import concourse.tile as tile
from concourse.bass_utils import run_bass_kernel_spmd
from concourse.masks import make_identity
from concourse import library_config

P = 128
L = 2048
D = 512
B = 8
NL = L // P        # 16 query chunks
ND = D // P        # 4 feature chunks
NJ = L // 512      # 4 key blocks of 512
NT = 40
NLAD = 32          # threshold ladder steps
SCALE = 1.0 / math.sqrt(D)
NEG = -3.0e38
BIG = 3.0e38
SKIP_IDX = 99999.0  # scatter index sentinel (> bounds_check -> row skipped)

f32 = mybir.dt.float32
f16 = mybir.dt.float16
bf16 = mybir.dt.bfloat16
u8 = mybir.dt.uint8
i32 = mybir.dt.int32
u32 = mybir.dt.uint32
AX = mybir.AxisListType
OP = mybir.AluOpType
ACTF = mybir.ActivationFunctionType


def build():
    nc = bacc.Bacc("TRN2", target_bir_lowering=False)

    # All big operands are host-packed into [128, wide] layouts so each
    # DMA partition line is a 4-16KB contiguous DRAM run (1KB lines were
    # descriptor-bound: ~26us of startup).
    x_d = nc.dram_tensor("x_nat", [L, D], f32, kind="ExternalInput")
    xth_d = nc.dram_tensor("xThp", [P, ND * L], bf16, kind="ExternalInput")
    xtl_d = nc.dram_tensor("xTlp", [P, ND * L], bf16, kind="ExternalInput")
    xnh_d = nc.dram_tensor("xNhp", [P, NL * D], bf16, kind="ExternalInput")
    ah_d = nc.dram_tensor("Ahp", [P, ND * D], bf16, kind="ExternalInput")
    af_d = nc.dram_tensor("Afp", [P, ND * D], f32, kind="ExternalInput")
    wvh_d = nc.dram_tensor("wvThp", [P, ND * D], bf16, kind="ExternalInput")
    wvl_d = nc.dram_tensor("wvTlp", [P, ND * D], bf16, kind="ExternalInput")
    xm_d = nc.dram_tensor("xmp", [P, 2 * ND], bf16, kind="ExternalInput")
    maskb_d = nc.dram_tensor("maskb", [L, L], bf16, kind="ExternalInput")
    cnt_d = nc.dram_tensor("countf", [L, L], u8, kind="ExternalInput")
    perm_d = nc.dram_tensor("perm16", [16, 8 * P], f16, kind="ExternalInput")
    qidx_d = nc.dram_tensor("qidxf", [P, NL], f32, kind="ExternalInput")
    crow_d = nc.dram_tensor("crow", [1, NLAD], f32, kind="ExternalInput")
    ctx_d = nc.dram_tensor("ctx", [L, D], f32, kind="ExternalOutput")

    with tile.TileContext(nc) as tc:
        with (
            tc.tile_pool(name="const", bufs=1) as cst,
            tc.tile_pool(name="xres", bufs=1) as xres,      # resident x / A / Wv
            tc.tile_pool(name="proj", bufs=1) as proj,      # QATb
            tc.tile_pool(name="mstuff", bufs=1) as mst,     # M / threshold smalls
            tc.tile_pool(name="mstream", bufs=3) as mstr,   # mask chunks
            tc.tile_pool(name="scr", bufs=3) as scr,        # TTR scratch
            tc.tile_pool(name="cand", bufs=1) as cnd,       # exact-stage tiles
            tc.tile_pool(name="expp", bufs=1) as expp,      # softmax/upd tiles
            tc.tile_pool(name="ps", bufs=2, space="PSUM") as ps,
            tc.tile_pool(name="psb", bufs=2, space="PSUM") as psb,    # bf16 transposes
            tc.tile_pool(name="ps_s", bufs=2, space="PSUM") as ps_s,  # S pairs
        ):
            # ---------------- constants ----------------
            ident = cst.tile([P, P], f32, tag="ident")
            make_identity(nc, ident[:])
            ident_b = cst.tile([P, P], bf16, tag="ident_b")
            nc.vector.tensor_copy(ident_b[:], ident[:])
            # preload the sparse_gather ucode so the serial tail does not
            # pay the library switch
            nc.gpsimd.load_library(library_config.sparse_gather)
            ones_r1 = cst.tile([1, P], f32, tag="ones_r1")
            nc.vector.memset(ones_r1[:], 1.0)
            ones_cf = cst.tile([P, 1], f32, tag="ones_cf")
            nc.vector.memset(ones_cf[:], 1.0)
            negbig = cst.tile([P, 1], f32, tag="negbig")
            nc.vector.memset(negbig[:], NEG)
            big9 = cst.tile([P, 1], f32, tag="big9")
            nc.vector.memset(big9[:], SKIP_IDX)
            qidx_f = cst.tile([P, NL], f32, tag="qidx_f")
            nc.sync.dma_start(qidx_f[:], qidx_d[:])
            crow = cst.tile([1, NLAD], f32, tag="crow")
            nc.sync.dma_start(crow[:], crow_d[:])
            perm16 = cst.tile([16, 8 * P], f16, tag="perm16")
            nc.sync.dma_start(perm16[:], perm_d[:])

            # ---------------- critical loads (packed, big lines) --------
            # Only Ahp + xThp gate the first matmuls; everything the tail
            # needs is DMA'd from inside the phase-2 loop so it doesn't
            # compete for startup bandwidth.
            Ahp = xres.tile([P, ND * D], bf16, tag="Ahp")
            nc.sync.dma_start(Ahp[:], ah_d[:])
            # x^T hi is packed jb-major: block jb holds [dc=0..3][512 cols]
            # so the first QA matmuls start after Ahp + one 0.5MB block
            xThp = xres.tile([P, ND * L], bf16, tag="xThp")
            for jb in range(NJ):
                nc.sync.dma_start(
                    xThp[:, jb * 2048 : (jb + 1) * 2048],
                    xth_d[:, jb * 2048 : (jb + 1) * 2048],
                )
            Afp = xres.tile([P, ND * D], f32, tag="Afp")
            wvhp = xres.tile([P, ND * D], bf16, tag="wvhp")
            wvlp = xres.tile([P, ND * D], bf16, tag="wvlp")
            xmp = xres.tile([P, 2 * ND], bf16, tag="xmp")
            xTlp = xres.tile([P, ND * L], bf16, tag="xTlp")
            xNhp = xres.tile([P, NL * D], bf16, tag="xNhp")

            # slice helpers over the packed tiles
            Ah = lambda dc, js: Ahp[:, dc * D + js.start : dc * D + js.stop]
            Af = lambda dc, js: Afp[:, dc * D + js.start : dc * D + js.stop]
            wvh = lambda dc, js: wvhp[:, dc * D + js.start : dc * D + js.stop]
            wvl = lambda dc, js: wvlp[:, dc * D + js.start : dc * D + js.stop]

            def _xt(tile_, dc, js):
                # jb-major packing: block jb*2048 + dc*512
                jb, r = divmod(js.start, 512)
                assert js.stop - js.start == 512 and r == 0
                off = jb * 2048 + dc * 512
                return tile_[:, off : off + 512]

            xTh = lambda dc, js: _xt(xThp, dc, js)
            xTl = lambda dc, js: _xt(xTlp, dc, js)
            xNh = lambda jc: xNhp[:, jc * D : (jc + 1) * D]
            SD = slice(0, D)
            SL = slice(0, L)

            # ---------------- phase 1: QA^T = A^T x^T (bf16) ------------
            QATb = [proj.tile([P, L], bf16, tag=f"QATb{ic}", name=f"QATb{ic}") for ic in range(ND)]
            for jb in range(NJ):
                jsl = slice(jb * 512, (jb + 1) * 512)
                for ic in range(ND):
                    isl = slice(ic * P, (ic + 1) * P)
                    pq = ps.tile([P, 512], f32, tag="blk")
                    for dc in range(ND):
                        nc.tensor.matmul(
                            pq[:], Ah(dc, isl), xTh(dc, jsl),
                            start=(dc == 0), stop=(dc == ND - 1),
                        )
                    nc.scalar.copy(QATb[ic][:, jsl], pq[:])

            # ---------------- phase 2: approx S + masked max ------------
            # Per 128-query chunk: PE computes 4 S blocks; ScalarE evicts
            # them to a bf16 row [P, 2048]; DVE does one 2x bf16 mask-mult
            # + one wide reduce_max.  (tensor_tensor_reduce crashes TRN2
            # hardware, so the fused form is not available.)
            M_all = mst.tile([P, NL], f32, tag="M_all")
            M_lo = mst.tile([P, 8], f32, tag="M_lo")
            Trow = mst.tile([1, NLAD], f32, tag="Trow")
            Tb = mst.tile([P, NLAD], bf16, tag="Tb")
            for lc in range(NL):
                lsl = slice(lc * P, (lc + 1) * P)
                mkb = mstr.tile([P, L], bf16, tag="mkb")
                nc.sync.dma_start(mkb[:], maskb_d[lsl, :])
                # tail-only loads trickled in behind the mask stream
                if lc == 0:
                    nc.sync.dma_start(wvhp[:], wvh_d[:])
                    nc.sync.dma_start(wvlp[:], wvl_d[:])
                    nc.sync.dma_start(xmp[:], xm_d[:])
                elif lc == 2:
                    nc.sync.dma_start(Afp[:], af_d[:])
                elif lc == 5:
                    nc.sync.dma_start(xTlp[:], xtl_d[:])
                elif lc == 9:
                    nc.sync.dma_start(xNhp[:], xnh_d[:])
                sb1 = scr.tile([P, L], bf16, tag="sb1")
                for jp in range(2):
                    # paired PSUM banks -> one wide eviction per 1024 cols
                    pss = ps_s.tile([P, 1024], f32, tag="psSc", name="pssa")
                    for jh in range(2):
                        jb = jp * 2 + jh
                        jsl = slice(jb * 512, (jb + 1) * 512)
                        for ic in range(ND):
                            nc.tensor.matmul(
                                pss[:, jh * 512 : (jh + 1) * 512],
                                QATb[ic][:, lsl], xTh(ic, jsl),
                                start=(ic == 0), stop=(ic == ND - 1),
                            )
                    nc.scalar.copy(
                        sb1[:, jp * 1024 : (jp + 1) * 1024], pss[:]
                    )
                # masked max: two 2x-mode masked products, one 2x max
                # combine, then a half-width 1x reduce
                t0 = scr.tile([P, 1024], bf16, tag="t0m")
                nc.vector.tensor_tensor(
                    out=t0[:], in0=sb1[:, 0:1024], in1=mkb[:, 0:1024],
                    op=OP.mult,
                )
                t1 = scr.tile([P, 1024], bf16, tag="t1m")
                nc.vector.tensor_tensor(
                    out=t1[:], in0=sb1[:, 1024:2048], in1=mkb[:, 1024:2048],
                    op=OP.mult,
                )
                t2 = scr.tile([P, 1024], bf16, tag="t2m")
                nc.vector.tensor_tensor(
                    out=t2[:], in0=t0[:], in1=t1[:], op=OP.max
                )
                if lc < 8:
                    nc.vector.reduce_max(M_lo[:, lc : lc + 1], t2[:], axis=AX.X)
                    nc.vector.tensor_copy(
                        M_all[:, lc : lc + 1], M_lo[:, lc : lc + 1]
                    )
                else:
                    nc.vector.reduce_max(M_all[:, lc : lc + 1], t2[:], axis=AX.X)
                if lc == 7:
                    # ---- early threshold stats on the first 1024 rows --
                    # (mu/sigma only steer the ladder range; the counts
                    # below verify against the full M) -- this whole chain
                    # runs under the second half of the main phase.
                    stats2 = mst.tile([P, 2], f32, tag="stats2")
                    msq = mst.tile([P, 8], f32, tag="msq")
                    nc.vector.scalar_tensor_tensor(
                        out=msq[:], in0=M_lo[:], scalar=1.0, in1=M_lo[:],
                        op0=OP.mult, op1=OP.mult,
                        accum_out=stats2[:, 1:2],
                    )
                    nc.vector.tensor_reduce(
                        stats2[:, 0:1], M_lo[:], axis=AX.X, op=OP.add
                    )
                    pst = ps.tile([1, 2], f32, tag="blk")
                    nc.tensor.matmul(
                        pst[:1, :2], ones_cf[:], stats2[:], start=True, stop=True
                    )
                    srow = mst.tile([1, 2], f32, tag="srow")
                    nc.vector.tensor_copy(srow[:], pst[:1, :2])
                    musig = mst.tile([1, 2], f32, tag="musig")
                    nc.vector.tensor_scalar_mul(musig[:], srow[:], 1.0 / 1024.0)
                    mu = musig[:, 0:1]
                    mu2 = mst.tile([1, 1], f32, tag="mu2")
                    nc.vector.tensor_tensor(out=mu2[:], in0=mu, in1=mu, op=OP.mult)
                    var = mst.tile([1, 1], f32, tag="var")
                    nc.vector.tensor_tensor(
                        out=var[:], in0=musig[:, 1:2], in1=mu2[:], op=OP.subtract
                    )
                    sigma = mst.tile([1, 1], f32, tag="sigma")
                    nc.scalar.sqrt(sigma[:], var[:])
                    nc.vector.tensor_tensor(
                        out=Trow[:], in0=crow[:],
                        in1=sigma[:].to_broadcast([1, NLAD]), op=OP.mult,
                    )
                    nc.vector.tensor_tensor(
                        out=Trow[:], in0=Trow[:], in1=mu.to_broadcast([1, NLAD]),
                        op=OP.add,
                    )
                    ptb = ps.tile([P, NLAD], f32, tag="blk")
                    nc.tensor.matmul(
                        ptb[:P, :NLAD], ones_r1[:], Trow[:], start=True, stop=True
                    )
                    nc.vector.tensor_copy(Tb[:], ptb[:P, :NLAD])

            # ---------------- phase 3: ladder counts --------------------
            M_b = mst.tile([P, NL], bf16, tag="M_b")
            nc.vector.tensor_copy(M_b[:], M_all[:])
            # cmp[p, j, f] = M[p, f] >= T[j]  (bf16 in/out -> 2x DVE, and
            # PE can column-sum the bf16 result)
            cmpb = mst.tile([P, NLAD * NL], bf16, tag="cmpb")
            nc.vector.tensor_tensor(
                out=cmpb[:].rearrange("p (j f) -> p j f", f=NL),
                in0=M_b[:].rearrange("p (o f) -> p o f", o=1).to_broadcast([P, NLAD, NL]),
                in1=Tb[:].rearrange("p (j o) -> p j o", o=1).to_broadcast([P, NLAD, NL]),
                op=OP.is_ge,
            )
            # ---------------- Vmean -> ctx init (PE idle slot) ----------
            pvm = ps.tile([1, D], f32, tag="blk")
            n = 0
            for dc in range(ND):
                for lh, rh in (
                    (xmp[:, dc : dc + 1], wvh(dc, SD)),
                    (xmp[:, ND + dc : ND + dc + 1], wvh(dc, SD)),
                    (xmp[:, dc : dc + 1], wvl(dc, SD)),
                ):
                    nc.tensor.matmul(
                        pvm[:1, :], lh, rh,
                        start=(n == 0), stop=(n == 3 * ND - 1),
                    )
                    n += 1
            vmean = mst.tile([1, D], f32, tag="vmean")
            nc.scalar.copy(vmean[:], pvm[:1, :])
            pvb = ps.tile([P, D], f32, tag="blk")
            nc.tensor.matmul(pvb[:], ones_r1[:], vmean[:], start=True, stop=True)
            vmean_bc = mst.tile([P, D], f32, tag="vmean_bc")
            nc.vector.tensor_copy(vmean_bc[:], pvb[:])
            for jc in range(NL):
                nc.sync.dma_start(ctx_d[jc * P : (jc + 1) * P, :], vmean_bc[:])

            cnt01 = mst.tile([P, NLAD], f32, tag="cnt01")
            nc.vector.tensor_reduce(
                cnt01[:], cmpb[:].rearrange("p (j f) -> p j f", f=NL),
                axis=AX.X, op=OP.add,
            )
            pcc = ps.tile([1, NLAD], f32, tag="blk")
            nc.tensor.matmul(pcc[:1, :NLAD], ones_cf[:], cnt01[:], start=True, stop=True)
            cntrow = mst.tile([1, NLAD], f32, tag="cntrow")
            nc.vector.tensor_copy(cntrow[:], pcc[:1, :NLAD])
            # largest T with count >= 88; fallback smallest T with count <= 127
            okm = mst.tile([1, NLAD], u8, tag="okm")
            nc.vector.tensor_scalar(
                okm[:], cntrow[:], 87.5, None, op0=OP.is_ge
            )
            negrow = mst.tile([1, NLAD], f32, tag="negrow")
            nc.vector.memset(negrow[:], NEG)
            bigrow = mst.tile([1, NLAD], f32, tag="bigrow")
            nc.vector.memset(bigrow[:], BIG)
            tsel = mst.tile([1, NLAD], f32, tag="tsel")
            nc.vector.select(tsel[:], okm[:], Trow[:], negrow[:])
            tstar = mst.tile([1, 1], f32, tag="tstar")
            nc.vector.reduce_max(tstar[:], tsel[:], axis=AX.X)
            ok2 = mst.tile([1, NLAD], u8, tag="ok2")
            nc.vector.tensor_scalar(
                ok2[:], cntrow[:], 127.5, None, op0=OP.is_le
            )
            tsel2 = mst.tile([1, NLAD], f32, tag="tsel2")
            nc.vector.select(tsel2[:], ok2[:], Trow[:], bigrow[:])
            tfb = mst.tile([1, 1], f32, tag="tfb")
            nc.vector.tensor_reduce(tfb[:], tsel2[:], axis=AX.X, op=OP.min)
            have = mst.tile([1, 1], u8, tag="have")
            nc.vector.tensor_scalar(
                have[:], tstar[:], -1.0e30, None, op0=OP.is_ge
            )
            tfin = mst.tile([1, 1], f32, tag="tfin")
            nc.vector.select(tfin[:], have[:], tstar[:], tfb[:])
            ptf = ps.tile([P, 1], f32, tag="blk")
            nc.tensor.matmul(ptf[:P, :1], ones_r1[:], tfin[:], start=True, stop=True)
            tbc = mst.tile([P, 1], f32, tag="tbc")
            nc.vector.tensor_copy(tbc[:], ptf[:P, :1])

            # selmask / candidate index compaction
            selmask = mst.tile([P, NL], u8, tag="selmask")
            nc.vector.tensor_scalar(
                selmask[:], M_all[:], tbc[:], 0.0,
                op0=OP.subtract, op1=OP.is_ge,
            )
            midx = mst.tile([P, NL], f32, tag="midx")
            nc.vector.memset(midx[:], -1.0)
            nc.vector.copy_predicated(midx[:], selmask[:], qidx_f[:])
            pwr = ps.tile([16, P], f32, tag="blk", name="pwr")
            nc.tensor.transpose(pwr[:16, :P], midx[:], ident[:])
            wrap_in = mst.tile([16, P], f32, tag="wrap_in")
            nc.vector.tensor_copy(wrap_in[:], pwr[:16, :P])
            spg = mst.tile([16, 8], f32, tag="spg")
            nfound = mst.tile([1, 1], u32, tag="nfound")
            nc.gpsimd.sparse_gather(out=spg[:], in_=wrap_in[:], num_found=nfound[:])
            spg_cl = mst.tile([16, 8], f32, tag="spg_cl")
            nc.vector.tensor_scalar_max(spg_cl[:], spg[:], 0.0)
            nc.vector.tensor_scalar_min(spg_cl[:], spg_cl[:], float(L - 1))
            # fp16 keeps indices <= 2047 exact and avoids the fp32 double
            # LDWEIGHTS cost of the one-hot unwrap
            spg_h = mst.tile([16, 8], f16, tag="spg_h")
            nc.vector.tensor_copy(spg_h[:], spg_cl[:])
            pcq = ps.tile([P, 1], f32, tag="blk", name="pcq")
            for f in range(8):
                nc.tensor.matmul(
                    pcq[:P, :1], perm16[:, f * P : (f + 1) * P],
                    spg_h[:, f : f + 1],
                    start=(f == 0), stop=(f == 7),
                )
            candq_f = mst.tile([P, 1], f32, tag="candq_f")
            nc.vector.tensor_copy(candq_f[:], pcq[:P, :1])
            candq_i = mst.tile([P, 1], i32, tag="candq_i")
            nc.vector.tensor_copy(candq_i[:], pcq[:P, :1])
            nf_f = mst.tile([1, 1], f32, tag="nf_f")
            nc.vector.tensor_copy(nf_f[:], nfound[:])
            pnb = ps.tile([P, 1], f32, tag="blk")
            nc.tensor.matmul(pnb[:P, :1], ones_r1[:], nf_f[:], start=True, stop=True)
            nbc = mst.tile([P, 1], f32, tag="nbc")
            nc.vector.tensor_copy(nbc[:], pnb[:P, :1])
            invalid = mst.tile([P, 1], u8, tag="invalid")
            nc.vector.tensor_tensor(
                out=invalid[:], in0=qidx_f[:, 0:1], in1=nbc[:], op=OP.is_ge
            )

            # Keep-warm: ~3.5us of throwaway matmuls gated on candq_h so
            # they run exactly during the gather window; a >3.4us PE idle
            # here would drop the HAM clock to 1.2GHz for the whole exact
            # stage.
            candq_h = mst.tile([P, 1], bf16, tag="candq_h")
            nc.vector.tensor_copy(candq_h[:], pcq[:P, :1])
            pwarm = ps.tile([1, 512], f32, tag="blk", name="pwarm")
            for w in range(14):
                nc.tensor.matmul(
                    pwarm[:1, :512], candq_h[:, :1], xThp[:, 0:512],
                    start=True, stop=True,
                )

            # ---------------- phase 4: exact stage ----------------------
            x_cand = cnd.tile([P, D], f32, tag="x_cand")
            nc.gpsimd.indirect_dma_start(
                out=x_cand[:], out_offset=None, in_=x_d[:],
                in_offset=bass.IndirectOffsetOnAxis(ap=candq_i[:, :1], axis=0),
            )
            gm = cnd.tile([P, L], bf16, tag="gm")
            nc.gpsimd.indirect_dma_start(
                out=gm[:], out_offset=None, in_=maskb_d[:],
                in_offset=bass.IndirectOffsetOnAxis(ap=candq_i[:, :1], axis=0),
            )
            gc = cnd.tile([P, L], u8, tag="gc")
            nc.gpsimd.indirect_dma_start(
                out=gc[:], out_offset=None, in_=cnt_d[:],
                in_offset=bass.IndirectOffsetOnAxis(ap=candq_i[:, :1], axis=0),
            )

            # x_cand^T (fp32 — exact G via fp32 matmul, no hi/lo casts)
            xcT = [cnd.tile([P, P], f32, tag=f"xcT{dc}", name=f"xcT{dc}") for dc in range(ND)]
            for dc in range(ND):
                pxc = ps.tile([P, P], f32, tag="blk")
                nc.tensor.transpose(
                    pxc[:P, :P], x_cand[:, dc * P : (dc + 1) * P], ident[:]
                )
                nc.vector.tensor_copy(xcT[dc][:], pxc[:P, :P])

            # G^T computed directly: GT[dout, cand] = sum_din A[din, dout]^T
            # x_cand^T[din, cand] — 16 fp32 N=128 matmuls, no gsb round-trip
            GTh = [cnd.tile([P, P], bf16, tag=f"GTh{dc}", name=f"GTh{dc}") for dc in range(ND)]
            GTl = [cnd.tile([P, P], bf16, tag=f"GTl{dc}", name=f"GTl{dc}") for dc in range(ND)]
            for do in range(ND):
                osl = slice(do * P, (do + 1) * P)
                pgt = ps.tile([P, P], f32, tag="blk")
                for di in range(ND):
                    nc.tensor.matmul(
                        pgt[:P, :P], Af(di, osl), xcT[di][:],
                        start=(di == 0), stop=(di == ND - 1),
                    )
                nc.vector.tensor_copy(GTh[do][:], pgt[:P, :P])
                nc.vector.tensor_tensor(
                    out=GTl[do][:], in0=pgt[:P, :P], in1=GTh[do][:],
                    op=OP.subtract,
                )

            # S_cand = G @ x^T (3-term bf16), 2 held [P,1024] PSUM pairs
            psS = []
            cmax = cnd.tile([P, 2], f32, tag="cmax")
            csum = cnd.tile([P, 2], f32, tag="csum")
            for jp in range(2):
                pss2 = ps_s.tile([P, 1024], f32, tag="psSc")
                psS.append(pss2)
                for jh in range(2):
                    jb = jp * 2 + jh
                    jsl = slice(jb * 512, (jb + 1) * 512)
                    n = 0
                    for dc in range(ND):
                        for lh, rh in (
                            (GTh[dc][:], xTh(dc, jsl)),
                            (GTl[dc][:], xTh(dc, jsl)),
                            (GTh[dc][:], xTl(dc, jsl)),
                        ):
                            nc.tensor.matmul(
                                pss2[:, jh * 512 : (jh + 1) * 512], lh, rh,
                                start=(n == 0), stop=(n == 3 * ND - 1),
                            )
                            n += 1
                psl = slice(jp * 1024, (jp + 1) * 1024)
                s3 = scr.tile([P, 1024], f32, tag="scrt2")
                nc.vector.tensor_tensor(
                    out=s3[:], in0=pss2[:], in1=gm[:, psl], op=OP.mult
                )
                nc.vector.reduce_max(cmax[:, jp : jp + 1], s3[:], axis=AX.X)
                s4 = scr.tile([P, 1024], f32, tag="scrt2")
                nc.vector.scalar_tensor_tensor(
                    out=s4[:], in0=pss2[:], scalar=-1.0 / L, in1=gc[:, psl],
                    op0=OP.mult, op1=OP.mult,
                    accum_out=csum[:, jp : jp + 1],
                )
            u1 = cnd.tile([P, 1], f32, tag="u1")
            u2 = cnd.tile([P, 1], f32, tag="u2")
            M_cand = cnd.tile([P, 1], f32, tag="M_cand")
            nc.vector.reduce_max(u1[:], cmax[:], axis=AX.X)
            nc.vector.reduce_sum(u2[:], csum[:], axis=AX.X)
            nc.vector.tensor_tensor(out=M_cand[:], in0=u1[:], in1=u2[:], op=OP.add)
            nc.vector.copy_predicated(M_cand[:], invalid[:], negbig[:])

            # exact top-40 threshold among candidates
            pmc = ps.tile([1, P], f32, tag="blk")
            nc.tensor.transpose(pmc[:1, :P], M_cand[:], ident[:])
            mcT = cnd.tile([1, P], f32, tag="mcT")
            nc.vector.tensor_copy(mcT[:], pmc[:1, :P])
            etop = cnd.tile([1, NT], f32, tag="etop")
            for r in range(5):
                nc.vector.max(out=etop[:, 8 * r : 8 * r + 8], in_=mcT[:])
                if r < 4:
                    nc.vector.match_replace(
                        out=mcT[:], in_to_replace=etop[:, 8 * r : 8 * r + 8],
                        in_values=mcT[:], imm_value=NEG,
                    )
            pte = ps.tile([P, 1], f32, tag="blk")
            nc.tensor.matmul(
                pte[:P, :1], ones_r1[:], etop[:, NT - 1 : NT], start=True, stop=True
            )
            tebc = cnd.tile([P, 1], f32, tag="tebc")
            nc.vector.tensor_copy(tebc[:], pte[:P, :1])
            sel2 = cnd.tile([P, 1], u8, tag="sel2")
            nc.vector.tensor_tensor(
                out=sel2[:], in0=M_cand[:], in1=tebc[:], op=OP.is_ge
            )
            scat_f = cnd.tile([P, 1], f32, tag="scat_f")
            nc.vector.tensor_copy(scat_f[:], big9[:])
            nc.vector.copy_predicated(scat_f[:], sel2[:], candq_f[:])
            scat_i = cnd.tile([P, 1], i32, tag="scat_i")
            nc.vector.tensor_copy(scat_i[:], scat_f[:])

            # ---------------- phase 5: softmax + update -----------------
            exp_sb = expp.tile([P, L], bf16, tag="exp_sb")
            sume4 = expp.tile([P, 2], f32, tag="sume4")
            for jp in range(2):
                psl = slice(jp * 1024, (jp + 1) * 1024)
                nc.scalar.activation(
                    out=exp_sb[:, psl], in_=psS[jp][:], func=ACTF.Exp,
                    bias=0.0, scale=SCALE,
                    accum_out=sume4[:, jp : jp + 1],
                )
            sume = expp.tile([P, 1], f32, tag="sume")
            nc.vector.reduce_sum(sume[:], sume4[:], axis=AX.X)
            recip = expp.tile([P, 1], f32, tag="recip")
            nc.vector.reciprocal(recip[:], sume[:])

            # expT transposes software-pipelined with the G2 accumulation
            # (depth 4) so the PE never idles long enough to re-throttle
            expT = [expp.tile([P, P], bf16, tag=f"expT{jc}", name=f"expT{jc}") for jc in range(NL)]
            pu = ps.tile([P, D], f32, tag="blk")

            def g2_mm(jc):
                nc.tensor.matmul(
                    pu[:], expT[jc][:], xNh(jc),
                    start=(jc == 0), stop=(jc == NL - 1),
                    skip_group_check=True,
                )

            for jc in range(NL):
                pet = psb.tile([P, P], bf16, tag="blkb")
                nc.tensor.transpose(
                    pet[:P, :P], exp_sb[:, jc * P : (jc + 1) * P], ident_b[:]
                )
                if jc % 2 == 0:
                    nc.vector.tensor_copy(expT[jc][:], pet[:P, :P])
                else:
                    nc.scalar.copy(expT[jc][:], pet[:P, :P])
                if jc >= 3:
                    g2_mm(jc - 3)
            for jc in range(NL - 3, NL):
                g2_mm(jc)
            g2b = expp.tile([P, D], bf16, tag="g2b")
            nc.scalar.copy(g2b[:], pu[:])
            G2T = [expp.tile([P, P], bf16, tag=f"G2T{dc}", name=f"G2T{dc}") for dc in range(ND)]
            for dc in range(ND):
                pg2 = psb.tile([P, P], bf16, tag="blkb")
                nc.tensor.transpose(
                    pg2[:P, :P], g2b[:, dc * P : (dc + 1) * P], ident_b[:]
                )
                nc.vector.tensor_copy(G2T[dc][:], pg2[:P, :P])
            # upd = G2 @ Wv^T / sums
            pup = ps.tile([P, D], f32, tag="blk")
            for dc in range(ND):
                nc.tensor.matmul(
                    pup[:], G2T[dc][:], wvh(dc, SD),
                    start=(dc == 0), stop=(dc == ND - 1),
                )
            upd = expp.tile([P, D], f32, tag="upd")
            nc.scalar.activation(
                out=upd[:], in_=pup[:], func=ACTF.Copy, bias=0.0, scale=recip[:]
            )
            nc.gpsimd.indirect_dma_start(
                out=ctx_d[:],
                out_offset=bass.IndirectOffsetOnAxis(ap=scat_i[:, :1], axis=0),
                in_=upd[:], in_offset=None,
                bounds_check=L - 1, oob_is_err=False,
            )

    nc.compile()
    return nc


_NC = None


def _get_nc():
    global _NC
    if _NC is None:
        _NC = build()
    return _NC


def _split_bf16(a):
    hi = a.astype(ml_dtypes.bfloat16)
    lo = (a - hi.astype(np.float32)).astype(ml_dtypes.bfloat16)
    return hi, lo


def _host_prep(x, Wq, Wk, Wv, index_sample):
    x = np.asarray(x, dtype=np.float32)
    Wq = np.asarray(Wq, dtype=np.float32)
    Wk = np.asarray(Wk, dtype=np.float32)
    Wv = np.asarray(Wv, dtype=np.float32)
    idx = np.asarray(index_sample)

    def pack(m):
        # [ND*P, W] -> [P, ND*W]: row dc*128+p lands at columns dc*W..+W
        nd = m.shape[0] // P
        return np.ascontiguousarray(
            m.reshape(nd, P, m.shape[1]).transpose(1, 0, 2).reshape(P, -1)
        )

    def pack_jb(m):
        # [ND*P, NJ*512] -> [P, NJ*ND*512] (jb-major blocks)
        nd = m.shape[0] // P
        nj = m.shape[1] // 512
        return np.ascontiguousarray(
            m.reshape(nd, P, nj, 512).transpose(1, 2, 0, 3).reshape(P, -1)
        )

    A = (Wq.T.astype(np.float64) @ Wk.astype(np.float64)).astype(np.float32)
    Ah = A.astype(ml_dtypes.bfloat16)
    wvh, wvl = _split_bf16(np.ascontiguousarray(Wv.T))

    rows = np.arange(L)[:, None]
    maskb = np.zeros((L, L), dtype=ml_dtypes.bfloat16)
    maskb[rows, idx] = 1
    countf = np.zeros((L, L), dtype=np.uint8)
    np.add.at(countf, (rows, idx), 1)

    perm16 = np.zeros((16, 8 * P), dtype=np.float16)
    for f in range(8):
        for p in range(16):
            perm16[p, f * P + p + 16 * f] = 1.0
    qidxf = (np.arange(P)[:, None] + 128 * np.arange(NL)[None, :]).astype(np.float32)
    crow = (1.2 + np.arange(NLAD, dtype=np.float32) * 0.134).reshape(1, NLAD)

    shared = {
        "Ahp": pack(Ah), "Afp": pack(A),
        "wvThp": pack(wvh), "wvTlp": pack(wvl),
        "maskb": maskb, "countf": countf, "perm16": perm16,
        "qidxf": qidxf, "crow": crow,
    }
    in_maps = []
    for b in range(B):
        xb = np.ascontiguousarray(x[b])
        xT = np.ascontiguousarray(xb.T)
        xth, xtl = _split_bf16(xT)
        xnh = xb.astype(ml_dtypes.bfloat16)
        xmean = xb.astype(np.float64).mean(axis=0).astype(np.float32)
        xmeh, xmel = _split_bf16(xmean.reshape(1, D))
        xm = np.concatenate(
            [xmeh.reshape(ND, P).T, xmel.reshape(ND, P).T], axis=1
        ).astype(ml_dtypes.bfloat16)
        in_maps.append(
            {
                "x_nat": xb,
                "xThp": pack_jb(xth),
                "xTlp": pack_jb(xtl),
                "xNhp": pack(xnh),
                "xmp": np.ascontiguousarray(xm),
                **shared,
            }
        )
    return in_maps


def kernel(x, Wq, Wk, Wv, index_sample, _trace=False, _result_box=None):
    in_maps = _host_prep(x, Wq, Wk, Wv, index_sample)
    nc = _get_nc()
    res = run_bass_kernel_spmd(nc, in_maps, core_ids=list(range(B)), trace=_trace)
    if _result_box is not None:
        _result_box.append(res)
    out = np.stack([np.asarray(res.results[b]["ctx"]) for b in range(B)], axis=0)
    return out


# revision 53
# speedup vs baseline: 2.0069x; 1.0278x over previous
"""Sparse attention (ProbSparse-style) Trainium2 Bass kernel, v2.

Problem (per batch element b, data-parallel over 8 NeuronCores):
  Q = x @ Wq.T ; K = x @ Wk.T ; V = x @ Wv.T            [L=2048, D=512]
  QK_sample[l,s] = Q[l] . K[index_sample[l,s]]           [L, 40]
  M[l] = max_s QK_sample - sum_s QK_sample / L
  sel = top40(M)  (as a set; the reference scatter makes order irrelevant)
  scores = Q[sel] @ K.T / sqrt(D); attn = softmax(scores)
  ctx = broadcast(mean(V)); ctx[sel] = attn @ V

Key ideas vs v1 baseline:
  - A = Wq^T @ Wk precomputed on host: S = (x A) x^T. Kills the K and Q
    projections entirely; both approx and exact scores contract against
    the resident x^T tiles.
  - Approx M = masked max of bf16 S only (the sum/L term is <= ~0.5 and
    is absorbed by the candidate margin; validated: true top-40 rows sit
    within rank <= 40 of the approx ordering).
  - Threshold via a 64-step mu + c*sigma ladder with on-device counts
    (one 3d-broadcast compare + reduce + PE column-sum), picking the
    largest T with count >= 88 (fallback: smallest T with count <= 127).
    Replaces the 62us GPSIMD kth_largest.
  - Exact stage on <= 128 candidates: G = x_cand A (3-term bf16),
    S_cand = G x^T (3-term bf16)  ->  ~1e-4-class absolute error,
    validated 26x under the seed-0 top-40 boundary gap.
  - Softmax without max subtraction (|S*scale| <= ~9, exp is safe),
    upd = (attn @ x) @ Wv^T (kills the V projection; V never built).

kernel(**inputs) accepts FULL inputs, returns FULL [8, 2048, 512] f32;
batch is sharded over 8 cores.
"""

import math

import numpy as np
import ml_dtypes

import concourse.bacc as bacc
import concourse.bass as bass
import concourse.mybir as mybir
import concourse.tile as tile
from concourse.bass_utils import run_bass_kernel_spmd
from concourse.masks import make_identity
from concourse import library_config

P = 128
L = 2048
D = 512
B = 8
NL = L // P        # 16 query chunks
ND = D // P        # 4 feature chunks
NJ = L // 512      # 4 key blocks of 512
NT = 40
NLAD = 32          # threshold ladder steps
SCALE = 1.0 / math.sqrt(D)
NEG = -3.0e38
BIG = 3.0e38
SKIP_IDX = 99999.0  # scatter index sentinel (> bounds_check -> row skipped)

f32 = mybir.dt.float32
f16 = mybir.dt.float16
bf16 = mybir.dt.bfloat16
u8 = mybir.dt.uint8
i32 = mybir.dt.int32
u32 = mybir.dt.uint32
AX = mybir.AxisListType
OP = mybir.AluOpType
ACTF = mybir.ActivationFunctionType


def build():
    nc = bacc.Bacc("TRN2", target_bir_lowering=False)

    # All big operands are host-packed into [128, wide] layouts so each
    # DMA partition line is a 4-16KB contiguous DRAM run (1KB lines were
    # descriptor-bound: ~26us of startup).
    x_d = nc.dram_tensor("x_nat", [L, D], f32, kind="ExternalInput")
    xth_d = nc.dram_tensor("xThp", [P, ND * L], bf16, kind="ExternalInput")
    xtl_d = nc.dram_tensor("xTlp", [P, ND * L], bf16, kind="ExternalInput")
    xnh_d = nc.dram_tensor("xNhp", [P, NL * D], bf16, kind="ExternalInput")
    ah_d = nc.dram_tensor("Ahp", [P, ND * D], bf16, kind="ExternalInput")
    af_d = nc.dram_tensor("Afp", [P, ND * D], f32, kind="ExternalInput")
    wvh_d = nc.dram_tensor("wvThp", [P, ND * D], bf16, kind="ExternalInput")
    wvl_d = nc.dram_tensor("wvTlp", [P, ND * D], bf16, kind="ExternalInput")
    xm_d = nc.dram_tensor("xmp", [P, 2 * ND], bf16, kind="ExternalInput")
    maskb_d = nc.dram_tensor("maskb", [L, L], bf16, kind="ExternalInput")
    cnt_d = nc.dram_tensor("countf", [L, L], u8, kind="ExternalInput")
    perm_d = nc.dram_tensor("perm16", [16, 8 * P], f16, kind="ExternalInput")
    qidx_d = nc.dram_tensor("qidxf", [P, NL], f32, kind="ExternalInput")
    crow_d = nc.dram_tensor("crow", [1, NLAD], f32, kind="ExternalInput")
    ctx_d = nc.dram_tensor("ctx", [L, D], f32, kind="ExternalOutput")

    with tile.TileContext(nc) as tc:
        with (
            tc.tile_pool(name="const", bufs=1) as cst,
            tc.tile_pool(name="xres", bufs=1) as xres,      # resident x / A / Wv
            tc.tile_pool(name="proj", bufs=1) as proj,      # QATb
            tc.tile_pool(name="mstuff", bufs=1) as mst,     # M / threshold smalls
            tc.tile_pool(name="mstream", bufs=3) as mstr,   # mask chunks
            tc.tile_pool(name="scr", bufs=3) as scr,        # TTR scratch
            tc.tile_pool(name="cand", bufs=1) as cnd,       # exact-stage tiles
            tc.tile_pool(name="expp", bufs=1) as expp,      # softmax/upd tiles
            tc.tile_pool(name="ps", bufs=2, space="PSUM") as ps,
            tc.tile_pool(name="psb", bufs=2, space="PSUM") as psb,    # bf16 transposes
            tc.tile_pool(name="ps_s", bufs=2, space="PSUM") as ps_s,  # S pairs
        ):
            # ---------------- constants ----------------
            ident = cst.tile([P, P], f32, tag="ident")
            make_identity(nc, ident[:])
            ident_b = cst.tile([P, P], bf16, tag="ident_b")
            nc.vector.tensor_copy(ident_b[:], ident[:])
            # preload the sparse_gather ucode so the serial tail does not
            # pay the library switch
            nc.gpsimd.load_library(library_config.sparse_gather)
            ones_r1 = cst.tile([1, P], f32, tag="ones_r1")
            nc.vector.memset(ones_r1[:], 1.0)
            ones_cf = cst.tile([P, 1], f32, tag="ones_cf")
            nc.vector.memset(ones_cf[:], 1.0)
            negbig = cst.tile([P, 1], f32, tag="negbig")
            nc.vector.memset(negbig[:], NEG)
            big9 = cst.tile([P, 1], f32, tag="big9")
            nc.vector.memset(big9[:], SKIP_IDX)
            qidx_f = cst.tile([P, NL], f32, tag="qidx_f")
            nc.sync.dma_start(qidx_f[:], qidx_d[:])
            crow = cst.tile([1, NLAD], f32, tag="crow")
            nc.sync.dma_start(crow[:], crow_d[:])
            perm16 = cst.tile([16, 8 * P], f16, tag="perm16")
            nc.sync.dma_start(perm16[:], perm_d[:])

            # ---------------- critical loads (packed, big lines) --------
            # Only Ahp + xThp gate the first matmuls; everything the tail
            # needs is DMA'd from inside the phase-2 loop so it doesn't
            # compete for startup bandwidth.
            Ahp = xres.tile([P, ND * D], bf16, tag="Ahp")
            nc.sync.dma_start(Ahp[:], ah_d[:])
            # x^T hi is packed jb-major: block jb holds [dc=0..3][512 cols]
            # so the first QA matmuls start after Ahp + one 0.5MB block
            xThp = xres.tile([P, ND * L], bf16, tag="xThp")
            for jb in range(NJ):
                nc.sync.dma_start(
                    xThp[:, jb * 2048 : (jb + 1) * 2048],
                    xth_d[:, jb * 2048 : (jb + 1) * 2048],
                )
            Afp = xres.tile([P, ND * D], f32, tag="Afp")
            wvhp = xres.tile([P, ND * D], bf16, tag="wvhp")
            wvlp = xres.tile([P, ND * D], bf16, tag="wvlp")
            xmp = xres.tile([P, 2 * ND], bf16, tag="xmp")
            xTlp = xres.tile([P, ND * L], bf16, tag="xTlp")
            xNhp = xres.tile([P, NL * D], bf16, tag="xNhp")

            # slice helpers over the packed tiles
            Ah = lambda dc, js: Ahp[:, dc * D + js.start : dc * D + js.stop]
            Af = lambda dc, js: Afp[:, dc * D + js.start : dc * D + js.stop]
            wvh = lambda dc, js: wvhp[:, dc * D + js.start : dc * D + js.stop]
            wvl = lambda dc, js: wvlp[:, dc * D + js.start : dc * D + js.stop]

            def _xt(tile_, dc, js):
                # jb-major packing: block jb*2048 + dc*512
                jb, r = divmod(js.start, 512)
                assert js.stop - js.start == 512 and r == 0
                off = jb * 2048 + dc * 512
                return tile_[:, off : off + 512]

            xTh = lambda dc, js: _xt(xThp, dc, js)
            xTl = lambda dc, js: _xt(xTlp, dc, js)
            xNh = lambda jc: xNhp[:, jc * D : (jc + 1) * D]
            SD = slice(0, D)
            SL = slice(0, L)

            # ---------------- phase 1: QA^T = A^T x^T (bf16) ------------
            QATb = [proj.tile([P, L], bf16, tag=f"QATb{ic}", name=f"QATb{ic}") for ic in range(ND)]
            for jb in range(NJ):
                jsl = slice(jb * 512, (jb + 1) * 512)
                for ic in range(ND):
                    isl = slice(ic * P, (ic + 1) * P)
                    pq = ps.tile([P, 512], f32, tag="blk")
                    for dc in range(ND):
                        nc.tensor.matmul(
                            pq[:], Ah(dc, isl), xTh(dc, jsl),
                            start=(dc == 0), stop=(dc == ND - 1),
                        )
                    nc.scalar.copy(QATb[ic][:, jsl], pq[:])

            # ---------------- phase 2: approx S + masked max ------------
            # Per 128-query chunk: PE computes 4 S blocks; ScalarE evicts
            # them to a bf16 row [P, 2048]; DVE does one 2x bf16 mask-mult
            # + one wide reduce_max.  (tensor_tensor_reduce crashes TRN2
            # hardware, so the fused form is not available.)
            M_all = mst.tile([P, NL], f32, tag="M_all")
            M_lo = mst.tile([P, 8], f32, tag="M_lo")
            Trow = mst.tile([1, NLAD], f32, tag="Trow")
            Tb = mst.tile([P, NLAD], bf16, tag="Tb")
            for lc in range(NL):
                lsl = slice(lc * P, (lc + 1) * P)
                mkb = mstr.tile([P, L], bf16, tag="mkb")
                nc.sync.dma_start(mkb[:], maskb_d[lsl, :])
                # tail-only loads trickled in behind the mask stream
                if lc == 0:
                    nc.sync.dma_start(wvhp[:], wvh_d[:])
                    nc.sync.dma_start(wvlp[:], wvl_d[:])
                    nc.sync.dma_start(xmp[:], xm_d[:])
                elif lc == 2:
                    nc.sync.dma_start(Afp[:], af_d[:])
                elif lc == 5:
                    nc.sync.dma_start(xTlp[:], xtl_d[:])
                elif lc == 9:
                    nc.sync.dma_start(xNhp[:], xnh_d[:])
                sb1 = scr.tile([P, L], bf16, tag="sb1")
                for jp in range(2):
                    # paired PSUM banks -> one wide eviction per 1024 cols
                    pss = ps_s.tile([P, 1024], f32, tag="psSc", name="pssa")
                    for jh in range(2):
                        jb = jp * 2 + jh
                        jsl = slice(jb * 512, (jb + 1) * 512)
                        for ic in range(ND):
                            nc.tensor.matmul(
                                pss[:, jh * 512 : (jh + 1) * 512],
                                QATb[ic][:, lsl], xTh(ic, jsl),
                                start=(ic == 0), stop=(ic == ND - 1),
                            )
                    nc.scalar.copy(
                        sb1[:, jp * 1024 : (jp + 1) * 1024], pss[:]
                    )
                # masked max: two 2x-mode masked products, one 2x max
                # combine, then a half-width 1x reduce
                t0 = scr.tile([P, 1024], bf16, tag="t0m")
                nc.vector.tensor_tensor(
                    out=t0[:], in0=sb1[:, 0:1024], in1=mkb[:, 0:1024],
                    op=OP.mult,
                )
                t1 = scr.tile([P, 1024], bf16, tag="t1m")
                nc.vector.tensor_tensor(
                    out=t1[:], in0=sb1[:, 1024:2048], in1=mkb[:, 1024:2048],
                    op=OP.mult,
                )
                t2 = scr.tile([P, 1024], bf16, tag="t2m")
                nc.vector.tensor_tensor(
                    out=t2[:], in0=t0[:], in1=t1[:], op=OP.max
                )
                if lc < 8:
                    nc.vector.reduce_max(M_lo[:, lc : lc + 1], t2[:], axis=AX.X)
                    nc.vector.tensor_copy(
                        M_all[:, lc : lc + 1], M_lo[:, lc : lc + 1]
                    )
                else:
                    nc.vector.reduce_max(M_all[:, lc : lc + 1], t2[:], axis=AX.X)
                if lc == 7:
                    # ---- early threshold stats on the first 1024 rows --
                    # (mu/sigma only steer the ladder range; the counts
                    # below verify against the full M) -- this whole chain
                    # runs under the second half of the main phase.
                    stats2 = mst.tile([P, 2], f32, tag="stats2")
                    msq = mst.tile([P, 8], f32, tag="msq")
                    nc.vector.scalar_tensor_tensor(
                        out=msq[:], in0=M_lo[:], scalar=1.0, in1=M_lo[:],
                        op0=OP.mult, op1=OP.mult,
                        accum_out=stats2[:, 1:2],
                    )
                    nc.vector.tensor_reduce(
                        stats2[:, 0:1], M_lo[:], axis=AX.X, op=OP.add
                    )
                    pst = ps.tile([1, 2], f32, tag="blk")
                    nc.tensor.matmul(
                        pst[:1, :2], ones_cf[:], stats2[:], start=True, stop=True
                    )
                    srow = mst.tile([1, 2], f32, tag="srow")
                    nc.vector.tensor_copy(srow[:], pst[:1, :2])
                    musig = mst.tile([1, 2], f32, tag="musig")
                    nc.vector.tensor_scalar_mul(musig[:], srow[:], 1.0 / 1024.0)
                    mu = musig[:, 0:1]
                    mu2 = mst.tile([1, 1], f32, tag="mu2")
                    nc.vector.tensor_tensor(out=mu2[:], in0=mu, in1=mu, op=OP.mult)
                    var = mst.tile([1, 1], f32, tag="var")
                    nc.vector.tensor_tensor(
                        out=var[:], in0=musig[:, 1:2], in1=mu2[:], op=OP.subtract
                    )
                    sigma = mst.tile([1, 1], f32, tag="sigma")
                    nc.scalar.sqrt(sigma[:], var[:])
                    # dummy exp: pull the Exp act-table load off the tail's
                    # critical path (table switch costs ~1.3us)
                    expd = mst.tile([1, 1], f32, tag="expd")
                    nc.scalar.activation(
                        out=expd[:], in_=var[:], func=ACTF.Exp,
                        bias=0.0, scale=1.0,
                    )
                    nc.vector.tensor_tensor(
                        out=Trow[:], in0=crow[:],
                        in1=sigma[:].to_broadcast([1, NLAD]), op=OP.mult,
                    )
                    nc.vector.tensor_tensor(
                        out=Trow[:], in0=Trow[:], in1=mu.to_broadcast([1, NLAD]),
                        op=OP.add,
                    )
                    ptb = ps.tile([P, NLAD], f32, tag="blk")
                    nc.tensor.matmul(
                        ptb[:P, :NLAD], ones_r1[:], Trow[:], start=True, stop=True
                    )
                    nc.vector.tensor_copy(Tb[:], ptb[:P, :NLAD])

            # ---------------- phase 3: ladder counts --------------------
            M_b = mst.tile([P, NL], bf16, tag="M_b")
            nc.vector.tensor_copy(M_b[:], M_all[:])
            # cmp[p, j, f] = M[p, f] >= T[j]  (bf16 in/out -> 2x DVE, and
            # PE can column-sum the bf16 result)
            cmpb = mst.tile([P, NLAD * NL], bf16, tag="cmpb")
            nc.vector.tensor_tensor(
                out=cmpb[:].rearrange("p (j f) -> p j f", f=NL),
                in0=M_b[:].rearrange("p (o f) -> p o f", o=1).to_broadcast([P, NLAD, NL]),
                in1=Tb[:].rearrange("p (j o) -> p j o", o=1).to_broadcast([P, NLAD, NL]),
                op=OP.is_ge,
            )
            # ---------------- Vmean -> ctx init (PE idle slot) ----------
            pvm = ps.tile([1, D], f32, tag="blk")
            n = 0
            for dc in range(ND):
                for lh, rh in (
                    (xmp[:, dc : dc + 1], wvh(dc, SD)),
                    (xmp[:, ND + dc : ND + dc + 1], wvh(dc, SD)),
                    (xmp[:, dc : dc + 1], wvl(dc, SD)),
                ):
                    nc.tensor.matmul(
                        pvm[:1, :], lh, rh,
                        start=(n == 0), stop=(n == 3 * ND - 1),
                    )
                    n += 1
            vmean = mst.tile([1, D], f32, tag="vmean")
            nc.scalar.copy(vmean[:], pvm[:1, :])
            pvb = ps.tile([P, D], f32, tag="blk")
            nc.tensor.matmul(pvb[:], ones_r1[:], vmean[:], start=True, stop=True)
            vmean_bc = mst.tile([P, D], f32, tag="vmean_bc")
            nc.vector.tensor_copy(vmean_bc[:], pvb[:])
            for jc in range(NL):
                nc.sync.dma_start(ctx_d[jc * P : (jc + 1) * P, :], vmean_bc[:])

            cnt01 = mst.tile([P, NLAD], f32, tag="cnt01")
            nc.vector.tensor_reduce(
                cnt01[:], cmpb[:].rearrange("p (j f) -> p j f", f=NL),
                axis=AX.X, op=OP.add,
            )
            pcc = ps.tile([1, NLAD], f32, tag="blk")
            nc.tensor.matmul(pcc[:1, :NLAD], ones_cf[:], cnt01[:], start=True, stop=True)
            cntrow = mst.tile([1, NLAD], f32, tag="cntrow")
            nc.vector.tensor_copy(cntrow[:], pcc[:1, :NLAD])
            # largest T with count >= 88; fallback smallest T with count <= 127
            okm = mst.tile([1, NLAD], u8, tag="okm")
            nc.vector.tensor_scalar(
                okm[:], cntrow[:], 87.5, None, op0=OP.is_ge
            )
            negrow = mst.tile([1, NLAD], f32, tag="negrow")
            nc.vector.memset(negrow[:], NEG)
            bigrow = mst.tile([1, NLAD], f32, tag="bigrow")
            nc.vector.memset(bigrow[:], BIG)
            tsel = mst.tile([1, NLAD], f32, tag="tsel")
            nc.vector.select(tsel[:], okm[:], Trow[:], negrow[:])
            tstar = mst.tile([1, 1], f32, tag="tstar")
            nc.vector.reduce_max(tstar[:], tsel[:], axis=AX.X)
            ok2 = mst.tile([1, NLAD], u8, tag="ok2")
            nc.vector.tensor_scalar(
                ok2[:], cntrow[:], 127.5, None, op0=OP.is_le
            )
            tsel2 = mst.tile([1, NLAD], f32, tag="tsel2")
            nc.vector.select(tsel2[:], ok2[:], Trow[:], bigrow[:])
            tfb = mst.tile([1, 1], f32, tag="tfb")
            nc.vector.tensor_reduce(tfb[:], tsel2[:], axis=AX.X, op=OP.min)
            have = mst.tile([1, 1], u8, tag="have")
            nc.vector.tensor_scalar(
                have[:], tstar[:], -1.0e30, None, op0=OP.is_ge
            )
            tfin = mst.tile([1, 1], f32, tag="tfin")
            nc.vector.select(tfin[:], have[:], tstar[:], tfb[:])
            ptf = ps.tile([P, 1], f32, tag="blk")
            nc.tensor.matmul(ptf[:P, :1], ones_r1[:], tfin[:], start=True, stop=True)
            tbc = mst.tile([P, 1], f32, tag="tbc")
            nc.vector.tensor_copy(tbc[:], ptf[:P, :1])

            # selmask / candidate index compaction
            selmask = mst.tile([P, NL], u8, tag="selmask")
            nc.vector.tensor_scalar(
                selmask[:], M_all[:], tbc[:], 0.0,
                op0=OP.subtract, op1=OP.is_ge,
            )
            midx = mst.tile([P, NL], f32, tag="midx")
            nc.vector.memset(midx[:], -1.0)
            nc.vector.copy_predicated(midx[:], selmask[:], qidx_f[:])
            pwr = ps.tile([16, P], f32, tag="blk", name="pwr")
            nc.tensor.transpose(pwr[:16, :P], midx[:], ident[:])
            wrap_in = mst.tile([16, P], f32, tag="wrap_in")
            nc.vector.tensor_copy(wrap_in[:], pwr[:16, :P])
            spg = mst.tile([16, 8], f32, tag="spg")
            nfound = mst.tile([1, 1], u32, tag="nfound")
            nc.gpsimd.sparse_gather(out=spg[:], in_=wrap_in[:], num_found=nfound[:])
            spg_cl = mst.tile([16, 8], f32, tag="spg_cl")
            nc.vector.tensor_scalar_max(spg_cl[:], spg[:], 0.0)
            nc.vector.tensor_scalar_min(spg_cl[:], spg_cl[:], float(L - 1))
            # fp16 keeps indices <= 2047 exact and avoids the fp32 double
            # LDWEIGHTS cost of the one-hot unwrap
            spg_h = mst.tile([16, 8], f16, tag="spg_h")
            nc.vector.tensor_copy(spg_h[:], spg_cl[:])
            pcq = ps.tile([P, 1], f32, tag="blk", name="pcq")
            for f in range(8):
                nc.tensor.matmul(
                    pcq[:P, :1], perm16[:, f * P : (f + 1) * P],
                    spg_h[:, f : f + 1],
                    start=(f == 0), stop=(f == 7),
                )
            candq_f = mst.tile([P, 1], f32, tag="candq_f")
            nc.vector.tensor_copy(candq_f[:], pcq[:P, :1])
            candq_i = mst.tile([P, 1], i32, tag="candq_i")
            nc.vector.tensor_copy(candq_i[:], pcq[:P, :1])
            nf_f = mst.tile([1, 1], f32, tag="nf_f")
            nc.vector.tensor_copy(nf_f[:], nfound[:])
            pnb = ps.tile([P, 1], f32, tag="blk")
            nc.tensor.matmul(pnb[:P, :1], ones_r1[:], nf_f[:], start=True, stop=True)
            nbc = mst.tile([P, 1], f32, tag="nbc")
            nc.vector.tensor_copy(nbc[:], pnb[:P, :1])
            invalid = mst.tile([P, 1], u8, tag="invalid")
            nc.vector.tensor_tensor(
                out=invalid[:], in0=qidx_f[:, 0:1], in1=nbc[:], op=OP.is_ge
            )

            # Keep-warm: ~3.5us of throwaway matmuls gated on candq_h so
            # they run exactly during the gather window; a >3.4us PE idle
            # here would drop the HAM clock to 1.2GHz for the whole exact
            # stage.
            candq_h = mst.tile([P, 1], bf16, tag="candq_h")
            nc.vector.tensor_copy(candq_h[:], pcq[:P, :1])
            pwarm = ps.tile([1, 512], f32, tag="blk", name="pwarm")
            for w in range(14):
                nc.tensor.matmul(
                    pwarm[:1, :512], candq_h[:, :1], xThp[:, 0:512],
                    start=True, stop=True,
                )

            # ---------------- phase 4: exact stage ----------------------
            x_cand = cnd.tile([P, D], f32, tag="x_cand")
            nc.gpsimd.indirect_dma_start(
                out=x_cand[:], out_offset=None, in_=x_d[:],
                in_offset=bass.IndirectOffsetOnAxis(ap=candq_i[:, :1], axis=0),
            )
            gm = cnd.tile([P, L], bf16, tag="gm")
            nc.gpsimd.indirect_dma_start(
                out=gm[:], out_offset=None, in_=maskb_d[:],
                in_offset=bass.IndirectOffsetOnAxis(ap=candq_i[:, :1], axis=0),
            )
            gc = cnd.tile([P, L], u8, tag="gc")
            nc.gpsimd.indirect_dma_start(
                out=gc[:], out_offset=None, in_=cnt_d[:],
                in_offset=bass.IndirectOffsetOnAxis(ap=candq_i[:, :1], axis=0),
            )

            # x_cand^T (fp32 — exact G via fp32 matmul, no hi/lo casts)
            xcT = [cnd.tile([P, P], f32, tag=f"xcT{dc}", name=f"xcT{dc}") for dc in range(ND)]
            for dc in range(ND):
                pxc = ps.tile([P, P], f32, tag="blk")
                nc.tensor.transpose(
                    pxc[:P, :P], x_cand[:, dc * P : (dc + 1) * P], ident[:]
                )
                nc.vector.tensor_copy(xcT[dc][:], pxc[:P, :P])

            # G^T computed directly: GT[dout, cand] = sum_din A[din, dout]^T
            # x_cand^T[din, cand] — 16 fp32 N=128 matmuls, no gsb round-trip
            GTh = [cnd.tile([P, P], bf16, tag=f"GTh{dc}", name=f"GTh{dc}") for dc in range(ND)]
            GTl = [cnd.tile([P, P], bf16, tag=f"GTl{dc}", name=f"GTl{dc}") for dc in range(ND)]
            for do in range(ND):
                osl = slice(do * P, (do + 1) * P)
                pgt = ps.tile([P, P], f32, tag="blk")
                for di in range(ND):
                    nc.tensor.matmul(
                        pgt[:P, :P], Af(di, osl), xcT[di][:],
                        start=(di == 0), stop=(di == ND - 1),
                    )
                nc.vector.tensor_copy(GTh[do][:], pgt[:P, :P])
                nc.vector.tensor_tensor(
                    out=GTl[do][:], in0=pgt[:P, :P], in1=GTh[do][:],
                    op=OP.subtract,
                )

            # S_cand = G @ x^T (3-term bf16), 2 held [P,1024] PSUM pairs
            psS = []
            cmax = cnd.tile([P, 2], f32, tag="cmax")
            csum = cnd.tile([P, 2], f32, tag="csum")
            for jp in range(2):
                pss2 = ps_s.tile([P, 1024], f32, tag="psSc")
                psS.append(pss2)
                for jh in range(2):
                    jb = jp * 2 + jh
                    jsl = slice(jb * 512, (jb + 1) * 512)
                    n = 0
                    for dc in range(ND):
                        for lh, rh in (
                            (GTh[dc][:], xTh(dc, jsl)),
                            (GTl[dc][:], xTh(dc, jsl)),
                            (GTh[dc][:], xTl(dc, jsl)),
                        ):
                            nc.tensor.matmul(
                                pss2[:, jh * 512 : (jh + 1) * 512], lh, rh,
                                start=(n == 0), stop=(n == 3 * ND - 1),
                            )
                            n += 1

            # ---------------- phase 5: softmax + update -----------------
            exp_sb = expp.tile([P, L], bf16, tag="exp_sb")
            sume4 = expp.tile([P, 2], f32, tag="sume4")
            for jp in range(2):
                psl = slice(jp * 1024, (jp + 1) * 1024)
                nc.scalar.activation(
                    out=exp_sb[:, psl], in_=psS[jp][:], func=ACTF.Exp,
                    bias=0.0, scale=SCALE,
                    accum_out=sume4[:, jp : jp + 1],
                )
            sume = expp.tile([P, 1], f32, tag="sume")
            nc.vector.reduce_sum(sume[:], sume4[:], axis=AX.X)
            recip = expp.tile([P, 1], f32, tag="recip")
            nc.vector.reciprocal(recip[:], sume[:])

            # expT transposes software-pipelined with the G2 accumulation
            # (depth 4) so the PE never idles long enough to re-throttle
            expT = [expp.tile([P, P], bf16, tag=f"expT{jc}", name=f"expT{jc}") for jc in range(NL)]
            pu = ps.tile([P, D], f32, tag="blk")

            def g2_mm(jc):
                nc.tensor.matmul(
                    pu[:], expT[jc][:], xNh(jc),
                    start=(jc == 0), stop=(jc == NL - 1),
                    skip_group_check=True,
                )

            for jc in range(NL):
                pet = psb.tile([P, P], bf16, tag="blkb")
                nc.tensor.transpose(
                    pet[:P, :P], exp_sb[:, jc * P : (jc + 1) * P], ident_b[:]
                )
                if jc % 2 == 0:
                    nc.vector.tensor_copy(expT[jc][:], pet[:P, :P])
                else:
                    nc.scalar.copy(expT[jc][:], pet[:P, :P])
                if jc >= 3:
                    g2_mm(jc - 3)
            for jc in range(NL - 3, NL):
                g2_mm(jc)

            # ---- exact M + top-40 (emitted late so the expT copies get
            # DVE priority; this chain only gates the scatter indices) ----
            for jp in range(2):
                pss2 = psS[jp]
                psl = slice(jp * 1024, (jp + 1) * 1024)
                s3 = scr.tile([P, 1024], f32, tag="scrt2")
                nc.vector.tensor_tensor(
                    out=s3[:], in0=pss2[:], in1=gm[:, psl], op=OP.mult
                )
                nc.vector.reduce_max(cmax[:, jp : jp + 1], s3[:], axis=AX.X)
                s4 = scr.tile([P, 1024], f32, tag="scrt2")
                nc.vector.scalar_tensor_tensor(
                    out=s4[:], in0=pss2[:], scalar=-1.0 / L, in1=gc[:, psl],
                    op0=OP.mult, op1=OP.mult,
                    accum_out=csum[:, jp : jp + 1],
                )
            u1 = cnd.tile([P, 1], f32, tag="u1")
            u2 = cnd.tile([P, 1], f32, tag="u2")
            M_cand = cnd.tile([P, 1], f32, tag="M_cand")
            nc.vector.reduce_max(u1[:], cmax[:], axis=AX.X)
            nc.vector.reduce_sum(u2[:], csum[:], axis=AX.X)
            nc.vector.tensor_tensor(out=M_cand[:], in0=u1[:], in1=u2[:], op=OP.add)
            nc.vector.copy_predicated(M_cand[:], invalid[:], negbig[:])

            # exact top-40 threshold among candidates
            pmc = ps.tile([1, P], f32, tag="blk")
            nc.tensor.transpose(pmc[:1, :P], M_cand[:], ident[:])
            mcT = cnd.tile([1, P], f32, tag="mcT")
            nc.vector.tensor_copy(mcT[:], pmc[:1, :P])
            etop = cnd.tile([1, NT], f32, tag="etop")
            for r in range(5):
                nc.vector.max(out=etop[:, 8 * r : 8 * r + 8], in_=mcT[:])
                if r < 4:
                    nc.vector.match_replace(
                        out=mcT[:], in_to_replace=etop[:, 8 * r : 8 * r + 8],
                        in_values=mcT[:], imm_value=NEG,
                    )
            pte = ps.tile([P, 1], f32, tag="blk")
            nc.tensor.matmul(
                pte[:P, :1], ones_r1[:], etop[:, NT - 1 : NT], start=True, stop=True
            )
            tebc = cnd.tile([P, 1], f32, tag="tebc")
            nc.vector.tensor_copy(tebc[:], pte[:P, :1])
            sel2 = cnd.tile([P, 1], u8, tag="sel2")
            nc.vector.tensor_tensor(
                out=sel2[:], in0=M_cand[:], in1=tebc[:], op=OP.is_ge
            )
            scat_f = cnd.tile([P, 1], f32, tag="scat_f")
            nc.vector.tensor_copy(scat_f[:], big9[:])
            nc.vector.copy_predicated(scat_f[:], sel2[:], candq_f[:])
            scat_i = cnd.tile([P, 1], i32, tag="scat_i")
            nc.vector.tensor_copy(scat_i[:], scat_f[:])
            g2b = expp.tile([P, D], bf16, tag="g2b")
            nc.scalar.copy(g2b[:], pu[:])
            G2T = [expp.tile([P, P], bf16, tag=f"G2T{dc}", name=f"G2T{dc}") for dc in range(ND)]
            for dc in range(ND):
                pg2 = psb.tile([P, P], bf16, tag="blkb")
                nc.tensor.transpose(
                    pg2[:P, :P], g2b[:, dc * P : (dc + 1) * P], ident_b[:]
                )
                nc.vector.tensor_copy(G2T[dc][:], pg2[:P, :P])
            # upd = G2 @ Wv^T / sums
            pup = ps.tile([P, D], f32, tag="blk")
            for dc in range(ND):
                nc.tensor.matmul(
                    pup[:], G2T[dc][:], wvh(dc, SD),
                    start=(dc == 0), stop=(dc == ND - 1),
                )
            upd = expp.tile([P, D], f32, tag="upd")
            nc.scalar.activation(
                out=upd[:], in_=pup[:], func=ACTF.Copy, bias=0.0, scale=recip[:]
            )
            nc.gpsimd.indirect_dma_start(
                out=ctx_d[:],
                out_offset=bass.IndirectOffsetOnAxis(ap=scat_i[:, :1], axis=0),
                in_=upd[:], in_offset=None,
                bounds_check=L - 1, oob_is_err=False,
            )

    nc.compile()
    return nc


_NC = None


def _get_nc():
    global _NC
    if _NC is None:
        _NC = build()
    return _NC


def _split_bf16(a):
    hi = a.astype(ml_dtypes.bfloat16)
    lo = (a - hi.astype(np.float32)).astype(ml_dtypes.bfloat16)
    return hi, lo


def _host_prep(x, Wq, Wk, Wv, index_sample):
    x = np.asarray(x, dtype=np.float32)
    Wq = np.asarray(Wq, dtype=np.float32)
    Wk = np.asarray(Wk, dtype=np.float32)
    Wv = np.asarray(Wv, dtype=np.float32)
    idx = np.asarray(index_sample)

    def pack(m):
        # [ND*P, W] -> [P, ND*W]: row dc*128+p lands at columns dc*W..+W
        nd = m.shape[0] // P
        return np.ascontiguousarray(
            m.reshape(nd, P, m.shape[1]).transpose(1, 0, 2).reshape(P, -1)
        )

    def pack_jb(m):
        # [ND*P, NJ*512] -> [P, NJ*ND*512] (jb-major blocks)
        nd = m.shape[0] // P
        nj = m.shape[1] // 512
        return np.ascontiguousarray(
            m.reshape(nd, P, nj, 512).transpose(1, 2, 0, 3).reshape(P, -1)
        )

    A = (Wq.T.astype(np.float64) @ Wk.astype(np.float64)).astype(np.float32)
    Ah = A.astype(ml_dtypes.bfloat16)
    wvh, wvl = _split_bf16(np.ascontiguousarray(Wv.T))

    rows = np.arange(L)[:, None]
    maskb = np.zeros((L, L), dtype=ml_dtypes.bfloat16)
    maskb[rows, idx] = 1
    countf = np.zeros((L, L), dtype=np.uint8)
    np.add.at(countf, (rows, idx), 1)

    perm16 = np.zeros((16, 8 * P), dtype=np.float16)
    for f in range(8):
        for p in range(16):
            perm16[p, f * P + p + 16 * f] = 1.0
    qidxf = (np.arange(P)[:, None] + 128 * np.arange(NL)[None, :]).astype(np.float32)
    crow = (1.2 + np.arange(NLAD, dtype=np.float32) * 0.134).reshape(1, NLAD)

    shared = {
        "Ahp": pack(Ah), "Afp": pack(A),
        "wvThp": pack(wvh), "wvTlp": pack(wvl),
        "maskb": maskb, "countf": countf, "perm16": perm16,
        "qidxf": qidxf, "crow": crow,
    }
    in_maps = []
    for b in range(B):
        xb = np.ascontiguousarray(x[b])
        xT = np.ascontiguousarray(xb.T)
        xth, xtl = _split_bf16(xT)
        xnh = xb.astype(ml_dtypes.bfloat16)
        xmean = xb.astype(np.float64).mean(axis=0).astype(np.float32)
        xmeh, xmel = _split_bf16(xmean.reshape(1, D))
        xm = np.concatenate(
            [xmeh.reshape(ND, P).T, xmel.reshape(ND, P).T], axis=1
        ).astype(ml_dtypes.bfloat16)
        in_maps.append(
            {
                "x_nat": xb,
                "xThp": pack_jb(xth),
                "xTlp": pack_jb(xtl),
                "xNhp": pack(xnh),
                "xmp": np.ascontiguousarray(xm),
                **shared,
            }
        )
    return in_maps


def kernel(x, Wq, Wk, Wv, index_sample, _trace=False, _result_box=None):
    in_maps = _host_prep(x, Wq, Wk, Wv, index_sample)
    nc = _get_nc()
    res = run_bass_kernel_spmd(nc, in_maps, core_ids=list(range(B)), trace=_trace)
    if _result_box is not None:
        _result_box.append(res)
    out = np.stack([np.asarray(res.results[b]["ctx"]) for b in range(B)], axis=0)
    return out


# revision 54
# speedup vs baseline: 2.0124x; 1.0028x over previous
"""Sparse attention (ProbSparse-style) Trainium2 Bass kernel, v2.

Problem (per batch element b, data-parallel over 8 NeuronCores):
  Q = x @ Wq.T ; K = x @ Wk.T ; V = x @ Wv.T            [L=2048, D=512]
  QK_sample[l,s] = Q[l] . K[index_sample[l,s]]           [L, 40]
  M[l] = max_s QK_sample - sum_s QK_sample / L
  sel = top40(M)  (as a set; the reference scatter makes order irrelevant)
  scores = Q[sel] @ K.T / sqrt(D); attn = softmax(scores)
  ctx = broadcast(mean(V)); ctx[sel] = attn @ V

Key ideas vs v1 baseline:
  - A = Wq^T @ Wk precomputed on host: S = (x A) x^T. Kills the K and Q
    projections entirely; both approx and exact scores contract against
    the resident x^T tiles.
  - Approx M = masked max of bf16 S only (the sum/L term is <= ~0.5 and
    is absorbed by the candidate margin; validated: true top-40 rows sit
    within rank <= 40 of the approx ordering).
  - Threshold via a 64-step mu + c*sigma ladder with on-device counts
    (one 3d-broadcast compare + reduce + PE column-sum), picking the
    largest T with count >= 88 (fallback: smallest T with count <= 127).
    Replaces the 62us GPSIMD kth_largest.
  - Exact stage on <= 128 candidates: G = x_cand A (3-term bf16),
    S_cand = G x^T (3-term bf16)  ->  ~1e-4-class absolute error,
    validated 26x under the seed-0 top-40 boundary gap.
  - Softmax without max subtraction (|S*scale| <= ~9, exp is safe),
    upd = (attn @ x) @ Wv^T (kills the V projection; V never built).

kernel(**inputs) accepts FULL inputs, returns FULL [8, 2048, 512] f32;
batch is sharded over 8 cores.
"""

import math

import numpy as np
import ml_dtypes

import concourse.bacc as bacc
import concourse.bass as bass
import concourse.mybir as mybir
import concourse.tile as tile
from concourse.bass_utils import run_bass_kernel_spmd
from concourse.masks import make_identity
from concourse import library_config

P = 128
L = 2048
D = 512
B = 8
NL = L // P        # 16 query chunks
ND = D // P        # 4 feature chunks
NJ = L // 512      # 4 key blocks of 512
NT = 40
NLAD = 32          # threshold ladder steps
SCALE = 1.0 / math.sqrt(D)
NEG = -3.0e38
BIG = 3.0e38
SKIP_IDX = 99999.0  # scatter index sentinel (> bounds_check -> row skipped)

f32 = mybir.dt.float32
f16 = mybir.dt.float16
bf16 = mybir.dt.bfloat16
u8 = mybir.dt.uint8
i32 = mybir.dt.int32
u32 = mybir.dt.uint32
AX = mybir.AxisListType
OP = mybir.AluOpType
ACTF = mybir.ActivationFunctionType


def build():
    nc = bacc.Bacc("TRN2", target_bir_lowering=False)

    # All big operands are host-packed into [128, wide] layouts so each
    # DMA partition line is a 4-16KB contiguous DRAM run (1KB lines were
    # descriptor-bound: ~26us of startup).
    x_d = nc.dram_tensor("x_nat", [L, D], f32, kind="ExternalInput")
    xth_d = nc.dram_tensor("xThp", [P, ND * L], bf16, kind="ExternalInput")
    xtl_d = nc.dram_tensor("xTlp", [P, ND * L], bf16, kind="ExternalInput")
    xnh_d = nc.dram_tensor("xNhp", [P, NL * D], bf16, kind="ExternalInput")
    ah_d = nc.dram_tensor("Ahp", [P, ND * D], bf16, kind="ExternalInput")
    af_d = nc.dram_tensor("Afp", [P, ND * D], f32, kind="ExternalInput")
    wvh_d = nc.dram_tensor("wvThp", [P, ND * D], bf16, kind="ExternalInput")
    wvl_d = nc.dram_tensor("wvTlp", [P, ND * D], bf16, kind="ExternalInput")
    xm_d = nc.dram_tensor("xmp", [P, 2 * ND], bf16, kind="ExternalInput")
    maskb_d = nc.dram_tensor("maskb", [L, L], bf16, kind="ExternalInput")
    cnt_d = nc.dram_tensor("countf", [L, L], u8, kind="ExternalInput")
    perm_d = nc.dram_tensor("perm16", [16, 8 * P], f16, kind="ExternalInput")
    qidx_d = nc.dram_tensor("qidxf", [P, NL], f32, kind="ExternalInput")
    crow_d = nc.dram_tensor("crow", [1, NLAD], f32, kind="ExternalInput")
    ctx_d = nc.dram_tensor("ctx", [L, D], f32, kind="ExternalOutput")

    with tile.TileContext(nc) as tc:
        with (
            tc.tile_pool(name="const", bufs=1) as cst,
            tc.tile_pool(name="xres", bufs=1) as xres,      # resident x / A / Wv
            tc.tile_pool(name="proj", bufs=1) as proj,      # QATb
            tc.tile_pool(name="mstuff", bufs=1) as mst,     # M / threshold smalls
            tc.tile_pool(name="mstream", bufs=3) as mstr,   # mask chunks
            tc.tile_pool(name="scr", bufs=3) as scr,        # TTR scratch
            tc.tile_pool(name="cand", bufs=1) as cnd,       # exact-stage tiles
            tc.tile_pool(name="expp", bufs=1) as expp,      # softmax/upd tiles
            tc.tile_pool(name="ps", bufs=2, space="PSUM") as ps,
            tc.tile_pool(name="psb", bufs=2, space="PSUM") as psb,    # bf16 transposes
            tc.tile_pool(name="ps_s", bufs=2, space="PSUM") as ps_s,  # S pairs
        ):
            # ---------------- constants ----------------
            ident = cst.tile([P, P], f32, tag="ident")
            make_identity(nc, ident[:])
            ident_b = cst.tile([P, P], bf16, tag="ident_b")
            nc.vector.tensor_copy(ident_b[:], ident[:])
            # preload the sparse_gather ucode so the serial tail does not
            # pay the library switch
            nc.gpsimd.load_library(library_config.sparse_gather)
            ones_r1 = cst.tile([1, P], f32, tag="ones_r1")
            nc.vector.memset(ones_r1[:], 1.0)
            ones_cf = cst.tile([P, 1], f32, tag="ones_cf")
            nc.vector.memset(ones_cf[:], 1.0)
            negbig = cst.tile([P, 1], f32, tag="negbig")
            nc.vector.memset(negbig[:], NEG)
            big9 = cst.tile([P, 1], f32, tag="big9")
            nc.vector.memset(big9[:], SKIP_IDX)
            qidx_f = cst.tile([P, NL], f32, tag="qidx_f")
            nc.sync.dma_start(qidx_f[:], qidx_d[:])
            crow = cst.tile([1, NLAD], f32, tag="crow")
            nc.sync.dma_start(crow[:], crow_d[:])
            perm16 = cst.tile([16, 8 * P], f16, tag="perm16")
            nc.sync.dma_start(perm16[:], perm_d[:])

            # ---------------- critical loads (packed, big lines) --------
            # Only Ahp + xThp gate the first matmuls; everything the tail
            # needs is DMA'd from inside the phase-2 loop so it doesn't
            # compete for startup bandwidth.
            Ahp = xres.tile([P, ND * D], bf16, tag="Ahp")
            nc.sync.dma_start(Ahp[:], ah_d[:])
            # x^T hi is packed jb-major: block jb holds [dc=0..3][512 cols]
            # so the first QA matmuls start after Ahp + one 0.5MB block
            xThp = xres.tile([P, ND * L], bf16, tag="xThp")
            for jb in range(NJ):
                nc.sync.dma_start(
                    xThp[:, jb * 2048 : (jb + 1) * 2048],
                    xth_d[:, jb * 2048 : (jb + 1) * 2048],
                )
            Afp = xres.tile([P, ND * D], f32, tag="Afp")
            wvhp = xres.tile([P, ND * D], bf16, tag="wvhp")
            wvlp = xres.tile([P, ND * D], bf16, tag="wvlp")
            xmp = xres.tile([P, 2 * ND], bf16, tag="xmp")
            xTlp = xres.tile([P, ND * L], bf16, tag="xTlp")
            xNhp = xres.tile([P, NL * D], bf16, tag="xNhp")

            # slice helpers over the packed tiles
            Ah = lambda dc, js: Ahp[:, dc * D + js.start : dc * D + js.stop]
            Af = lambda dc, js: Afp[:, dc * D + js.start : dc * D + js.stop]
            wvh = lambda dc, js: wvhp[:, dc * D + js.start : dc * D + js.stop]
            wvl = lambda dc, js: wvlp[:, dc * D + js.start : dc * D + js.stop]

            def _xt(tile_, dc, js):
                # jb-major packing: block jb*2048 + dc*512
                jb, r = divmod(js.start, 512)
                assert js.stop - js.start == 512 and r == 0
                off = jb * 2048 + dc * 512
                return tile_[:, off : off + 512]

            xTh = lambda dc, js: _xt(xThp, dc, js)
            xTl = lambda dc, js: _xt(xTlp, dc, js)
            xNh = lambda jc: xNhp[:, jc * D : (jc + 1) * D]
            SD = slice(0, D)
            SL = slice(0, L)

            # ---------------- phase 1: QA^T = A^T x^T (bf16) ------------
            QATb = [proj.tile([P, L], bf16, tag=f"QATb{ic}", name=f"QATb{ic}") for ic in range(ND)]
            for jb in range(NJ):
                jsl = slice(jb * 512, (jb + 1) * 512)
                for ic in range(ND):
                    isl = slice(ic * P, (ic + 1) * P)
                    pq = ps.tile([P, 512], f32, tag="blk")
                    for dc in range(ND):
                        nc.tensor.matmul(
                            pq[:], Ah(dc, isl), xTh(dc, jsl),
                            start=(dc == 0), stop=(dc == ND - 1),
                        )
                    nc.scalar.copy(QATb[ic][:, jsl], pq[:])

            # ---------------- phase 2: approx S + masked max ------------
            # Per 128-query chunk: PE computes 4 S blocks; ScalarE evicts
            # them to a bf16 row [P, 2048]; DVE does one 2x bf16 mask-mult
            # + one wide reduce_max.  (tensor_tensor_reduce crashes TRN2
            # hardware, so the fused form is not available.)
            M_all = mst.tile([P, NL], f32, tag="M_all")
            M_lo = mst.tile([P, 8], f32, tag="M_lo")
            Trow = mst.tile([1, NLAD], f32, tag="Trow")
            Tb = mst.tile([P, NLAD], bf16, tag="Tb")
            for lc in range(NL):
                lsl = slice(lc * P, (lc + 1) * P)
                mkb = mstr.tile([P, L], bf16, tag="mkb")
                nc.sync.dma_start(mkb[:], maskb_d[lsl, :])
                # tail-only loads trickled in behind the mask stream
                if lc == 0:
                    nc.sync.dma_start(wvhp[:], wvh_d[:])
                    nc.sync.dma_start(wvlp[:], wvl_d[:])
                    nc.sync.dma_start(xmp[:], xm_d[:])
                elif lc == 2:
                    nc.sync.dma_start(Afp[:], af_d[:])
                elif lc == 5:
                    nc.sync.dma_start(xTlp[:], xtl_d[:])
                elif lc == 9:
                    nc.sync.dma_start(xNhp[:], xnh_d[:])
                sb1 = scr.tile([P, L], bf16, tag="sb1")
                for jp in range(2):
                    # paired PSUM banks -> one wide eviction per 1024 cols
                    pss = ps_s.tile([P, 1024], f32, tag="psSc", name="pssa")
                    for jh in range(2):
                        jb = jp * 2 + jh
                        jsl = slice(jb * 512, (jb + 1) * 512)
                        for ic in range(ND):
                            nc.tensor.matmul(
                                pss[:, jh * 512 : (jh + 1) * 512],
                                QATb[ic][:, lsl], xTh(ic, jsl),
                                start=(ic == 0), stop=(ic == ND - 1),
                            )
                    nc.scalar.copy(
                        sb1[:, jp * 1024 : (jp + 1) * 1024], pss[:]
                    )
                # masked max: two 2x-mode masked products, one 2x max
                # combine, then a half-width 1x reduce
                t0 = scr.tile([P, 1024], bf16, tag="t0m")
                nc.vector.tensor_tensor(
                    out=t0[:], in0=sb1[:, 0:1024], in1=mkb[:, 0:1024],
                    op=OP.mult,
                )
                t1 = scr.tile([P, 1024], bf16, tag="t1m")
                nc.vector.tensor_tensor(
                    out=t1[:], in0=sb1[:, 1024:2048], in1=mkb[:, 1024:2048],
                    op=OP.mult,
                )
                t2 = scr.tile([P, 1024], bf16, tag="t2m")
                nc.vector.tensor_tensor(
                    out=t2[:], in0=t0[:], in1=t1[:], op=OP.max
                )
                if lc < 8:
                    nc.vector.reduce_max(M_lo[:, lc : lc + 1], t2[:], axis=AX.X)
                    nc.vector.tensor_copy(
                        M_all[:, lc : lc + 1], M_lo[:, lc : lc + 1]
                    )
                else:
                    nc.vector.reduce_max(M_all[:, lc : lc + 1], t2[:], axis=AX.X)
                if lc == 7:
                    # ---- early threshold stats on the first 1024 rows --
                    # (mu/sigma only steer the ladder range; the counts
                    # below verify against the full M) -- this whole chain
                    # runs under the second half of the main phase.
                    stats2 = mst.tile([P, 2], f32, tag="stats2")
                    msq = mst.tile([P, 8], f32, tag="msq")
                    nc.vector.scalar_tensor_tensor(
                        out=msq[:], in0=M_lo[:], scalar=1.0, in1=M_lo[:],
                        op0=OP.mult, op1=OP.mult,
                        accum_out=stats2[:, 1:2],
                    )
                    nc.vector.tensor_reduce(
                        stats2[:, 0:1], M_lo[:], axis=AX.X, op=OP.add
                    )
                    pst = ps.tile([1, 2], f32, tag="blk")
                    nc.tensor.matmul(
                        pst[:1, :2], ones_cf[:], stats2[:], start=True, stop=True
                    )
                    srow = mst.tile([1, 2], f32, tag="srow")
                    nc.vector.tensor_copy(srow[:], pst[:1, :2])
                    musig = mst.tile([1, 2], f32, tag="musig")
                    nc.vector.tensor_scalar_mul(musig[:], srow[:], 1.0 / 1024.0)
                    mu = musig[:, 0:1]
                    mu2 = mst.tile([1, 1], f32, tag="mu2")
                    nc.vector.tensor_tensor(out=mu2[:], in0=mu, in1=mu, op=OP.mult)
                    var = mst.tile([1, 1], f32, tag="var")
                    nc.vector.tensor_tensor(
                        out=var[:], in0=musig[:, 1:2], in1=mu2[:], op=OP.subtract
                    )
                    sigma = mst.tile([1, 1], f32, tag="sigma")
                    nc.scalar.sqrt(sigma[:], var[:])
                    # dummy exp: pull the Exp act-table load off the tail's
                    # critical path (table switch costs ~1.3us)
                    expd = mst.tile([1, 1], f32, tag="expd")
                    nc.scalar.activation(
                        out=expd[:], in_=var[:], func=ACTF.Exp,
                        bias=0.0, scale=0.0,
                    )
                    nc.vector.tensor_tensor(
                        out=Trow[:], in0=crow[:],
                        in1=sigma[:].to_broadcast([1, NLAD]), op=OP.mult,
                    )
                    nc.vector.tensor_tensor(
                        out=Trow[:], in0=Trow[:], in1=mu.to_broadcast([1, NLAD]),
                        op=OP.add,
                    )
                    ptb = ps.tile([P, NLAD], f32, tag="blk")
                    nc.tensor.matmul(
                        ptb[:P, :NLAD], ones_r1[:], Trow[:], start=True, stop=True
                    )
                    nc.vector.tensor_copy(Tb[:], ptb[:P, :NLAD])

            # ---------------- phase 3: ladder counts --------------------
            M_b = mst.tile([P, NL], bf16, tag="M_b")
            nc.vector.tensor_copy(M_b[:], M_all[:])
            # cmp[p, j, f] = M[p, f] >= T[j]  (bf16 in/out -> 2x DVE, and
            # PE can column-sum the bf16 result)
            cmpb = mst.tile([P, NLAD * NL], bf16, tag="cmpb")
            nc.vector.tensor_tensor(
                out=cmpb[:].rearrange("p (j f) -> p j f", f=NL),
                in0=M_b[:].rearrange("p (o f) -> p o f", o=1).to_broadcast([P, NLAD, NL]),
                in1=Tb[:].rearrange("p (j o) -> p j o", o=1).to_broadcast([P, NLAD, NL]),
                op=OP.is_ge,
            )
            # ---------------- Vmean -> ctx init (PE idle slot) ----------
            pvm = ps.tile([1, D], f32, tag="blk")
            n = 0
            for dc in range(ND):
                for lh, rh in (
                    (xmp[:, dc : dc + 1], wvh(dc, SD)),
                    (xmp[:, ND + dc : ND + dc + 1], wvh(dc, SD)),
                    (xmp[:, dc : dc + 1], wvl(dc, SD)),
                ):
                    nc.tensor.matmul(
                        pvm[:1, :], lh, rh,
                        start=(n == 0), stop=(n == 3 * ND - 1),
                    )
                    n += 1
            vmean = mst.tile([1, D], f32, tag="vmean")
            nc.scalar.copy(vmean[:], pvm[:1, :])
            pvb = ps.tile([P, D], f32, tag="blk")
            nc.tensor.matmul(pvb[:], ones_r1[:], vmean[:], start=True, stop=True)
            vmean_bc = mst.tile([P, D], f32, tag="vmean_bc")
            nc.vector.tensor_copy(vmean_bc[:], pvb[:])
            for jc in range(NL):
                nc.sync.dma_start(ctx_d[jc * P : (jc + 1) * P, :], vmean_bc[:])

            cnt01 = mst.tile([P, NLAD], f32, tag="cnt01")
            nc.vector.tensor_reduce(
                cnt01[:], cmpb[:].rearrange("p (j f) -> p j f", f=NL),
                axis=AX.X, op=OP.add,
            )
            pcc = ps.tile([1, NLAD], f32, tag="blk")
            nc.tensor.matmul(pcc[:1, :NLAD], ones_cf[:], cnt01[:], start=True, stop=True)
            cntrow = mst.tile([1, NLAD], f32, tag="cntrow")
            nc.vector.tensor_copy(cntrow[:], pcc[:1, :NLAD])
            # largest T with count >= 88; fallback smallest T with count <= 127
            okm = mst.tile([1, NLAD], u8, tag="okm")
            nc.vector.tensor_scalar(
                okm[:], cntrow[:], 87.5, None, op0=OP.is_ge
            )
            negrow = mst.tile([1, NLAD], f32, tag="negrow")
            nc.vector.memset(negrow[:], NEG)
            bigrow = mst.tile([1, NLAD], f32, tag="bigrow")
            nc.vector.memset(bigrow[:], BIG)
            tsel = mst.tile([1, NLAD], f32, tag="tsel")
            nc.vector.select(tsel[:], okm[:], Trow[:], negrow[:])
            tstar = mst.tile([1, 1], f32, tag="tstar")
            nc.vector.reduce_max(tstar[:], tsel[:], axis=AX.X)
            ok2 = mst.tile([1, NLAD], u8, tag="ok2")
            nc.vector.tensor_scalar(
                ok2[:], cntrow[:], 127.5, None, op0=OP.is_le
            )
            tsel2 = mst.tile([1, NLAD], f32, tag="tsel2")
            nc.vector.select(tsel2[:], ok2[:], Trow[:], bigrow[:])
            tfb = mst.tile([1, 1], f32, tag="tfb")
            nc.vector.tensor_reduce(tfb[:], tsel2[:], axis=AX.X, op=OP.min)
            have = mst.tile([1, 1], u8, tag="have")
            nc.vector.tensor_scalar(
                have[:], tstar[:], -1.0e30, None, op0=OP.is_ge
            )
            tfin = mst.tile([1, 1], f32, tag="tfin")
            nc.vector.select(tfin[:], have[:], tstar[:], tfb[:])
            ptf = ps.tile([P, 1], f32, tag="blk")
            nc.tensor.matmul(ptf[:P, :1], ones_r1[:], tfin[:], start=True, stop=True)
            tbc = mst.tile([P, 1], f32, tag="tbc")
            nc.vector.tensor_copy(tbc[:], ptf[:P, :1])

            # selmask / candidate index compaction
            selmask = mst.tile([P, NL], u8, tag="selmask")
            nc.vector.tensor_scalar(
                selmask[:], M_all[:], tbc[:], 0.0,
                op0=OP.subtract, op1=OP.is_ge,
            )
            midx = mst.tile([P, NL], f32, tag="midx")
            nc.vector.memset(midx[:], -1.0)
            nc.vector.copy_predicated(midx[:], selmask[:], qidx_f[:])
            pwr = ps.tile([16, P], f32, tag="blk", name="pwr")
            nc.tensor.transpose(pwr[:16, :P], midx[:], ident[:])
            wrap_in = mst.tile([16, P], f32, tag="wrap_in")
            nc.vector.tensor_copy(wrap_in[:], pwr[:16, :P])
            spg = mst.tile([16, 8], f32, tag="spg")
            nfound = mst.tile([1, 1], u32, tag="nfound")
            nc.gpsimd.sparse_gather(out=spg[:], in_=wrap_in[:], num_found=nfound[:])
            spg_cl = mst.tile([16, 8], f32, tag="spg_cl")
            nc.vector.tensor_scalar_max(spg_cl[:], spg[:], 0.0)
            nc.vector.tensor_scalar_min(spg_cl[:], spg_cl[:], float(L - 1))
            # fp16 keeps indices <= 2047 exact and avoids the fp32 double
            # LDWEIGHTS cost of the one-hot unwrap
            spg_h = mst.tile([16, 8], f16, tag="spg_h")
            nc.vector.tensor_copy(spg_h[:], spg_cl[:])
            pcq = ps.tile([P, 1], f32, tag="blk", name="pcq")
            for f in range(8):
                nc.tensor.matmul(
                    pcq[:P, :1], perm16[:, f * P : (f + 1) * P],
                    spg_h[:, f : f + 1],
                    start=(f == 0), stop=(f == 7),
                )
            candq_f = mst.tile([P, 1], f32, tag="candq_f")
            nc.vector.tensor_copy(candq_f[:], pcq[:P, :1])
            candq_i = mst.tile([P, 1], i32, tag="candq_i")
            nc.vector.tensor_copy(candq_i[:], pcq[:P, :1])
            nf_f = mst.tile([1, 1], f32, tag="nf_f")
            nc.vector.tensor_copy(nf_f[:], nfound[:])
            pnb = ps.tile([P, 1], f32, tag="blk")
            nc.tensor.matmul(pnb[:P, :1], ones_r1[:], nf_f[:], start=True, stop=True)
            nbc = mst.tile([P, 1], f32, tag="nbc")
            nc.vector.tensor_copy(nbc[:], pnb[:P, :1])
            invalid = mst.tile([P, 1], u8, tag="invalid")
            nc.vector.tensor_tensor(
                out=invalid[:], in0=qidx_f[:, 0:1], in1=nbc[:], op=OP.is_ge
            )

            # Keep-warm: ~3.5us of throwaway matmuls gated on candq_h so
            # they run exactly during the gather window; a >3.4us PE idle
            # here would drop the HAM clock to 1.2GHz for the whole exact
            # stage.
            candq_h = mst.tile([P, 1], bf16, tag="candq_h")
            nc.vector.tensor_copy(candq_h[:], pcq[:P, :1])
            pwarm = ps.tile([1, 512], f32, tag="blk", name="pwarm")
            for w in range(14):
                nc.tensor.matmul(
                    pwarm[:1, :512], candq_h[:, :1], xThp[:, 0:512],
                    start=True, stop=True,
                )

            # ---------------- phase 4: exact stage ----------------------
            x_cand = cnd.tile([P, D], f32, tag="x_cand")
            nc.gpsimd.indirect_dma_start(
                out=x_cand[:], out_offset=None, in_=x_d[:],
                in_offset=bass.IndirectOffsetOnAxis(ap=candq_i[:, :1], axis=0),
            )
            gm = cnd.tile([P, L], bf16, tag="gm")
            nc.gpsimd.indirect_dma_start(
                out=gm[:], out_offset=None, in_=maskb_d[:],
                in_offset=bass.IndirectOffsetOnAxis(ap=candq_i[:, :1], axis=0),
            )
            gc = cnd.tile([P, L], u8, tag="gc")
            nc.gpsimd.indirect_dma_start(
                out=gc[:], out_offset=None, in_=cnt_d[:],
                in_offset=bass.IndirectOffsetOnAxis(ap=candq_i[:, :1], axis=0),
            )

            # x_cand^T (fp32 — exact G via fp32 matmul, no hi/lo casts)
            xcT = [cnd.tile([P, P], f32, tag=f"xcT{dc}", name=f"xcT{dc}") for dc in range(ND)]
            for dc in range(ND):
                pxc = ps.tile([P, P], f32, tag="blk")
                nc.tensor.transpose(
                    pxc[:P, :P], x_cand[:, dc * P : (dc + 1) * P], ident[:]
                )
                nc.vector.tensor_copy(xcT[dc][:], pxc[:P, :P])

            # G^T computed directly: GT[dout, cand] = sum_din A[din, dout]^T
            # x_cand^T[din, cand] — 16 fp32 N=128 matmuls, no gsb round-trip
            GTh = [cnd.tile([P, P], bf16, tag=f"GTh{dc}", name=f"GTh{dc}") for dc in range(ND)]
            GTl = [cnd.tile([P, P], bf16, tag=f"GTl{dc}", name=f"GTl{dc}") for dc in range(ND)]
            for do in range(ND):
                osl = slice(do * P, (do + 1) * P)
                pgt = ps.tile([P, P], f32, tag="blk")
                for di in range(ND):
                    nc.tensor.matmul(
                        pgt[:P, :P], Af(di, osl), xcT[di][:],
                        start=(di == 0), stop=(di == ND - 1),
                    )
                nc.vector.tensor_copy(GTh[do][:], pgt[:P, :P])
                nc.vector.tensor_tensor(
                    out=GTl[do][:], in0=pgt[:P, :P], in1=GTh[do][:],
                    op=OP.subtract,
                )

            # S_cand = G @ x^T (3-term bf16), 2 held [P,1024] PSUM pairs
            psS = []
            cmax = cnd.tile([P, 2], f32, tag="cmax")
            csum = cnd.tile([P, 2], f32, tag="csum")
            for jp in range(2):
                pss2 = ps_s.tile([P, 1024], f32, tag="psSc")
                psS.append(pss2)
                for jh in range(2):
                    jb = jp * 2 + jh
                    jsl = slice(jb * 512, (jb + 1) * 512)
                    n = 0
                    for dc in range(ND):
                        for lh, rh in (
                            (GTh[dc][:], xTh(dc, jsl)),
                            (GTl[dc][:], xTh(dc, jsl)),
                            (GTh[dc][:], xTl(dc, jsl)),
                        ):
                            nc.tensor.matmul(
                                pss2[:, jh * 512 : (jh + 1) * 512], lh, rh,
                                start=(n == 0), stop=(n == 3 * ND - 1),
                            )
                            n += 1

            # ---------------- phase 5: softmax + update -----------------
            exp_sb = expp.tile([P, L], bf16, tag="exp_sb")
            sume4 = expp.tile([P, 2], f32, tag="sume4")
            for jp in range(2):
                psl = slice(jp * 1024, (jp + 1) * 1024)
                nc.scalar.activation(
                    out=exp_sb[:, psl], in_=psS[jp][:], func=ACTF.Exp,
                    bias=0.0, scale=SCALE,
                    accum_out=sume4[:, jp : jp + 1],
                )
            sume = expp.tile([P, 1], f32, tag="sume")
            nc.vector.reduce_sum(sume[:], sume4[:], axis=AX.X)
            recip = expp.tile([P, 1], f32, tag="recip")
            nc.vector.reciprocal(recip[:], sume[:])

            # expT transposes software-pipelined with the G2 accumulation
            # (depth 4) so the PE never idles long enough to re-throttle
            expT = [expp.tile([P, P], bf16, tag=f"expT{jc}", name=f"expT{jc}") for jc in range(NL)]
            pu = ps.tile([P, D], f32, tag="blk")

            def g2_mm(jc):
                nc.tensor.matmul(
                    pu[:], expT[jc][:], xNh(jc),
                    start=(jc == 0), stop=(jc == NL - 1),
                    skip_group_check=True,
                )

            for jc in range(NL):
                pet = psb.tile([P, P], bf16, tag="blkb")
                nc.tensor.transpose(
                    pet[:P, :P], exp_sb[:, jc * P : (jc + 1) * P], ident_b[:]
                )
                if jc % 2 == 0:
                    nc.vector.tensor_copy(expT[jc][:], pet[:P, :P])
                else:
                    nc.scalar.copy(expT[jc][:], pet[:P, :P])
                if jc >= 3:
                    g2_mm(jc - 3)
            for jc in range(NL - 3, NL):
                g2_mm(jc)

            # ---- exact M + top-40 (emitted late so the expT copies get
            # DVE priority; this chain only gates the scatter indices) ----
            for jp in range(2):
                pss2 = psS[jp]
                psl = slice(jp * 1024, (jp + 1) * 1024)
                s3 = scr.tile([P, 1024], f32, tag="scrt2")
                nc.vector.tensor_tensor(
                    out=s3[:], in0=pss2[:], in1=gm[:, psl], op=OP.mult
                )
                nc.vector.reduce_max(cmax[:, jp : jp + 1], s3[:], axis=AX.X)
                s4 = scr.tile([P, 1024], f32, tag="scrt2")
                nc.vector.scalar_tensor_tensor(
                    out=s4[:], in0=pss2[:], scalar=-1.0 / L, in1=gc[:, psl],
                    op0=OP.mult, op1=OP.mult,
                    accum_out=csum[:, jp : jp + 1],
                )
            u1 = cnd.tile([P, 1], f32, tag="u1")
            u2 = cnd.tile([P, 1], f32, tag="u2")
            M_cand = cnd.tile([P, 1], f32, tag="M_cand")
            nc.vector.reduce_max(u1[:], cmax[:], axis=AX.X)
            nc.vector.reduce_sum(u2[:], csum[:], axis=AX.X)
            nc.vector.tensor_tensor(out=M_cand[:], in0=u1[:], in1=u2[:], op=OP.add)
            nc.vector.copy_predicated(M_cand[:], invalid[:], negbig[:])

            # exact top-40 threshold among candidates
            pmc = ps.tile([1, P], f32, tag="blk")
            nc.tensor.transpose(pmc[:1, :P], M_cand[:], ident[:])
            mcT = cnd.tile([1, P], f32, tag="mcT")
            nc.vector.tensor_copy(mcT[:], pmc[:1, :P])
            etop = cnd.tile([1, NT], f32, tag="etop")
            for r in range(5):
                nc.vector.max(out=etop[:, 8 * r : 8 * r + 8], in_=mcT[:])
                if r < 4:
                    nc.vector.match_replace(
                        out=mcT[:], in_to_replace=etop[:, 8 * r : 8 * r + 8],
                        in_values=mcT[:], imm_value=NEG,
                    )
            pte = ps.tile([P, 1], f32, tag="blk")
            nc.tensor.matmul(
                pte[:P, :1], ones_r1[:], etop[:, NT - 1 : NT], start=True, stop=True
            )
            tebc = cnd.tile([P, 1], f32, tag="tebc")
            nc.vector.tensor_copy(tebc[:], pte[:P, :1])
            sel2 = cnd.tile([P, 1], u8, tag="sel2")
            nc.vector.tensor_tensor(
                out=sel2[:], in0=M_cand[:], in1=tebc[:], op=OP.is_ge
            )
            scat_f = cnd.tile([P, 1], f32, tag="scat_f")
            nc.vector.tensor_copy(scat_f[:], big9[:])
            nc.vector.copy_predicated(scat_f[:], sel2[:], candq_f[:])
            scat_i = cnd.tile([P, 1], i32, tag="scat_i")
            nc.vector.tensor_copy(scat_i[:], scat_f[:])
            g2b = expp.tile([P, D], bf16, tag="g2b")
            nc.scalar.copy(g2b[:], pu[:])
            G2T = [expp.tile([P, P], bf16, tag=f"G2T{dc}", name=f"G2T{dc}") for dc in range(ND)]
            for dc in range(ND):
                pg2 = psb.tile([P, P], bf16, tag="blkb")
                nc.tensor.transpose(
                    pg2[:P, :P], g2b[:, dc * P : (dc + 1) * P], ident_b[:]
                )
                nc.vector.tensor_copy(G2T[dc][:], pg2[:P, :P])
            # upd = G2 @ Wv^T / sums
            pup = ps.tile([P, D], f32, tag="blk")
            for dc in range(ND):
                nc.tensor.matmul(
                    pup[:], G2T[dc][:], wvh(dc, SD),
                    start=(dc == 0), stop=(dc == ND - 1),
                )
            upd = expp.tile([P, D], f32, tag="upd")
            nc.scalar.activation(
                out=upd[:], in_=pup[:], func=ACTF.Copy, bias=0.0, scale=recip[:]
            )
            nc.gpsimd.indirect_dma_start(
                out=ctx_d[:],
                out_offset=bass.IndirectOffsetOnAxis(ap=scat_i[:, :1], axis=0),
                in_=upd[:], in_offset=None,
                bounds_check=L - 1, oob_is_err=False,
            )

    nc.compile()
    return nc


_NC = None


def _get_nc():
    global _NC
    if _NC is None:
        _NC = build()
    return _NC


def _split_bf16(a):
    hi = a.astype(ml_dtypes.bfloat16)
    lo = (a - hi.astype(np.float32)).astype(ml_dtypes.bfloat16)
    return hi, lo


def _host_prep(x, Wq, Wk, Wv, index_sample):
    x = np.asarray(x, dtype=np.float32)
    Wq = np.asarray(Wq, dtype=np.float32)
    Wk = np.asarray(Wk, dtype=np.float32)
    Wv = np.asarray(Wv, dtype=np.float32)
    idx = np.asarray(index_sample)

    def pack(m):
        # [ND*P, W] -> [P, ND*W]: row dc*128+p lands at columns dc*W..+W
        nd = m.shape[0] // P
        return np.ascontiguousarray(
            m.reshape(nd, P, m.shape[1]).transpose(1, 0, 2).reshape(P, -1)
        )

    def pack_jb(m):
        # [ND*P, NJ*512] -> [P, NJ*ND*512] (jb-major blocks)
        nd = m.shape[0] // P
        nj = m.shape[1] // 512
        return np.ascontiguousarray(
            m.reshape(nd, P, nj, 512).transpose(1, 2, 0, 3).reshape(P, -1)
        )

    A = (Wq.T.astype(np.float64) @ Wk.astype(np.float64)).astype(np.float32)
    Ah = A.astype(ml_dtypes.bfloat16)
    wvh, wvl = _split_bf16(np.ascontiguousarray(Wv.T))

    rows = np.arange(L)[:, None]
    maskb = np.zeros((L, L), dtype=ml_dtypes.bfloat16)
    maskb[rows, idx] = 1
    countf = np.zeros((L, L), dtype=np.uint8)
    np.add.at(countf, (rows, idx), 1)

    perm16 = np.zeros((16, 8 * P), dtype=np.float16)
    for f in range(8):
        for p in range(16):
            perm16[p, f * P + p + 16 * f] = 1.0
    qidxf = (np.arange(P)[:, None] + 128 * np.arange(NL)[None, :]).astype(np.float32)
    crow = (1.2 + np.arange(NLAD, dtype=np.float32) * 0.134).reshape(1, NLAD)

    shared = {
        "Ahp": pack(Ah), "Afp": pack(A),
        "wvThp": pack(wvh), "wvTlp": pack(wvl),
        "maskb": maskb, "countf": countf, "perm16": perm16,
        "qidxf": qidxf, "crow": crow,
    }
    in_maps = []
    for b in range(B):
        xb = np.ascontiguousarray(x[b])
        xT = np.ascontiguousarray(xb.T)
        xth, xtl = _split_bf16(xT)
        xnh = xb.astype(ml_dtypes.bfloat16)
        xmean = xb.astype(np.float64).mean(axis=0).astype(np.float32)
        xmeh, xmel = _split_bf16(xmean.reshape(1, D))
        xm = np.concatenate(
            [xmeh.reshape(ND, P).T, xmel.reshape(ND, P).T], axis=1
        ).astype(ml_dtypes.bfloat16)
        in_maps.append(
            {
                "x_nat": xb,
                "xThp": pack_jb(xth),
                "xTlp": pack_jb(xtl),
                "xNhp": pack(xnh),
                "xmp": np.ascontiguousarray(xm),
                **shared,
            }
        )
    return in_maps


def kernel(x, Wq, Wk, Wv, index_sample, _trace=False, _result_box=None):
    in_maps = _host_prep(x, Wq, Wk, Wv, index_sample)
    nc = _get_nc()
    res = run_bass_kernel_spmd(nc, in_maps, core_ids=list(range(B)), trace=_trace)
    if _result_box is not None:
        _result_box.append(res)
    out = np.stack([np.asarray(res.results[b]["ctx"]) for b in range(B)], axis=0)
    return out


# revision 68
# speedup vs baseline: 2.0786x; 1.0329x over previous
"""Sparse attention (ProbSparse-style) Trainium2 Bass kernel, v2.

Problem (per batch element b, data-parallel over 8 NeuronCores):
  Q = x @ Wq.T ; K = x @ Wk.T ; V = x @ Wv.T            [L=2048, D=512]
  QK_sample[l,s] = Q[l] . K[index_sample[l,s]]           [L, 40]
  M[l] = max_s QK_sample - sum_s QK_sample / L
  sel = top40(M)  (as a set; the reference scatter makes order irrelevant)
  scores = Q[sel] @ K.T / sqrt(D); attn = softmax(scores)
  ctx = broadcast(mean(V)); ctx[sel] = attn @ V

Key ideas vs v1 baseline:
  - A = Wq^T @ Wk precomputed on host: S = (x A) x^T. Kills the K and Q
    projections entirely; both approx and exact scores contract against
    the resident x^T tiles.
  - Approx M = masked max of bf16 S only (the sum/L term is <= ~0.5 and
    is absorbed by the candidate margin; validated: true top-40 rows sit
    within rank <= 40 of the approx ordering).
  - Threshold via a 64-step mu + c*sigma ladder with on-device counts
    (one 3d-broadcast compare + reduce + PE column-sum), picking the
    largest T with count >= 88 (fallback: smallest T with count <= 127).
    Replaces the 62us GPSIMD kth_largest.
  - Exact stage on <= 128 candidates: G = x_cand A (3-term bf16),
    S_cand = G x^T (3-term bf16)  ->  ~1e-4-class absolute error,
    validated 26x under the seed-0 top-40 boundary gap.
  - Softmax without max subtraction (|S*scale| <= ~9, exp is safe),
    upd = (attn @ x) @ Wv^T (kills the V projection; V never built).

kernel(**inputs) accepts FULL inputs, returns FULL [8, 2048, 512] f32;
batch is sharded over 8 cores.
"""

import math

import numpy as np
import ml_dtypes

import concourse.bacc as bacc
import concourse.bass as bass
import concourse.mybir as mybir
import concourse.tile as tile
from concourse.bass_utils import run_bass_kernel_spmd
from concourse.masks import make_identity
from concourse import library_config

P = 128
L = 2048
D = 512
B = 8
NL = L // P        # 16 query chunks
ND = D // P        # 4 feature chunks
NJ = L // 512      # 4 key blocks of 512
NT = 40
NLAD = 32          # threshold ladder steps
SCALE = 1.0 / math.sqrt(D)
NEG = -3.0e38
BIG = 3.0e38
SKIP_IDX = 99999.0  # scatter index sentinel (> bounds_check -> row skipped)

f32 = mybir.dt.float32
f32r = mybir.dt.float32r
f16 = mybir.dt.float16
bf16 = mybir.dt.bfloat16
u8 = mybir.dt.uint8
i32 = mybir.dt.int32
u32 = mybir.dt.uint32
AX = mybir.AxisListType
OP = mybir.AluOpType
ACTF = mybir.ActivationFunctionType


def build():
    nc = bacc.Bacc("TRN2", target_bir_lowering=False)

    # All big operands are host-packed into [128, wide] layouts so each
    # DMA partition line is a 4-16KB contiguous DRAM run (1KB lines were
    # descriptor-bound: ~26us of startup).
    x_d = nc.dram_tensor("x_nat", [L, D], f32, kind="ExternalInput")
    xth_d = nc.dram_tensor("xThp", [P, ND * L], bf16, kind="ExternalInput")
    xtf_d = nc.dram_tensor("xTfp", [P, ND * L], f32r, kind="ExternalInput")
    xnh_d = nc.dram_tensor("xNhp", [P, NL * D], bf16, kind="ExternalInput")
    ah_d = nc.dram_tensor("Ahp", [P, ND * D], bf16, kind="ExternalInput")
    af_d = nc.dram_tensor("Afp", [P, ND * D], f32, kind="ExternalInput")
    wvh_d = nc.dram_tensor("wvThp", [P, ND * D], bf16, kind="ExternalInput")
    wvl_d = nc.dram_tensor("wvTlp", [P, ND * D], bf16, kind="ExternalInput")
    xm_d = nc.dram_tensor("xmp", [P, 2 * ND], bf16, kind="ExternalInput")
    maskb_d = nc.dram_tensor("maskb", [L, L], bf16, kind="ExternalInput")
    mcg_d = nc.dram_tensor("mcg", [L, 2 * L], u8, kind="ExternalInput")
    perm_d = nc.dram_tensor("perm16", [16, 8 * P], f16, kind="ExternalInput")
    qidx_d = nc.dram_tensor("qidxf", [P, NL], f32, kind="ExternalInput")
    crow_d = nc.dram_tensor("crow", [1, NLAD], f32, kind="ExternalInput")
    ctx_d = nc.dram_tensor("ctx", [L, D], f32, kind="ExternalOutput")

    with tile.TileContext(nc) as tc:
        with (
            tc.tile_pool(name="const", bufs=1) as cst,
            tc.tile_pool(name="xres", bufs=1) as xres,      # resident x / A / Wv
            tc.tile_pool(name="proj", bufs=1) as proj,      # QATb
            tc.tile_pool(name="mstuff", bufs=1) as mst,     # M / threshold smalls
            tc.tile_pool(name="mstream", bufs=3) as mstr,   # mask chunks
            tc.tile_pool(name="scr", bufs=3) as scr,        # TTR scratch
            tc.tile_pool(name="cand", bufs=1) as cnd,       # exact-stage tiles
            tc.tile_pool(name="expp", bufs=1) as expp,      # softmax/upd tiles
            tc.tile_pool(name="ps", bufs=2, space="PSUM") as ps,
            tc.tile_pool(name="psb", bufs=2, space="PSUM") as psb,    # bf16 transposes
            tc.tile_pool(name="ps_s", bufs=2, space="PSUM") as ps_s,  # S pairs
        ):
            # ---------------- constants ----------------
            ident = cst.tile([P, P], f32, tag="ident")
            make_identity(nc, ident[:])
            ident_b = cst.tile([P, P], bf16, tag="ident_b")
            nc.vector.tensor_copy(ident_b[:], ident[:])
            # preload the sparse_gather ucode so the serial tail does not
            # pay the library switch
            nc.gpsimd.load_library(library_config.sparse_gather)
            ones_r1 = cst.tile([1, P], f32, tag="ones_r1")
            nc.vector.memset(ones_r1[:], 1.0)
            ones_cf = cst.tile([P, 1], f32, tag="ones_cf")
            nc.vector.memset(ones_cf[:], 1.0)
            negbig = cst.tile([P, 1], f32, tag="negbig")
            nc.vector.memset(negbig[:], NEG)
            big9 = cst.tile([P, 1], f32, tag="big9")
            nc.vector.memset(big9[:], SKIP_IDX)
            qidx_f = cst.tile([P, NL], f32, tag="qidx_f")
            nc.sync.dma_start(qidx_f[:], qidx_d[:])
            crow = cst.tile([1, NLAD], f32, tag="crow")
            nc.sync.dma_start(crow[:], crow_d[:])
            perm16 = cst.tile([16, 8 * P], f16, tag="perm16")
            nc.sync.dma_start(perm16[:], perm_d[:])

            # ---------------- critical loads (packed, big lines) --------
            # Only Ahp + xThp gate the first matmuls; everything the tail
            # needs is DMA'd from inside the phase-2 loop so it doesn't
            # compete for startup bandwidth.
            Ahp = xres.tile([P, ND * D], bf16, tag="Ahp")
            nc.sync.dma_start(Ahp[:], ah_d[:])
            # x^T hi is packed jb-major: block jb holds [dc=0..3][512 cols]
            # so the first QA matmuls start after Ahp + one 0.5MB block
            xThp = xres.tile([P, ND * L], bf16, tag="xThp")
            for jb in range(NJ):
                nc.sync.dma_start(
                    xThp[:, jb * 2048 : (jb + 1) * 2048],
                    xth_d[:, jb * 2048 : (jb + 1) * 2048],
                )
            Afp = xres.tile([P, ND * D], f32, tag="Afp")
            wvhp = xres.tile([P, ND * D], bf16, tag="wvhp")
            wvlp = xres.tile([P, ND * D], bf16, tag="wvlp")
            xmp = xres.tile([P, 2 * ND], bf16, tag="xmp")
            xTfp = xres.tile([P, ND * L], f32r, tag="xTfp")
            xNhp = xres.tile([P, NL * D], bf16, tag="xNhp")

            # slice helpers over the packed tiles
            Ah = lambda dc, js: Ahp[:, dc * D + js.start : dc * D + js.stop]
            Af = lambda dc, js: Afp[:, dc * D + js.start : dc * D + js.stop]
            wvh = lambda dc, js: wvhp[:, dc * D + js.start : dc * D + js.stop]
            wvl = lambda dc, js: wvlp[:, dc * D + js.start : dc * D + js.stop]

            def _xt(tile_, dc, js):
                # jb-major packing: block jb*2048 + dc*512
                jb, r = divmod(js.start, 512)
                assert js.stop - js.start == 512 and r == 0
                off = jb * 2048 + dc * 512
                return tile_[:, off : off + 512]

            xTh = lambda dc, js: _xt(xThp, dc, js)
            xTf = lambda dc, js: _xt(xTfp, dc, js)
            xNh = lambda jc: xNhp[:, jc * D : (jc + 1) * D]
            SD = slice(0, D)
            SL = slice(0, L)

            # ---------------- phase 1: QA^T = A^T x^T (bf16) ------------
            QATb = [proj.tile([P, L], bf16, tag=f"QATb{ic}", name=f"QATb{ic}") for ic in range(ND)]
            for jb in range(NJ):
                jsl = slice(jb * 512, (jb + 1) * 512)
                for ic in range(ND):
                    isl = slice(ic * P, (ic + 1) * P)
                    pq = ps.tile([P, 512], f32, tag="blk")
                    for dc in range(ND):
                        nc.tensor.matmul(
                            pq[:], Ah(dc, isl), xTh(dc, jsl),
                            start=(dc == 0), stop=(dc == ND - 1),
                        )
                    nc.scalar.copy(QATb[ic][:, jsl], pq[:])

            # ---------------- phase 2: approx S + masked max ------------
            # Per 128-query chunk: PE computes 4 S blocks; ScalarE evicts
            # them to a bf16 row [P, 2048]; DVE does one 2x bf16 mask-mult
            # + one wide reduce_max.  (tensor_tensor_reduce crashes TRN2
            # hardware, so the fused form is not available.)
            M_all = mst.tile([P, NL], f32, tag="M_all")
            M_lo = mst.tile([P, 8], f32, tag="M_lo")
            Trow = mst.tile([1, NLAD], f32, tag="Trow")
            Tb = mst.tile([P, NLAD], bf16, tag="Tb")
            for lc in range(NL):
                lsl = slice(lc * P, (lc + 1) * P)
                mkb = mstr.tile([P, L], bf16, tag="mkb")
                nc.sync.dma_start(mkb[:], maskb_d[lsl, :])
                # tail-only loads trickled in behind the mask stream
                if lc == 0:
                    nc.sync.dma_start(wvhp[:], wvh_d[:])
                    nc.sync.dma_start(wvlp[:], wvl_d[:])
                    nc.sync.dma_start(xmp[:], xm_d[:])
                elif lc == 2:
                    nc.sync.dma_start(Afp[:], af_d[:])
                elif lc == 5:
                    nc.sync.dma_start(xTfp[:], xtf_d[:])
                elif lc == 9:
                    nc.sync.dma_start(xNhp[:], xnh_d[:])
                sb1 = scr.tile([P, L], bf16, tag="sb1")
                for jp in range(2):
                    # paired PSUM banks -> one wide eviction per 1024 cols
                    pss = ps_s.tile([P, 1024], f32, tag="psSc", name="pssa")
                    for jh in range(2):
                        jb = jp * 2 + jh
                        jsl = slice(jb * 512, (jb + 1) * 512)
                        for ic in range(ND):
                            nc.tensor.matmul(
                                pss[:, jh * 512 : (jh + 1) * 512],
                                QATb[ic][:, lsl], xTh(ic, jsl),
                                start=(ic == 0), stop=(ic == ND - 1),
                            )
                    nc.scalar.copy(
                        sb1[:, jp * 1024 : (jp + 1) * 1024], pss[:]
                    )
                # masked max: two 2x-mode masked products, one 2x max
                # combine, then a half-width 1x reduce
                t0 = scr.tile([P, 1024], bf16, tag="t0m")
                nc.vector.tensor_tensor(
                    out=t0[:], in0=sb1[:, 0:1024], in1=mkb[:, 0:1024],
                    op=OP.mult,
                )
                t1 = scr.tile([P, 1024], bf16, tag="t1m")
                nc.vector.tensor_tensor(
                    out=t1[:], in0=sb1[:, 1024:2048], in1=mkb[:, 1024:2048],
                    op=OP.mult,
                )
                t2 = scr.tile([P, 1024], bf16, tag="t2m")
                nc.vector.tensor_tensor(
                    out=t2[:], in0=t0[:], in1=t1[:], op=OP.max
                )
                if lc < 8:
                    nc.vector.reduce_max(M_lo[:, lc : lc + 1], t2[:], axis=AX.X)
                    nc.vector.tensor_copy(
                        M_all[:, lc : lc + 1], M_lo[:, lc : lc + 1]
                    )
                else:
                    nc.vector.reduce_max(M_all[:, lc : lc + 1], t2[:], axis=AX.X)
                if lc == 7:
                    # ---- early threshold stats on the first 1024 rows --
                    # (mu/sigma only steer the ladder range; the counts
                    # below verify against the full M) -- this whole chain
                    # runs under the second half of the main phase.
                    stats2 = mst.tile([P, 2], f32, tag="stats2")
                    msq = mst.tile([P, 8], f32, tag="msq")
                    nc.vector.scalar_tensor_tensor(
                        out=msq[:], in0=M_lo[:], scalar=1.0, in1=M_lo[:],
                        op0=OP.mult, op1=OP.mult,
                        accum_out=stats2[:, 1:2],
                    )
                    nc.vector.tensor_reduce(
                        stats2[:, 0:1], M_lo[:], axis=AX.X, op=OP.add
                    )
                    pst = ps.tile([1, 2], f32, tag="blk")
                    nc.tensor.matmul(
                        pst[:1, :2], ones_cf[:], stats2[:], start=True, stop=True
                    )
                    srow = mst.tile([1, 2], f32, tag="srow")
                    nc.vector.tensor_copy(srow[:], pst[:1, :2])
                    musig = mst.tile([1, 2], f32, tag="musig")
                    nc.vector.tensor_scalar_mul(musig[:], srow[:], 1.0 / 1024.0)
                    mu = musig[:, 0:1]
                    mu2 = mst.tile([1, 1], f32, tag="mu2")
                    nc.vector.tensor_tensor(out=mu2[:], in0=mu, in1=mu, op=OP.mult)
                    var = mst.tile([1, 1], f32, tag="var")
                    nc.vector.tensor_tensor(
                        out=var[:], in0=musig[:, 1:2], in1=mu2[:], op=OP.subtract
                    )
                    sigma = mst.tile([1, 1], f32, tag="sigma")
                    nc.scalar.sqrt(sigma[:], var[:])
                    # dummy exp: pull the Exp act-table load off the tail's
                    # critical path (table switch costs ~1.3us)
                    expd = mst.tile([1, 1], f32, tag="expd")
                    nc.scalar.activation(
                        out=expd[:], in_=var[:], func=ACTF.Exp,
                        bias=0.0, scale=0.0,
                    )
                    nc.vector.tensor_tensor(
                        out=Trow[:], in0=crow[:],
                        in1=sigma[:].to_broadcast([1, NLAD]), op=OP.mult,
                    )
                    nc.vector.tensor_tensor(
                        out=Trow[:], in0=Trow[:], in1=mu.to_broadcast([1, NLAD]),
                        op=OP.add,
                    )
                    ptb = ps.tile([P, NLAD], f32, tag="blk")
                    nc.tensor.matmul(
                        ptb[:P, :NLAD], ones_r1[:], Trow[:], start=True, stop=True
                    )
                    nc.vector.tensor_copy(Tb[:], ptb[:P, :NLAD])

            # ---------------- phase 3: ladder counts --------------------
            M_b = mst.tile([P, NL], bf16, tag="M_b")
            nc.vector.tensor_copy(M_b[:], M_all[:])
            # cmp[p, j, f] = M[p, f] >= T[j]  (bf16 in/out -> 2x DVE, and
            # PE can column-sum the bf16 result)
            cmpb = mst.tile([P, NLAD * NL], bf16, tag="cmpb")
            nc.vector.tensor_tensor(
                out=cmpb[:].rearrange("p (j f) -> p j f", f=NL),
                in0=M_b[:].rearrange("p (o f) -> p o f", o=1).to_broadcast([P, NLAD, NL]),
                in1=Tb[:].rearrange("p (j o) -> p j o", o=1).to_broadcast([P, NLAD, NL]),
                op=OP.is_ge,
            )
            # ---------------- Vmean -> ctx init (PE idle slot) ----------
            pvm = ps.tile([1, D], f32, tag="blk")
            n = 0
            for dc in range(ND):
                for lh, rh in (
                    (xmp[:, dc : dc + 1], wvh(dc, SD)),
                    (xmp[:, ND + dc : ND + dc + 1], wvh(dc, SD)),
                    (xmp[:, dc : dc + 1], wvl(dc, SD)),
                ):
                    nc.tensor.matmul(
                        pvm[:1, :], lh, rh,
                        start=(n == 0), stop=(n == 3 * ND - 1),
                    )
                    n += 1
            vmean = mst.tile([1, D], f32, tag="vmean")
            nc.scalar.copy(vmean[:], pvm[:1, :])
            pvb = ps.tile([P, D], f32, tag="blk")
            nc.tensor.matmul(pvb[:], ones_r1[:], vmean[:], start=True, stop=True)
            vmean_bc = mst.tile([P, D], f32, tag="vmean_bc")
            nc.vector.tensor_copy(vmean_bc[:], pvb[:])
            for jc in range(NL):
                nc.sync.dma_start(ctx_d[jc * P : (jc + 1) * P, :], vmean_bc[:])

            cnt01 = mst.tile([P, NLAD], f32, tag="cnt01")
            nc.vector.tensor_reduce(
                cnt01[:], cmpb[:].rearrange("p (j f) -> p j f", f=NL),
                axis=AX.X, op=OP.add,
            )
            pcc = ps.tile([1, NLAD], f32, tag="blk")
            nc.tensor.matmul(pcc[:1, :NLAD], ones_cf[:], cnt01[:], start=True, stop=True)
            cntrow = mst.tile([1, NLAD], f32, tag="cntrow")
            nc.vector.tensor_copy(cntrow[:], pcc[:1, :NLAD])
            # largest T with count >= 88; fallback smallest T with count <= 127
            okm = mst.tile([1, NLAD], u8, tag="okm")
            nc.vector.tensor_scalar(
                okm[:], cntrow[:], 87.5, None, op0=OP.is_ge
            )
            negrow = mst.tile([1, NLAD], f32, tag="negrow")
            nc.vector.memset(negrow[:], NEG)
            bigrow = mst.tile([1, NLAD], f32, tag="bigrow")
            nc.vector.memset(bigrow[:], BIG)
            tsel = mst.tile([1, NLAD], f32, tag="tsel")
            nc.vector.select(tsel[:], okm[:], Trow[:], negrow[:])
            tstar = mst.tile([1, 1], f32, tag="tstar")
            nc.vector.reduce_max(tstar[:], tsel[:], axis=AX.X)
            ok2 = mst.tile([1, NLAD], u8, tag="ok2")
            nc.vector.tensor_scalar(
                ok2[:], cntrow[:], 127.5, None, op0=OP.is_le
            )
            tsel2 = mst.tile([1, NLAD], f32, tag="tsel2")
            nc.vector.select(tsel2[:], ok2[:], Trow[:], bigrow[:])
            tfb = mst.tile([1, 1], f32, tag="tfb")
            nc.vector.tensor_reduce(tfb[:], tsel2[:], axis=AX.X, op=OP.min)
            have = mst.tile([1, 1], u8, tag="have")
            nc.vector.tensor_scalar(
                have[:], tstar[:], -1.0e30, None, op0=OP.is_ge
            )
            tfin = mst.tile([1, 1], f32, tag="tfin")
            nc.vector.select(tfin[:], have[:], tstar[:], tfb[:])
            ptf = ps.tile([P, 1], f32, tag="blk")
            nc.tensor.matmul(ptf[:P, :1], ones_r1[:], tfin[:], start=True, stop=True)
            tbc = mst.tile([P, 1], f32, tag="tbc")
            nc.vector.tensor_copy(tbc[:], ptf[:P, :1])

            # selmask / candidate index compaction
            selmask = mst.tile([P, NL], u8, tag="selmask")
            nc.vector.tensor_scalar(
                selmask[:], M_all[:], tbc[:], 0.0,
                op0=OP.subtract, op1=OP.is_ge,
            )
            midx = mst.tile([P, NL], f32, tag="midx")
            nc.vector.memset(midx[:], -1.0)
            nc.vector.copy_predicated(midx[:], selmask[:], qidx_f[:])
            pwr = ps.tile([16, P], f32, tag="blk", name="pwr")
            nc.tensor.transpose(pwr[:16, :P], midx[:], ident[:])
            wrap_in = mst.tile([16, P], f32, tag="wrap_in")
            nc.vector.tensor_copy(wrap_in[:], pwr[:16, :P])
            spg = mst.tile([16, 8], f32, tag="spg")
            nfound = mst.tile([1, 1], u32, tag="nfound")
            nc.gpsimd.sparse_gather(out=spg[:], in_=wrap_in[:], num_found=nfound[:])
            spg_cl = mst.tile([16, 8], f32, tag="spg_cl")
            nc.vector.tensor_scalar(
                spg_cl[:], spg[:], 0.0, float(L - 1), op0=OP.max, op1=OP.min
            )
            # fp16 keeps indices <= 2047 exact and avoids the fp32 double
            # LDWEIGHTS cost of the one-hot unwrap
            spg_h = mst.tile([16, 8], f16, tag="spg_h")
            nc.vector.tensor_copy(spg_h[:], spg_cl[:])
            pcq = ps.tile([P, 1], f32, tag="blk", name="pcq")
            for f in range(8):
                nc.tensor.matmul(
                    pcq[:P, :1], perm16[:, f * P : (f + 1) * P],
                    spg_h[:, f : f + 1],
                    start=(f == 0), stop=(f == 7),
                )
            candq_f = mst.tile([P, 1], f32, tag="candq_f")
            nc.vector.tensor_copy(candq_f[:], pcq[:P, :1])
            candq_i = mst.tile([P, 1], i32, tag="candq_i")
            nc.vector.tensor_copy(candq_i[:], pcq[:P, :1])
            nf_f = mst.tile([1, 1], f32, tag="nf_f")
            nc.vector.tensor_copy(nf_f[:], nfound[:])
            pnb = ps.tile([P, 1], f32, tag="blk")
            nc.tensor.matmul(pnb[:P, :1], ones_r1[:], nf_f[:], start=True, stop=True)
            nbc = mst.tile([P, 1], f32, tag="nbc")
            nc.vector.tensor_copy(nbc[:], pnb[:P, :1])
            invalid = mst.tile([P, 1], u8, tag="invalid")
            nc.vector.tensor_tensor(
                out=invalid[:], in0=qidx_f[:, 0:1], in1=nbc[:], op=OP.is_ge
            )

            # Keep-warm: ~3.5us of throwaway matmuls gated on candq_h so
            # they run exactly during the gather window; a >3.4us PE idle
            # here would drop the HAM clock to 1.2GHz for the whole exact
            # stage.
            candq_h = mst.tile([P, 1], bf16, tag="candq_h")
            nc.vector.tensor_copy(candq_h[:], pcq[:P, :1])
            pwarm = ps.tile([1, 512], f32, tag="blk", name="pwarm")
            for w in range(14):
                nc.tensor.matmul(
                    pwarm[:1, :512], candq_h[:, :1], xThp[:, 0:512],
                    start=True, stop=True,
                )

            # ---------------- phase 4: exact stage ----------------------
            x_cand = cnd.tile([P, D], f32, tag="x_cand")
            nc.gpsimd.indirect_dma_start(
                out=x_cand[:], out_offset=None, in_=x_d[:],
                in_offset=bass.IndirectOffsetOnAxis(ap=candq_i[:, :1], axis=0),
            )
            # combined mask++count row gather (one SWDGE pass)
            gmc = cnd.tile([P, 2 * L], u8, tag="gmc")
            nc.gpsimd.indirect_dma_start(
                out=gmc[:], out_offset=None, in_=mcg_d[:],
                in_offset=bass.IndirectOffsetOnAxis(ap=candq_i[:, :1], axis=0),
            )

            # x_cand^T (fp32 — exact G via fp32 matmul, no hi/lo casts)
            xcT = [cnd.tile([P, P], f32, tag=f"xcT{dc}", name=f"xcT{dc}") for dc in range(ND)]
            for dc in range(ND):
                pxc = ps.tile([P, P], f32, tag="blk")
                nc.tensor.transpose(
                    pxc[:P, :P], x_cand[:, dc * P : (dc + 1) * P], ident[:]
                )
                nc.vector.tensor_copy(xcT[dc][:], pxc[:P, :P])

            # G^T computed directly: GT[dout, cand] = sum_din A[din, dout]^T
            # x_cand^T[din, cand] — 16 fp32 N=128 matmuls, no gsb round-trip
            GT = [cnd.tile([P, P], f32r, tag=f"GT{dc}", name=f"GT{dc}") for dc in range(ND)]
            for do in range(ND):
                osl = slice(do * P, (do + 1) * P)
                pgt = ps.tile([P, P], f32, tag="blk")
                for di in range(ND):
                    nc.tensor.matmul(
                        pgt[:P, :P], Af(di, osl), xcT[di][:],
                        start=(di == 0), stop=(di == ND - 1),
                    )
                nc.vector.tensor_copy(GT[do][:], pgt[:P, :P])

            # S_cand = G @ x^T in fp32r (full-rate fp32-class matmul),
            # 2 held [P,1024] PSUM pairs
            psS = []
            cmax = cnd.tile([P, 2], f32, tag="cmax")
            csum = cnd.tile([P, 2], f32, tag="csum")
            for jp in range(2):
                pss2 = ps_s.tile([P, 1024], f32, tag="psSc")
                psS.append(pss2)
                for jh in range(2):
                    jb = jp * 2 + jh
                    jsl = slice(jb * 512, (jb + 1) * 512)
                    for dc in range(ND):
                        nc.tensor.matmul(
                            pss2[:, jh * 512 : (jh + 1) * 512],
                            GT[dc][:], xTf(dc, jsl),
                            start=(dc == 0), stop=(dc == ND - 1),
                        )

            # ---------------- phase 5: softmax + update -----------------
            exp_sb = expp.tile([P, L], bf16, tag="exp_sb")
            sume4 = expp.tile([P, 2], f32, tag="sume4")
            for jp in range(2):
                psl = slice(jp * 1024, (jp + 1) * 1024)
                nc.scalar.activation(
                    out=exp_sb[:, psl], in_=psS[jp][:], func=ACTF.Exp,
                    bias=0.0, scale=SCALE,
                    accum_out=sume4[:, jp : jp + 1],
                )
            sume = expp.tile([P, 1], f32, tag="sume")
            nc.vector.reduce_sum(sume[:], sume4[:], axis=AX.X)
            recip = expp.tile([P, 1], f32, tag="recip")
            nc.vector.reciprocal(recip[:], sume[:])

            # expT transposes software-pipelined with the G2 accumulation
            # (depth 4) so the PE never idles long enough to re-throttle
            expT = [expp.tile([P, P], bf16, tag=f"expT{jc}", name=f"expT{jc}") for jc in range(NL)]
            pu = ps.tile([P, D], f32, tag="blk")

            def g2_mm(jc):
                nc.tensor.matmul(
                    pu[:], expT[jc][:], xNh(jc),
                    start=(jc == 0), stop=(jc == NL - 1),
                    skip_group_check=True,
                )

            for jc in range(NL):
                pet = psb.tile([P, P], bf16, tag="blkb")
                nc.tensor.transpose(
                    pet[:P, :P], exp_sb[:, jc * P : (jc + 1) * P], ident_b[:]
                )
                if jc % 2 == 0:
                    nc.vector.tensor_copy(expT[jc][:], pet[:P, :P])
                else:
                    nc.scalar.copy(expT[jc][:], pet[:P, :P])
                if jc >= 3:
                    g2_mm(jc - 3)
            for jc in range(NL - 3, NL):
                g2_mm(jc)

            # ---- exact M + top-40 (emitted late so the expT copies get
            # DVE priority; this chain only gates the scatter indices) ----
            for jp in range(2):
                pss2 = psS[jp]
                psl = slice(jp * 1024, (jp + 1) * 1024)
                s3 = scr.tile([P, 1024], f32, tag="scrt2")
                nc.vector.tensor_tensor(
                    out=s3[:], in0=pss2[:], in1=gmc[:, psl], op=OP.mult
                )
                nc.vector.reduce_max(cmax[:, jp : jp + 1], s3[:], axis=AX.X)
                s4 = scr.tile([P, 1024], f32, tag="scrt2")
                nc.vector.scalar_tensor_tensor(
                    out=s4[:], in0=pss2[:], scalar=-1.0 / L,
                    in1=gmc[:, L + psl.start : L + psl.stop],
                    op0=OP.mult, op1=OP.mult,
                    accum_out=csum[:, jp : jp + 1],
                )
            u1 = cnd.tile([P, 1], f32, tag="u1")
            u2 = cnd.tile([P, 1], f32, tag="u2")
            M_cand = cnd.tile([P, 1], f32, tag="M_cand")
            nc.vector.reduce_max(u1[:], cmax[:], axis=AX.X)
            nc.vector.reduce_sum(u2[:], csum[:], axis=AX.X)
            nc.vector.tensor_tensor(out=M_cand[:], in0=u1[:], in1=u2[:], op=OP.add)
            nc.vector.copy_predicated(M_cand[:], invalid[:], negbig[:])

            # exact top-40 threshold among candidates
            pmc = ps.tile([1, P], f32, tag="blk")
            nc.tensor.transpose(pmc[:1, :P], M_cand[:], ident[:])
            mcT = cnd.tile([1, P], f32, tag="mcT")
            nc.vector.tensor_copy(mcT[:], pmc[:1, :P])
            etop = cnd.tile([1, NT], f32, tag="etop")
            for r in range(5):
                nc.vector.max(out=etop[:, 8 * r : 8 * r + 8], in_=mcT[:])
                if r < 4:
                    nc.vector.match_replace(
                        out=mcT[:], in_to_replace=etop[:, 8 * r : 8 * r + 8],
                        in_values=mcT[:], imm_value=NEG,
                    )
            pte = ps.tile([P, 1], f32, tag="blk")
            nc.tensor.matmul(
                pte[:P, :1], ones_r1[:], etop[:, NT - 1 : NT], start=True, stop=True
            )
            tebc = cnd.tile([P, 1], f32, tag="tebc")
            nc.vector.tensor_copy(tebc[:], pte[:P, :1])
            sel2 = cnd.tile([P, 1], u8, tag="sel2")
            nc.vector.tensor_tensor(
                out=sel2[:], in0=M_cand[:], in1=tebc[:], op=OP.is_ge
            )
            scat_f = cnd.tile([P, 1], f32, tag="scat_f")
            nc.vector.tensor_copy(scat_f[:], big9[:])
            nc.vector.copy_predicated(scat_f[:], sel2[:], candq_f[:])
            scat_i = cnd.tile([P, 1], i32, tag="scat_i")
            nc.vector.tensor_copy(scat_i[:], scat_f[:])
            g2b = expp.tile([P, D], bf16, tag="g2b")
            nc.scalar.copy(g2b[:], pu[:])
            G2T = [expp.tile([P, P], bf16, tag=f"G2T{dc}", name=f"G2T{dc}") for dc in range(ND)]
            for dc in range(ND):
                pg2 = psb.tile([P, P], bf16, tag="blkb")
                nc.tensor.transpose(
                    pg2[:P, :P], g2b[:, dc * P : (dc + 1) * P], ident_b[:]
                )
                nc.vector.tensor_copy(G2T[dc][:], pg2[:P, :P])
            # upd = G2 @ Wv^T / sums
            pup = ps.tile([P, D], f32, tag="blk")
            for dc in range(ND):
                nc.tensor.matmul(
                    pup[:], G2T[dc][:], wvh(dc, SD),
                    start=(dc == 0), stop=(dc == ND - 1),
                )
            upd = expp.tile([P, D], f32, tag="upd")
            nc.scalar.activation(
                out=upd[:], in_=pup[:], func=ACTF.Copy, bias=0.0, scale=recip[:]
            )
            nc.gpsimd.indirect_dma_start(
                out=ctx_d[:],
                out_offset=bass.IndirectOffsetOnAxis(ap=scat_i[:, :1], axis=0),
                in_=upd[:], in_offset=None,
                bounds_check=L - 1, oob_is_err=False,
            )

    nc.compile()
    return nc


_NC = None


def _get_nc():
    global _NC
    if _NC is None:
        _NC = build()
    return _NC


def _split_bf16(a):
    hi = a.astype(ml_dtypes.bfloat16)
    lo = (a - hi.astype(np.float32)).astype(ml_dtypes.bfloat16)
    return hi, lo


def _host_prep(x, Wq, Wk, Wv, index_sample):
    x = np.asarray(x, dtype=np.float32)
    Wq = np.asarray(Wq, dtype=np.float32)
    Wk = np.asarray(Wk, dtype=np.float32)
    Wv = np.asarray(Wv, dtype=np.float32)
    idx = np.asarray(index_sample)

    def pack(m):
        # [ND*P, W] -> [P, ND*W]: row dc*128+p lands at columns dc*W..+W
        nd = m.shape[0] // P
        return np.ascontiguousarray(
            m.reshape(nd, P, m.shape[1]).transpose(1, 0, 2).reshape(P, -1)
        )

    def pack_jb(m):
        # [ND*P, NJ*512] -> [P, NJ*ND*512] (jb-major blocks)
        nd = m.shape[0] // P
        nj = m.shape[1] // 512
        return np.ascontiguousarray(
            m.reshape(nd, P, nj, 512).transpose(1, 2, 0, 3).reshape(P, -1)
        )

    A = (Wq.T.astype(np.float64) @ Wk.astype(np.float64)).astype(np.float32)
    Ah = A.astype(ml_dtypes.bfloat16)
    wvh, wvl = _split_bf16(np.ascontiguousarray(Wv.T))

    rows = np.arange(L)[:, None]
    maskb = np.zeros((L, L), dtype=ml_dtypes.bfloat16)
    maskb[rows, idx] = 1
    mcg = np.zeros((L, 2 * L), dtype=np.uint8)
    mcg[rows, idx] = 1
    np.add.at(mcg, (rows, L + idx), 1)

    perm16 = np.zeros((16, 8 * P), dtype=np.float16)
    for f in range(8):
        for p in range(16):
            perm16[p, f * P + p + 16 * f] = 1.0
    qidxf = (np.arange(P)[:, None] + 128 * np.arange(NL)[None, :]).astype(np.float32)
    crow = (1.2 + np.arange(NLAD, dtype=np.float32) * 0.134).reshape(1, NLAD)

    shared = {
        "Ahp": pack(Ah), "Afp": pack(A),
        "wvThp": pack(wvh), "wvTlp": pack(wvl),
        "maskb": maskb, "mcg": mcg, "perm16": perm16,
        "qidxf": qidxf, "crow": crow,
    }
    in_maps = []
    for b in range(B):
        xb = np.ascontiguousarray(x[b])
        xT = np.ascontiguousarray(xb.T)
        xth = xT.astype(ml_dtypes.bfloat16)
        xnh = xb.astype(ml_dtypes.bfloat16)
        xmean = xb.astype(np.float64).mean(axis=0).astype(np.float32)
        xmeh, xmel = _split_bf16(xmean.reshape(1, D))
        xm = np.concatenate(
            [xmeh.reshape(ND, P).T, xmel.reshape(ND, P).T], axis=1
        ).astype(ml_dtypes.bfloat16)
        in_maps.append(
            {
                "x_nat": xb,
                "xThp": pack_jb(xth),
                "xTfp": pack_jb(xT),
                "xNhp": pack(xnh),
                "xmp": np.ascontiguousarray(xm),
                **shared,
            }
        )
    return in_maps


def kernel(x, Wq, Wk, Wv, index_sample, _trace=False, _result_box=None):
    in_maps = _host_prep(x, Wq, Wk, Wv, index_sample)
    nc = _get_nc()
    res = run_bass_kernel_spmd(nc, in_maps, core_ids=list(range(B)), trace=_trace)
    if _result_box is not None:
        _result_box.append(res)
    out = np.stack([np.asarray(res.results[b]["ctx"]) for b in range(B)], axis=0)
    return out


# revision 72
# speedup vs baseline: 2.1092x; 1.0147x over previous
"""Sparse attention (ProbSparse-style) Trainium2 Bass kernel, v2.

Problem (per batch element b, data-parallel over 8 NeuronCores):
  Q = x @ Wq.T ; K = x @ Wk.T ; V = x @ Wv.T            [L=2048, D=512]
  QK_sample[l,s] = Q[l] . K[index_sample[l,s]]           [L, 40]
  M[l] = max_s QK_sample - sum_s QK_sample / L
  sel = top40(M)  (as a set; the reference scatter makes order irrelevant)
  scores = Q[sel] @ K.T / sqrt(D); attn = softmax(scores)
  ctx = broadcast(mean(V)); ctx[sel] = attn @ V

Key ideas vs v1 baseline:
  - A = Wq^T @ Wk precomputed on host: S = (x A) x^T. Kills the K and Q
    projections entirely; both approx and exact scores contract against
    the resident x^T tiles.
  - Approx M = masked max of bf16 S only (the sum/L term is <= ~0.5 and
    is absorbed by the candidate margin; validated: true top-40 rows sit
    within rank <= 40 of the approx ordering).
  - Threshold via a 64-step mu + c*sigma ladder with on-device counts
    (one 3d-broadcast compare + reduce + PE column-sum), picking the
    largest T with count >= 88 (fallback: smallest T with count <= 127).
    Replaces the 62us GPSIMD kth_largest.
  - Exact stage on <= 128 candidates: G = x_cand A (3-term bf16),
    S_cand = G x^T (3-term bf16)  ->  ~1e-4-class absolute error,
    validated 26x under the seed-0 top-40 boundary gap.
  - Softmax without max subtraction (|S*scale| <= ~9, exp is safe),
    upd = (attn @ x) @ Wv^T (kills the V projection; V never built).

kernel(**inputs) accepts FULL inputs, returns FULL [8, 2048, 512] f32;
batch is sharded over 8 cores.
"""

import math

import numpy as np
import ml_dtypes

import concourse.bacc as bacc
import concourse.bass as bass
import concourse.mybir as mybir
import concourse.tile as tile
from concourse.bass_utils import run_bass_kernel_spmd
from concourse.masks import make_identity
from concourse import library_config

P = 128
L = 2048
D = 512
B = 8
NL = L // P        # 16 query chunks
ND = D // P        # 4 feature chunks
NJ = L // 512      # 4 key blocks of 512
NT = 40
NLAD = 32          # threshold ladder steps
SCALE = 1.0 / math.sqrt(D)
NEG = -3.0e38
BIG = 3.0e38
SKIP_IDX = 99999.0  # scatter index sentinel (> bounds_check -> row skipped)

f32 = mybir.dt.float32
f32r = mybir.dt.float32r
f16 = mybir.dt.float16
bf16 = mybir.dt.bfloat16
u8 = mybir.dt.uint8
i32 = mybir.dt.int32
u32 = mybir.dt.uint32
AX = mybir.AxisListType
OP = mybir.AluOpType
ACTF = mybir.ActivationFunctionType


def build():
    nc = bacc.Bacc("TRN2", target_bir_lowering=False)

    # All big operands are host-packed into [128, wide] layouts so each
    # DMA partition line is a 4-16KB contiguous DRAM run (1KB lines were
    # descriptor-bound: ~26us of startup).
    x_d = nc.dram_tensor("x_nat", [L, D], f32, kind="ExternalInput")
    xth_d = nc.dram_tensor("xThp", [P, ND * L], bf16, kind="ExternalInput")
    xtf_d = nc.dram_tensor("xTfp", [P, ND * L], f32r, kind="ExternalInput")
    xnh_d = nc.dram_tensor("xNhp", [P, NL * D], bf16, kind="ExternalInput")
    ah_d = nc.dram_tensor("Ahp", [P, ND * D], bf16, kind="ExternalInput")
    af_d = nc.dram_tensor("Afp", [P, ND * D], f32, kind="ExternalInput")
    wvh_d = nc.dram_tensor("wvThp", [P, ND * D], bf16, kind="ExternalInput")
    wvl_d = nc.dram_tensor("wvTlp", [P, ND * D], bf16, kind="ExternalInput")
    xm_d = nc.dram_tensor("xmp", [P, 2 * ND], bf16, kind="ExternalInput")
    maskb_d = nc.dram_tensor("maskb", [L, L], bf16, kind="ExternalInput")
    mcg_d = nc.dram_tensor("mcg", [L, 2 * L], u8, kind="ExternalInput")
    perm_d = nc.dram_tensor("perm16", [16, 8 * P], f16, kind="ExternalInput")
    qidx_d = nc.dram_tensor("qidxf", [P, NL], f32, kind="ExternalInput")
    crow_d = nc.dram_tensor("crow", [1, NLAD], f32, kind="ExternalInput")
    ctx_d = nc.dram_tensor("ctx", [L, D], f32, kind="ExternalOutput")

    with tile.TileContext(nc) as tc:
        with (
            tc.tile_pool(name="const", bufs=1) as cst,
            tc.tile_pool(name="xres", bufs=1) as xres,      # resident x / A / Wv
            tc.tile_pool(name="proj", bufs=1) as proj,      # QATb
            tc.tile_pool(name="mstuff", bufs=1) as mst,     # M / threshold smalls
            tc.tile_pool(name="mstream", bufs=3) as mstr,   # mask chunks
            tc.tile_pool(name="scr", bufs=3) as scr,        # TTR scratch
            tc.tile_pool(name="cand", bufs=1) as cnd,       # exact-stage tiles
            tc.tile_pool(name="expp", bufs=1) as expp,      # softmax/upd tiles
            tc.tile_pool(name="ps", bufs=2, space="PSUM") as ps,
            tc.tile_pool(name="psb", bufs=2, space="PSUM") as psb,    # bf16 transposes
            tc.tile_pool(name="ps_s", bufs=2, space="PSUM") as ps_s,  # S pairs
        ):
            # ---------------- constants ----------------
            ident = cst.tile([P, P], f32, tag="ident")
            make_identity(nc, ident[:])
            ident_b = cst.tile([P, P], bf16, tag="ident_b")
            nc.vector.tensor_copy(ident_b[:], ident[:])
            # preload the sparse_gather ucode so the serial tail does not
            # pay the library switch
            nc.gpsimd.load_library(library_config.sparse_gather)
            ones_r1 = cst.tile([1, P], f32, tag="ones_r1")
            nc.vector.memset(ones_r1[:], 1.0)
            ones_cf = cst.tile([P, 1], f32, tag="ones_cf")
            nc.vector.memset(ones_cf[:], 1.0)
            negbig = cst.tile([P, 1], f32, tag="negbig")
            nc.vector.memset(negbig[:], NEG)
            big9 = cst.tile([P, 1], f32, tag="big9")
            nc.vector.memset(big9[:], SKIP_IDX)
            qidx_f = cst.tile([P, NL], f32, tag="qidx_f")
            nc.sync.dma_start(qidx_f[:], qidx_d[:])
            crow = cst.tile([1, NLAD], f32, tag="crow")
            nc.sync.dma_start(crow[:], crow_d[:])
            perm16 = cst.tile([16, 8 * P], f16, tag="perm16")
            nc.sync.dma_start(perm16[:], perm_d[:])

            # ---------------- critical loads (packed, big lines) --------
            # Only Ahp + xThp gate the first matmuls; everything the tail
            # needs is DMA'd from inside the phase-2 loop so it doesn't
            # compete for startup bandwidth.
            Ahp = xres.tile([P, ND * D], bf16, tag="Ahp")
            nc.sync.dma_start(Ahp[:], ah_d[:])
            # x^T hi is packed jb-major: block jb holds [dc=0..3][512 cols]
            # so the first QA matmuls start after Ahp + one 0.5MB block
            xThp = xres.tile([P, ND * L], bf16, tag="xThp")
            for jb in range(NJ):
                nc.sync.dma_start(
                    xThp[:, jb * 2048 : (jb + 1) * 2048],
                    xth_d[:, jb * 2048 : (jb + 1) * 2048],
                )
            Afp = xres.tile([P, ND * D], f32, tag="Afp")
            wvhp = xres.tile([P, ND * D], bf16, tag="wvhp")
            wvlp = xres.tile([P, ND * D], bf16, tag="wvlp")
            xmp = xres.tile([P, 2 * ND], bf16, tag="xmp")
            xTfp = xres.tile([P, ND * L], f32r, tag="xTfp")
            xNhp = xres.tile([P, NL * D], bf16, tag="xNhp")

            # slice helpers over the packed tiles
            Ah = lambda dc, js: Ahp[:, dc * D + js.start : dc * D + js.stop]
            Af = lambda dc, js: Afp[:, dc * D + js.start : dc * D + js.stop]
            wvh = lambda dc, js: wvhp[:, dc * D + js.start : dc * D + js.stop]
            wvl = lambda dc, js: wvlp[:, dc * D + js.start : dc * D + js.stop]

            def _xt(tile_, dc, js):
                # jb-major packing: block jb*2048 + dc*512
                jb, r = divmod(js.start, 512)
                assert js.stop - js.start == 512 and r == 0
                off = jb * 2048 + dc * 512
                return tile_[:, off : off + 512]

            xTh = lambda dc, js: _xt(xThp, dc, js)
            xTf = lambda dc, js: _xt(xTfp, dc, js)
            xNh = lambda jc: xNhp[:, jc * D : (jc + 1) * D]
            SD = slice(0, D)
            SL = slice(0, L)

            # ---------------- phase 1: QA^T = A^T x^T (bf16) ------------
            QATb = [proj.tile([P, L], bf16, tag=f"QATb{ic}", name=f"QATb{ic}") for ic in range(ND)]
            for jb in range(NJ):
                jsl = slice(jb * 512, (jb + 1) * 512)
                for ic in range(ND):
                    isl = slice(ic * P, (ic + 1) * P)
                    pq = ps.tile([P, 512], f32, tag="blk")
                    for dc in range(ND):
                        nc.tensor.matmul(
                            pq[:], Ah(dc, isl), xTh(dc, jsl),
                            start=(dc == 0), stop=(dc == ND - 1),
                        )
                    nc.scalar.copy(QATb[ic][:, jsl], pq[:])

            # ---------------- phase 2: approx S + masked max ------------
            # Per 128-query chunk: PE computes 4 S blocks; ScalarE evicts
            # them to a bf16 row [P, 2048]; DVE does one 2x bf16 mask-mult
            # + one wide reduce_max.  (tensor_tensor_reduce crashes TRN2
            # hardware, so the fused form is not available.)
            M_all = mst.tile([P, NL], f32, tag="M_all")
            M_lo = mst.tile([P, 8], f32, tag="M_lo")
            Trow = mst.tile([1, NLAD], f32, tag="Trow")
            Tb = mst.tile([P, NLAD], bf16, tag="Tb")
            for lc in range(NL):
                lsl = slice(lc * P, (lc + 1) * P)
                mkb = mstr.tile([P, L], bf16, tag="mkb")
                nc.sync.dma_start(mkb[:], maskb_d[lsl, :])
                # tail-only loads trickled in behind the mask stream
                if lc == 0:
                    nc.sync.dma_start(wvhp[:], wvh_d[:])
                    nc.sync.dma_start(wvlp[:], wvl_d[:])
                    nc.sync.dma_start(xmp[:], xm_d[:])
                elif lc == 2:
                    nc.sync.dma_start(Afp[:], af_d[:])
                elif lc == 5:
                    nc.sync.dma_start(xTfp[:], xtf_d[:])
                elif lc == 9:
                    nc.sync.dma_start(xNhp[:], xnh_d[:])
                sb1 = scr.tile([P, L], bf16, tag="sb1")
                for jp in range(2):
                    # paired PSUM banks -> one wide eviction per 1024 cols
                    pss = ps_s.tile([P, 1024], f32, tag="psSc", name="pssa")
                    for jh in range(2):
                        jb = jp * 2 + jh
                        jsl = slice(jb * 512, (jb + 1) * 512)
                        for ic in range(ND):
                            nc.tensor.matmul(
                                pss[:, jh * 512 : (jh + 1) * 512],
                                QATb[ic][:, lsl], xTh(ic, jsl),
                                start=(ic == 0), stop=(ic == ND - 1),
                            )
                    nc.scalar.copy(
                        sb1[:, jp * 1024 : (jp + 1) * 1024], pss[:]
                    )
                # masked max: two 2x-mode masked products, one 2x max
                # combine, then a half-width 1x reduce
                t0 = scr.tile([P, 1024], bf16, tag="t0m")
                nc.vector.tensor_tensor(
                    out=t0[:], in0=sb1[:, 0:1024], in1=mkb[:, 0:1024],
                    op=OP.mult,
                )
                t1 = scr.tile([P, 1024], bf16, tag="t1m")
                nc.vector.tensor_tensor(
                    out=t1[:], in0=sb1[:, 1024:2048], in1=mkb[:, 1024:2048],
                    op=OP.mult,
                )
                t2 = scr.tile([P, 1024], bf16, tag="t2m")
                nc.vector.tensor_tensor(
                    out=t2[:], in0=t0[:], in1=t1[:], op=OP.max
                )
                if lc < 8:
                    nc.vector.reduce_max(M_lo[:, lc : lc + 1], t2[:], axis=AX.X)
                    nc.vector.tensor_copy(
                        M_all[:, lc : lc + 1], M_lo[:, lc : lc + 1]
                    )
                else:
                    nc.vector.reduce_max(M_all[:, lc : lc + 1], t2[:], axis=AX.X)
                if lc == 7:
                    # ---- early threshold stats on the first 1024 rows --
                    # (mu/sigma only steer the ladder range; the counts
                    # below verify against the full M) -- this whole chain
                    # runs under the second half of the main phase.
                    stats2 = mst.tile([P, 2], f32, tag="stats2")
                    msq = mst.tile([P, 8], f32, tag="msq")
                    nc.vector.scalar_tensor_tensor(
                        out=msq[:], in0=M_lo[:], scalar=1.0, in1=M_lo[:],
                        op0=OP.mult, op1=OP.mult,
                        accum_out=stats2[:, 1:2],
                    )
                    nc.vector.tensor_reduce(
                        stats2[:, 0:1], M_lo[:], axis=AX.X, op=OP.add
                    )
                    pst = ps.tile([1, 2], f32, tag="blk")
                    nc.tensor.matmul(
                        pst[:1, :2], ones_cf[:], stats2[:], start=True, stop=True
                    )
                    srow = mst.tile([1, 2], f32, tag="srow")
                    nc.vector.tensor_copy(srow[:], pst[:1, :2])
                    musig = mst.tile([1, 2], f32, tag="musig")
                    nc.vector.tensor_scalar_mul(musig[:], srow[:], 1.0 / 1024.0)
                    mu = musig[:, 0:1]
                    mu2 = mst.tile([1, 1], f32, tag="mu2")
                    nc.vector.tensor_tensor(out=mu2[:], in0=mu, in1=mu, op=OP.mult)
                    var = mst.tile([1, 1], f32, tag="var")
                    nc.vector.tensor_tensor(
                        out=var[:], in0=musig[:, 1:2], in1=mu2[:], op=OP.subtract
                    )
                    sigma = mst.tile([1, 1], f32, tag="sigma")
                    nc.scalar.sqrt(sigma[:], var[:])
                    # dummy exp: pull the Exp act-table load off the tail's
                    # critical path (table switch costs ~1.3us)
                    expd = mst.tile([1, 1], f32, tag="expd")
                    nc.scalar.activation(
                        out=expd[:], in_=var[:], func=ACTF.Exp,
                        bias=0.0, scale=0.0,
                    )
                    nc.vector.tensor_tensor(
                        out=Trow[:], in0=crow[:],
                        in1=sigma[:].to_broadcast([1, NLAD]), op=OP.mult,
                    )
                    nc.vector.tensor_tensor(
                        out=Trow[:], in0=Trow[:], in1=mu.to_broadcast([1, NLAD]),
                        op=OP.add,
                    )
                    ptb = ps.tile([P, NLAD], f32, tag="blk")
                    nc.tensor.matmul(
                        ptb[:P, :NLAD], ones_r1[:], Trow[:], start=True, stop=True
                    )
                    nc.vector.tensor_copy(Tb[:], ptb[:P, :NLAD])

            # ---------------- phase 3: ladder counts --------------------
            M_b = mst.tile([P, NL], bf16, tag="M_b")
            nc.vector.tensor_copy(M_b[:], M_all[:])
            # cmp[p, j, f] = M[p, f] >= T[j]  (bf16 in/out -> 2x DVE, and
            # PE can column-sum the bf16 result)
            cmpb = mst.tile([P, NLAD * NL], bf16, tag="cmpb")
            nc.vector.tensor_tensor(
                out=cmpb[:].rearrange("p (j f) -> p j f", f=NL),
                in0=M_b[:].rearrange("p (o f) -> p o f", o=1).to_broadcast([P, NLAD, NL]),
                in1=Tb[:].rearrange("p (j o) -> p j o", o=1).to_broadcast([P, NLAD, NL]),
                op=OP.is_ge,
            )
            # ---------------- Vmean -> ctx init (PE idle slot) ----------
            pvm = ps.tile([1, D], f32, tag="blk")
            n = 0
            for dc in range(ND):
                for lh, rh in (
                    (xmp[:, dc : dc + 1], wvh(dc, SD)),
                    (xmp[:, ND + dc : ND + dc + 1], wvh(dc, SD)),
                    (xmp[:, dc : dc + 1], wvl(dc, SD)),
                ):
                    nc.tensor.matmul(
                        pvm[:1, :], lh, rh,
                        start=(n == 0), stop=(n == 3 * ND - 1),
                    )
                    n += 1
            vmean = mst.tile([1, D], f32, tag="vmean")
            nc.scalar.copy(vmean[:], pvm[:1, :])
            pvb = ps.tile([P, D], f32, tag="blk")
            nc.tensor.matmul(pvb[:], ones_r1[:], vmean[:], start=True, stop=True)
            vmean_bc = mst.tile([P, D], f32, tag="vmean_bc")
            nc.vector.tensor_copy(vmean_bc[:], pvb[:])
            for jc in range(NL):
                nc.sync.dma_start(ctx_d[jc * P : (jc + 1) * P, :], vmean_bc[:])

            cnt01 = mst.tile([P, NLAD], f32, tag="cnt01")
            nc.vector.tensor_reduce(
                cnt01[:], cmpb[:].rearrange("p (j f) -> p j f", f=NL),
                axis=AX.X, op=OP.add,
            )
            pcc = ps.tile([1, NLAD], f32, tag="blk")
            nc.tensor.matmul(pcc[:1, :NLAD], ones_cf[:], cnt01[:], start=True, stop=True)
            cntrow = mst.tile([1, NLAD], f32, tag="cntrow")
            nc.vector.tensor_copy(cntrow[:], pcc[:1, :NLAD])
            # largest T with count >= 88; fallback smallest T with count <= 127
            okm = mst.tile([1, NLAD], u8, tag="okm")
            nc.vector.tensor_scalar(
                okm[:], cntrow[:], 87.5, None, op0=OP.is_ge
            )
            negrow = mst.tile([1, NLAD], f32, tag="negrow")
            nc.vector.memset(negrow[:], NEG)
            bigrow = mst.tile([1, NLAD], f32, tag="bigrow")
            nc.vector.memset(bigrow[:], BIG)
            tsel = mst.tile([1, NLAD], f32, tag="tsel")
            nc.vector.select(tsel[:], okm[:], Trow[:], negrow[:])
            tstar = mst.tile([1, 1], f32, tag="tstar")
            nc.vector.reduce_max(tstar[:], tsel[:], axis=AX.X)
            ok2 = mst.tile([1, NLAD], u8, tag="ok2")
            nc.vector.tensor_scalar(
                ok2[:], cntrow[:], 127.5, None, op0=OP.is_le
            )
            tsel2 = mst.tile([1, NLAD], f32, tag="tsel2")
            nc.vector.select(tsel2[:], ok2[:], Trow[:], bigrow[:])
            tfb = mst.tile([1, 1], f32, tag="tfb")
            nc.vector.tensor_reduce(tfb[:], tsel2[:], axis=AX.X, op=OP.min)
            have = mst.tile([1, 1], u8, tag="have")
            nc.vector.tensor_scalar(
                have[:], tstar[:], -1.0e30, None, op0=OP.is_ge
            )
            tfin = mst.tile([1, 1], f32, tag="tfin")
            nc.vector.select(tfin[:], have[:], tstar[:], tfb[:])
            ptf = ps.tile([P, 1], f32, tag="blk")
            nc.tensor.matmul(ptf[:P, :1], ones_r1[:], tfin[:], start=True, stop=True)
            tbc = mst.tile([P, 1], f32, tag="tbc")
            nc.vector.tensor_copy(tbc[:], ptf[:P, :1])

            # selmask / candidate index compaction
            selmask = mst.tile([P, NL], u8, tag="selmask")
            nc.vector.tensor_scalar(
                selmask[:], M_all[:], tbc[:], 0.0,
                op0=OP.subtract, op1=OP.is_ge,
            )
            midx = mst.tile([P, NL], f32, tag="midx")
            nc.vector.memset(midx[:], -1.0)
            nc.vector.copy_predicated(midx[:], selmask[:], qidx_f[:])
            pwr = ps.tile([16, P], f32, tag="blk", name="pwr")
            nc.tensor.transpose(pwr[:16, :P], midx[:], ident[:])
            # mini keep-warm bridging the sparse_gather window
            midx_b = mst.tile([P, NL], bf16, tag="midx_b")
            nc.vector.tensor_copy(midx_b[:], midx[:])
            pwarm0 = ps.tile([16, 512], f32, tag="blk", name="pwarm0")
            for w in range(8):
                nc.tensor.matmul(
                    pwarm0[:16, :512], midx_b[:], xThp[:, 0:512],
                    start=True, stop=True,
                )
            wrap_in = mst.tile([16, P], f32, tag="wrap_in")
            nc.vector.tensor_copy(wrap_in[:], pwr[:16, :P])
            spg = mst.tile([16, 8], f32, tag="spg")
            nfound = mst.tile([1, 1], u32, tag="nfound")
            nc.gpsimd.sparse_gather(out=spg[:], in_=wrap_in[:], num_found=nfound[:])
            spg_cl = mst.tile([16, 8], f32, tag="spg_cl")
            nc.vector.tensor_scalar(
                spg_cl[:], spg[:], 0.0, float(L - 1), op0=OP.max, op1=OP.min
            )
            # fp16 keeps indices <= 2047 exact and avoids the fp32 double
            # LDWEIGHTS cost of the one-hot unwrap
            spg_h = mst.tile([16, 8], f16, tag="spg_h")
            nc.vector.tensor_copy(spg_h[:], spg_cl[:])
            pcq = ps.tile([P, 1], f32, tag="blk", name="pcq")
            for f in range(8):
                nc.tensor.matmul(
                    pcq[:P, :1], perm16[:, f * P : (f + 1) * P],
                    spg_h[:, f : f + 1],
                    start=(f == 0), stop=(f == 7),
                )
            candq_f = mst.tile([P, 1], f32, tag="candq_f")
            nc.vector.tensor_copy(candq_f[:], pcq[:P, :1])
            candq_i = mst.tile([P, 1], i32, tag="candq_i")
            nc.vector.tensor_copy(candq_i[:], pcq[:P, :1])
            nf_f = mst.tile([1, 1], f32, tag="nf_f")
            nc.vector.tensor_copy(nf_f[:], nfound[:])
            pnb = ps.tile([P, 1], f32, tag="blk")
            nc.tensor.matmul(pnb[:P, :1], ones_r1[:], nf_f[:], start=True, stop=True)
            nbc = mst.tile([P, 1], f32, tag="nbc")
            nc.vector.tensor_copy(nbc[:], pnb[:P, :1])
            invalid = mst.tile([P, 1], u8, tag="invalid")
            nc.vector.tensor_tensor(
                out=invalid[:], in0=qidx_f[:, 0:1], in1=nbc[:], op=OP.is_ge
            )

            # Keep-warm: ~3.5us of throwaway matmuls gated on candq_h so
            # they run exactly during the gather window; a >3.4us PE idle
            # here would drop the HAM clock to 1.2GHz for the whole exact
            # stage.
            candq_h = mst.tile([P, 1], bf16, tag="candq_h")
            nc.vector.tensor_copy(candq_h[:], pcq[:P, :1])
            pwarm = ps.tile([1, 512], f32, tag="blk", name="pwarm")
            for w in range(14):
                nc.tensor.matmul(
                    pwarm[:1, :512], candq_h[:, :1], xThp[:, 0:512],
                    start=True, stop=True,
                )

            # ---------------- phase 4: exact stage ----------------------
            x_cand = cnd.tile([P, D], f32, tag="x_cand")
            nc.gpsimd.indirect_dma_start(
                out=x_cand[:], out_offset=None, in_=x_d[:],
                in_offset=bass.IndirectOffsetOnAxis(ap=candq_i[:, :1], axis=0),
            )
            # combined mask++count row gather (one SWDGE pass)
            gmc = cnd.tile([P, 2 * L], u8, tag="gmc")
            nc.gpsimd.indirect_dma_start(
                out=gmc[:], out_offset=None, in_=mcg_d[:],
                in_offset=bass.IndirectOffsetOnAxis(ap=candq_i[:, :1], axis=0),
            )

            # x_cand^T (fp32 — exact G via fp32 matmul, no hi/lo casts)
            xcT = [cnd.tile([P, P], f32, tag=f"xcT{dc}", name=f"xcT{dc}") for dc in range(ND)]
            for dc in range(ND):
                pxc = ps.tile([P, P], f32, tag="blk")
                nc.tensor.transpose(
                    pxc[:P, :P], x_cand[:, dc * P : (dc + 1) * P], ident[:]
                )
                nc.vector.tensor_copy(xcT[dc][:], pxc[:P, :P])

            # G^T computed directly: GT[dout, cand] = sum_din A[din, dout]^T
            # x_cand^T[din, cand] — 16 fp32 N=128 matmuls, no gsb round-trip
            GT = [cnd.tile([P, P], f32r, tag=f"GT{dc}", name=f"GT{dc}") for dc in range(ND)]
            for do in range(ND):
                osl = slice(do * P, (do + 1) * P)
                pgt = ps.tile([P, P], f32, tag="blk")
                for di in range(ND):
                    nc.tensor.matmul(
                        pgt[:P, :P], Af(di, osl), xcT[di][:],
                        start=(di == 0), stop=(di == ND - 1),
                    )
                nc.vector.tensor_copy(GT[do][:], pgt[:P, :P])

            # S_cand = G @ x^T in fp32r (full-rate fp32-class matmul),
            # 2 held [P,1024] PSUM pairs
            psS = []
            cmax = cnd.tile([P, 2], f32, tag="cmax")
            csum = cnd.tile([P, 2], f32, tag="csum")
            for jp in range(2):
                pss2 = ps_s.tile([P, 1024], f32, tag="psSc")
                psS.append(pss2)
                for jh in range(2):
                    jb = jp * 2 + jh
                    jsl = slice(jb * 512, (jb + 1) * 512)
                    for dc in range(ND):
                        nc.tensor.matmul(
                            pss2[:, jh * 512 : (jh + 1) * 512],
                            GT[dc][:], xTf(dc, jsl),
                            start=(dc == 0), stop=(dc == ND - 1),
                        )

            # ---------------- phase 5: softmax + update -----------------
            exp_sb = expp.tile([P, L], bf16, tag="exp_sb")
            sume4 = expp.tile([P, 2], f32, tag="sume4")
            for jp in range(2):
                psl = slice(jp * 1024, (jp + 1) * 1024)
                nc.scalar.activation(
                    out=exp_sb[:, psl], in_=psS[jp][:], func=ACTF.Exp,
                    bias=0.0, scale=SCALE,
                    accum_out=sume4[:, jp : jp + 1],
                )
            sume = expp.tile([P, 1], f32, tag="sume")
            nc.vector.reduce_sum(sume[:], sume4[:], axis=AX.X)
            recip = expp.tile([P, 1], f32, tag="recip")
            nc.vector.reciprocal(recip[:], sume[:])

            # ---- exact M (DVE-only; runs concurrently with the PE's
            # expT/G2 pipeline below — its transpose copies live on ACT) --
            for jp in range(2):
                pss2 = psS[jp]
                psl = slice(jp * 1024, (jp + 1) * 1024)
                s3 = scr.tile([P, 1024], f32, tag="scrt2")
                nc.vector.tensor_tensor(
                    out=s3[:], in0=pss2[:], in1=gmc[:, psl], op=OP.mult
                )
                nc.vector.reduce_max(cmax[:, jp : jp + 1], s3[:], axis=AX.X)
                s4 = scr.tile([P, 1024], f32, tag="scrt2")
                nc.vector.scalar_tensor_tensor(
                    out=s4[:], in0=pss2[:], scalar=-1.0 / L,
                    in1=gmc[:, L + psl.start : L + psl.stop],
                    op0=OP.mult, op1=OP.mult,
                    accum_out=csum[:, jp : jp + 1],
                )
            u1 = cnd.tile([P, 1], f32, tag="u1")
            u2 = cnd.tile([P, 1], f32, tag="u2")
            M_cand = cnd.tile([P, 1], f32, tag="M_cand")
            nc.vector.reduce_max(u1[:], cmax[:], axis=AX.X)
            nc.vector.reduce_sum(u2[:], csum[:], axis=AX.X)
            nc.vector.tensor_tensor(out=M_cand[:], in0=u1[:], in1=u2[:], op=OP.add)
            nc.vector.copy_predicated(M_cand[:], invalid[:], negbig[:])

            # expT transposes software-pipelined with the G2 accumulation
            # (depth 4) so the PE never idles long enough to re-throttle
            expT = [expp.tile([P, P], bf16, tag=f"expT{jc}", name=f"expT{jc}") for jc in range(NL)]
            pu = ps.tile([P, D], f32, tag="blk")

            def g2_mm(jc):
                nc.tensor.matmul(
                    pu[:], expT[jc][:], xNh(jc),
                    start=(jc == 0), stop=(jc == NL - 1),
                    skip_group_check=True,
                )

            for jc in range(NL):
                pet = psb.tile([P, P], bf16, tag="blkb")
                nc.tensor.transpose(
                    pet[:P, :P], exp_sb[:, jc * P : (jc + 1) * P], ident_b[:]
                )
                nc.scalar.copy(expT[jc][:], pet[:P, :P])
                if jc >= 3:
                    g2_mm(jc - 3)
            for jc in range(NL - 3, NL):
                g2_mm(jc)

            # top-40 scan starts here so the max8 chain (DVE) overlaps the
            # G2T/upd section below
            pmc = ps.tile([1, P], f32, tag="blk")
            nc.tensor.transpose(pmc[:1, :P], M_cand[:], ident[:])
            mcT = cnd.tile([1, P], f32, tag="mcT")
            nc.vector.tensor_copy(mcT[:], pmc[:1, :P])
            etop = cnd.tile([1, NT], f32, tag="etop")
            for r in range(5):
                nc.vector.max(out=etop[:, 8 * r : 8 * r + 8], in_=mcT[:])
                if r < 4:
                    nc.vector.match_replace(
                        out=mcT[:], in_to_replace=etop[:, 8 * r : 8 * r + 8],
                        in_values=mcT[:], imm_value=NEG,
                    )

            g2b = expp.tile([P, D], bf16, tag="g2b")
            nc.scalar.copy(g2b[:], pu[:])
            G2T = [expp.tile([P, P], bf16, tag=f"G2T{dc}", name=f"G2T{dc}") for dc in range(ND)]
            for dc in range(ND):
                pg2 = psb.tile([P, P], bf16, tag="blkb")
                nc.tensor.transpose(
                    pg2[:P, :P], g2b[:, dc * P : (dc + 1) * P], ident_b[:]
                )
                nc.scalar.copy(G2T[dc][:], pg2[:P, :P])
            # upd = G2 @ Wv^T / sums
            pup = ps.tile([P, D], f32, tag="blk")
            for dc in range(ND):
                nc.tensor.matmul(
                    pup[:], G2T[dc][:], wvh(dc, SD),
                    start=(dc == 0), stop=(dc == ND - 1),
                )
            upd = expp.tile([P, D], f32, tag="upd")
            nc.scalar.activation(
                out=upd[:], in_=pup[:], func=ACTF.Copy, bias=0.0, scale=recip[:]
            )

            # scatter-index selection
            pte = ps.tile([P, 1], f32, tag="blk")
            nc.tensor.matmul(
                pte[:P, :1], ones_r1[:], etop[:, NT - 1 : NT], start=True, stop=True
            )
            tebc = cnd.tile([P, 1], f32, tag="tebc")
            nc.vector.tensor_copy(tebc[:], pte[:P, :1])
            sel2 = cnd.tile([P, 1], u8, tag="sel2")
            nc.vector.tensor_tensor(
                out=sel2[:], in0=M_cand[:], in1=tebc[:], op=OP.is_ge
            )
            scat_f = cnd.tile([P, 1], f32, tag="scat_f")
            nc.vector.tensor_copy(scat_f[:], big9[:])
            nc.vector.copy_predicated(scat_f[:], sel2[:], candq_f[:])
            scat_i = cnd.tile([P, 1], i32, tag="scat_i")
            nc.vector.tensor_copy(scat_i[:], scat_f[:])
            nc.gpsimd.indirect_dma_start(
                out=ctx_d[:],
                out_offset=bass.IndirectOffsetOnAxis(ap=scat_i[:, :1], axis=0),
                in_=upd[:], in_offset=None,
                bounds_check=L - 1, oob_is_err=False,
            )

    nc.compile()
    return nc


_NC = None


def _get_nc():
    global _NC
    if _NC is None:
        _NC = build()
    return _NC


def _split_bf16(a):
    hi = a.astype(ml_dtypes.bfloat16)
    lo = (a - hi.astype(np.float32)).astype(ml_dtypes.bfloat16)
    return hi, lo


def _host_prep(x, Wq, Wk, Wv, index_sample):
    x = np.asarray(x, dtype=np.float32)
    Wq = np.asarray(Wq, dtype=np.float32)
    Wk = np.asarray(Wk, dtype=np.float32)
    Wv = np.asarray(Wv, dtype=np.float32)
    idx = np.asarray(index_sample)

    def pack(m):
        # [ND*P, W] -> [P, ND*W]: row dc*128+p lands at columns dc*W..+W
        nd = m.shape[0] // P
        return np.ascontiguousarray(
            m.reshape(nd, P, m.shape[1]).transpose(1, 0, 2).reshape(P, -1)
        )

    def pack_jb(m):
        # [ND*P, NJ*512] -> [P, NJ*ND*512] (jb-major blocks)
        nd = m.shape[0] // P
        nj = m.shape[1] // 512
        return np.ascontiguousarray(
            m.reshape(nd, P, nj, 512).transpose(1, 2, 0, 3).reshape(P, -1)
        )

    A = (Wq.T.astype(np.float64) @ Wk.astype(np.float64)).astype(np.float32)
    Ah = A.astype(ml_dtypes.bfloat16)
    wvh, wvl = _split_bf16(np.ascontiguousarray(Wv.T))

    rows = np.arange(L)[:, None]
    maskb = np.zeros((L, L), dtype=ml_dtypes.bfloat16)
    maskb[rows, idx] = 1
    mcg = np.zeros((L, 2 * L), dtype=np.uint8)
    mcg[rows, idx] = 1
    np.add.at(mcg, (rows, L + idx), 1)

    perm16 = np.zeros((16, 8 * P), dtype=np.float16)
    for f in range(8):
        for p in range(16):
            perm16[p, f * P + p + 16 * f] = 1.0
    qidxf = (np.arange(P)[:, None] + 128 * np.arange(NL)[None, :]).astype(np.float32)
    crow = (1.2 + np.arange(NLAD, dtype=np.float32) * 0.134).reshape(1, NLAD)

    shared = {
        "Ahp": pack(Ah), "Afp": pack(A),
        "wvThp": pack(wvh), "wvTlp": pack(wvl),
        "maskb": maskb, "mcg": mcg, "perm16": perm16,
        "qidxf": qidxf, "crow": crow,
    }
    in_maps = []
    for b in range(B):
        xb = np.ascontiguousarray(x[b])
        xT = np.ascontiguousarray(xb.T)
        xth = xT.astype(ml_dtypes.bfloat16)
        xnh = xb.astype(ml_dtypes.bfloat16)
        xmean = xb.astype(np.float64).mean(axis=0).astype(np.float32)
        xmeh, xmel = _split_bf16(xmean.reshape(1, D))
        xm = np.concatenate(
            [xmeh.reshape(ND, P).T, xmel.reshape(ND, P).T], axis=1
        ).astype(ml_dtypes.bfloat16)
        in_maps.append(
            {
                "x_nat": xb,
                "xThp": pack_jb(xth),
                "xTfp": pack_jb(xT),
                "xNhp": pack(xnh),
                "xmp": np.ascontiguousarray(xm),
                **shared,
            }
        )
    return in_maps


def kernel(x, Wq, Wk, Wv, index_sample, _trace=False, _result_box=None):
    in_maps = _host_prep(x, Wq, Wk, Wv, index_sample)
    nc = _get_nc()
    res = run_bass_kernel_spmd(nc, in_maps, core_ids=list(range(B)), trace=_trace)
    if _result_box is not None:
        _result_box.append(res)
    out = np.stack([np.asarray(res.results[b]["ctx"]) for b in range(B)], axis=0)
    return out
